# revision 1
# baseline (speedup 1.0000x reference)
"""Trainium2 Bass kernel for nn_BilinearLinformerCapsuleFC.

Strategy: data-parallel over batch (32 -> 4 per core x 8 cores).
Per core, per batch item:
  conv1/convq as block-diagonal grouped-conv matmuls (9 shifted taps,
  PSUM-accumulated), Linformer key projection per capsule, keys/queries
  transposed on the PE, then column-softmax attention computed in
  S^T [keys, queries] layout: row-group-packed K@Q^T matmuls -> one big
  ACT exp with accum_out row-sum (the softmax-over-queries normalizer)
  -> K' = K/Z -> col-group-packed K'^T @ E matmuls accumulating Out^T in
  a single PSUM bank. Output conv + LayerNorm (rsqrt via exp(-0.5 ln))
  with DRAM-roundtrip relayouts for the 16<->49 flat reinterpretations.
"""
import numpy as np
import ml_dtypes

import concourse.bass as bass
import concourse.mybir as mybir
import concourse.tile as tile
from concourse import bacc
from concourse.bass import ds
from concourse.bass_utils import run_bass_kernel_spmd

BF16 = mybir.dt.bfloat16
F32 = mybir.dt.float32
AF = mybir.ActivationFunctionType
ALU = mybir.AluOpType

B, IN_N, IN_D, H, OUT_N, OUT_D, HO, HID = 32, 32, 16, 14, 32, 16, 7, 64
C = IN_N * IN_D            # 512
NB = 4                     # batch items per core
NKEY = IN_N * HID + HO * HO  # 2097
J = OUT_N * HO * HO        # 1568
NT_I = 17                  # i tiles (16x128 + 49)
NT_J = 13                  # j tiles (12x128 + 32)
JPAD = NT_J * 2048         # 26624, padded flat len per batch
EPS = 1e-5
CHUNKS = [(0, 512), (512, 512), (1024, 512), (1536, 32)]  # j chunks
SCALE = IN_D ** -0.5       # 0.25

_PROG = None


def _rows_i(it):
    return 128 if it < 16 else NKEY - 16 * 128  # 49


def _rows_j(jt):
    return 128 if jt < 12 else J - 12 * 128  # 32


def _build():
    nc = bacc.Bacc("TRN2", target_bir_lowering=False, debug=False, num_devices=1)

    cp_img = nc.dram_tensor("cp_img", [NB, C, 16, 16], BF16, kind="ExternalInput")
    qp_img = nc.dram_tensor("qp_img", [NB, C, 9, 9], BF16, kind="ExternalInput")
    w1bd = nc.dram_tensor("w1bd", [9, 4, 128, 128], BF16, kind="ExternalInput")
    wqbd = nc.dram_tensor("wqbd", [9, 4, 128, 128], BF16, kind="ExternalInput")
    ep = nc.dram_tensor("ep", [IN_N, 196, HID], BF16, kind="ExternalInput")
    rel_k = nc.dram_tensor("rel_k", [49, 16], F32, kind="ExternalInput")
    rel_kt = nc.dram_tensor("rel_kt", [16, 49], BF16, kind="ExternalInput")
    gam_d = nc.dram_tensor("gam", [16], F32, kind="ExternalInput")
    bet_d = nc.dram_tensor("bet", [16], F32, kind="ExternalInput")
    ident_d = nc.dram_tensor("ident", [128, 128], BF16, kind="ExternalInput")
    bident_d = nc.dram_tensor("bident", [128, 16], BF16, kind="ExternalInput")

    fq = nc.dram_tensor("fq", [NB, JPAD], BF16)
    fo = nc.dram_tensor("fo", [NB, JPAD], BF16)
    f3 = nc.dram_tensor("f3", [NB, JPAD], F32)
    outy = nc.dram_tensor("outy", [NB, JPAD], F32, kind="ExternalOutput")

    def dmae(i):
        return nc.sync

    with tile.TileContext(nc) as tc:
        from contextlib import ExitStack
        with ExitStack() as ctx:
            consts = ctx.enter_context(tc.tile_pool(name="consts", bufs=1))
            big = ctx.enter_context(tc.tile_pool(name="big", bufs=1))
            perb = ctx.enter_context(tc.tile_pool(name="perb", bufs=2))
            ebuf = ctx.enter_context(tc.tile_pool(name="ebuf", bufs=2))
            small = ctx.enter_context(tc.tile_pool(name="small", bufs=3))
            pp_s = ctx.enter_context(tc.tile_pool(name="pp_s", bufs=1, space="PSUM"))
            pp_o = ctx.enter_context(tc.tile_pool(name="pp_o", bufs=2, space="PSUM"))
            pp_sm = ctx.enter_context(tc.tile_pool(name="pp_sm", bufs=2, space="PSUM"))

            # ---- constants ----
            ident = consts.tile([128, 128], BF16)
            nc.sync.dma_start(out=ident, in_=ident_d[:, :])
            bident = consts.tile([128, 16], BF16)
            nc.sync.dma_start(out=bident, in_=bident_d[:, :])
            w1s = consts.tile([128, 9, 4, 128], BF16)
            nc.sync.dma_start(out=w1s, in_=bass.AP(
                tensor=w1bd, offset=0,
                ap=[[128, 128], [4 * 128 * 128, 9], [128 * 128, 4], [1, 128]]))
            wqs = consts.tile([128, 9, 4, 128], BF16)
            nc.sync.dma_start(out=wqs, in_=bass.AP(
                tensor=wqbd, offset=0,
                ap=[[128, 128], [4 * 128 * 128, 9], [128 * 128, 4], [1, 128]]))
            eps_s = consts.tile([98, 2, IN_N, HID], BF16)
            for hf in range(2):
                nc.sync.dma_start(out=eps_s[:, hf, :, :], in_=bass.AP(
                    tensor=ep, offset=hf * 98 * HID,
                    ap=[[HID, 98], [196 * HID, IN_N], [1, HID]]))
            gam_b = consts.tile([128, NT_J, 16], F32)
            nc.sync.dma_start(out=gam_b, in_=bass.AP(
                tensor=gam_d, offset=0, ap=[[0, 128], [0, NT_J], [1, 16]]))
            bet_b = consts.tile([128, NT_J, 16], F32)
            nc.sync.dma_start(out=bet_b, in_=bass.AP(
                tensor=bet_d, offset=0, ap=[[0, 128], [0, NT_J], [1, 16]]))
            eps_t = consts.tile([128, 1], F32)
            nc.vector.memset(eps_t, EPS)
            zpad = consts.tile([96, 16], F32)
            nc.vector.memset(zpad, 0.0)

            # ---- phase A: inputs come pre-padded from host ----
            x0 = big.tile([128, NB, 4, 16, 16], BF16)
            x0q = big.tile([128, NB, 4, 9, 9], BF16)
            for b in range(NB):
                for blk in range(4):
                    nc.sync.dma_start(out=x0[:, b, blk, :, :],
                                      in_=cp_img[b, blk * 128:(blk + 1) * 128, :, :])
                    nc.sync.dma_start(out=x0q[:, b, blk, :, :],
                                      in_=qp_img[b, blk * 128:(blk + 1) * 128, :, :])

            # ---- conv1 (block-diag, tap-outer for weight reuse) ----
            x1 = big.tile([98, NB, 2, C], BF16)   # X1^T: [s_lo, b, s_half, c]
            for blk in range(4):
                for bp in range(2):
                    pc = pp_sm.tile([128, 2, 196], F32, tag="sm")
                    for tap in range(9):
                        ky, kx = tap // 3, tap % 3
                        nc.tensor.matmul(
                            out=pc[:, :, :], lhsT=w1s[:, tap, blk, :],
                            rhs=x0[:, bp * 2:bp * 2 + 2, blk,
                                   ky:ky + 14, kx:kx + 14],
                            start=(tap == 0), stop=(tap == 8))
                    for i in range(2):
                        b = bp * 2 + i
                        x1c = small.tile([128, 196], BF16, tag="x1c")
                        nc.vector.tensor_copy(out=x1c, in_=pc[:, i, :])
                        for hf in range(2):
                            pt = pp_sm.tile([98, 128], BF16, tag="sm")
                            nc.tensor.transpose(pt, x1c[:, hf * 98:(hf + 1) * 98],
                                                ident)
                            nc.vector.tensor_copy(
                                out=x1[:, b, hf, blk * 128:(blk + 1) * 128], in_=pt)

            # ---- k projection (per capsule), K assembly ----
            kt_rep = big.tile([128, NB, 2112], BF16)  # K^T replicas at part 0/32/64/96
            k_sb = big.tile([128, NB, NT_I, 16], F32)
            for b in range(NB):
                nc.sync.dma_start(out=k_sb[0:49, b, 16, :], in_=rel_k[:, :])
            for m in range(2):          # capsule halves (16 each)
                pk = pp_sm.tile([128, NB, 8, 16], F32, tag="sm")
                for q in range(16):
                    n2 = m * 16 + q
                    pslice = pk[64 * (q % 2):64 * (q % 2) + 64, :, q // 2, :]
                    tp = (0, 64) if (q % 2) else (0, 0)
                    for hf in range(2):
                        nc.tensor.matmul(
                            out=pslice, lhsT=eps_s[:, hf, n2, :],
                            rhs=x1[:, :, hf, n2::32],
                            start=(hf == 0), stop=(hf == 1),
                            tile_position=tp)
                for b in range(NB):
                    nc.vector.tensor_copy(out=k_sb[:, b, m * 8:(m + 1) * 8, :],
                                          in_=pk[:, b, :, :])

            # K -> bf16 -> K^T via PE transpose; replicate to row groups
            for b in range(NB):
                kbf = perb.tile([128, 16, 16], BF16, tag="kbf")
                nc.vector.tensor_copy(out=kbf, in_=k_sb[:, b, 0:16, :])
                for grp in range(4):
                    pt = pp_sm.tile([16, 512], BF16, tag="sm")
                    for u in range(4):
                        t = grp * 4 + u
                        nc.tensor.transpose(pt[:, u * 128:(u + 1) * 128],
                                            kbf[:, t, :], ident)
                    nc.scalar.copy(
                        out=kt_rep[0:16, b, grp * 512:(grp + 1) * 512], in_=pt)
                nc.sync.dma_start(out=kt_rep[0:16, b, 2048:2097], in_=rel_kt[:, :])
                for g in range(1, 4):
                    nc.sync.dma_start(out=kt_rep[32 * g:32 * g + 16, b, 0:2097],
                                      in_=kt_rep[0:16, b, 0:2097])

            # ---- convq -> fq roundtrip -> Q^T ----
            qt_rep = big.tile([128, NB, 1568], BF16)
            for blk in range(4):
                pc = pp_sm.tile([128, NB, 49], F32, tag="sm")
                for tap in range(9):
                    ky, kx = tap // 3, tap % 3
                    nc.tensor.matmul(
                        out=pc[:, :, :], lhsT=wqs[:, tap, blk, :],
                        rhs=x0q[:, :, blk, ky:ky + 7, kx:kx + 7],
                        start=(tap == 0), stop=(tap == 8))
                for b in range(NB):
                    xqc = small.tile([128, 49], BF16, tag="xqc")
                    nc.vector.tensor_copy(out=xqc, in_=pc[:, b, :])
                    nc.sync.dma_start(
                        out=fq[b, ds(blk * 6272, 6272)].rearrange("(p s) -> p s", p=128),
                        in_=xqc)
            for b in range(NB):
                # p-major reload: partition p holds flat [208p, 208p+208)
                q_all = perb.tile([128, NT_J, 16], BF16, tag="qall")
                nc.sync.dma_start(
                    out=q_all,
                    in_=fq[b, :].rearrange("(p t e) -> p t e", p=128, e=16))
                for t in range(NT_J):
                    # transpose j-rows {13p + t} -> QT cols strided by 13
                    pt = pp_sm.tile([16, 128], BF16, tag="sm")
                    rows = (J - 1 - t) // NT_J + 1
                    nc.tensor.transpose(pt[:, :rows], q_all[:rows, t, :],
                                        ident[:rows, :rows])
                    dst = qt_rep[0:16, b, t::NT_J]
                    nc.scalar.copy(out=dst[:, :rows], in_=pt[:, :rows])
                for g in range(1, 4):
                    nc.sync.dma_start(out=qt_rep[32 * g:32 * g + 16, b, :],
                                      in_=qt_rep[0:16, b, :])

            # ---- phase B: attention + output per batch ----
            for b in range(NB):
                ps_o = pp_o.tile([128, 512], F32, tag="po")
                for it in range(NT_I):
                    rows = _rows_i(it)
                    ps_s = pp_s.tile([128, 2048], F32, tag="ps")
                    for g, (c0, w) in enumerate(CHUNKS):
                        nc.tensor.matmul(
                            out=ps_s[:rows, c0:c0 + w],
                            lhsT=kt_rep[32 * g:32 * g + 16, b,
                                        it * 128:it * 128 + rows],
                            rhs=qt_rep[32 * g:32 * g + 16, b, c0:c0 + w],
                            start=True, stop=True, tile_position=(32 * g, 0))
                    e_sb = ebuf.tile([128, 1568], BF16, tag="e")
                    zcol = small.tile([128, 1], F32, tag="z")
                    nc.scalar.activation(out=e_sb[:rows, :], in_=ps_s[:rows, 0:1568],
                                         func=AF.Exp, scale=SCALE,
                                         accum_out=zcol[:rows, :])
                    rcol = small.tile([128, 1], F32, tag="r")
                    nc.vector.reciprocal(out=rcol[:rows, :], in_=zcol[:rows, :])
                    kp = small.tile([128, 16], BF16, tag="kp")
                    nc.vector.tensor_scalar_mul(out=kp[:rows, :],
                                                in0=k_sb[:rows, b, it, :],
                                                scalar1=rcol[:rows, :])
                    for g, (c0, w) in enumerate(CHUNKS):
                        nc.tensor.matmul(
                            out=ps_o[32 * g:32 * g + 16, 0:w],
                            lhsT=kp[:rows, :], rhs=e_sb[:rows, c0:c0 + w],
                            start=(it == 0), stop=(it == NT_I - 1),
                            tile_position=(0, 32 * g), skip_group_check=True)

                # Out^T -> flat Out rows (p-major: partition p = flat 208p..)
                otf = perb.tile([16, 1568], BF16, tag="ot")
                for g, (c0, w) in enumerate(CHUNKS):
                    nc.vector.tensor_copy(out=otf[:, c0:c0 + w],
                                          in_=ps_o[32 * g:32 * g + 16, 0:w])
                fo_sb = perb.tile([128, NT_J, 16], BF16, tag="fos")
                nc.vector.memset(fo_sb, 0.0)
                ps_ot = pp_sm.tile([128, 208], BF16, tag="sm")
                for t in range(NT_J):
                    rows = (J - 1 - t) // NT_J + 1
                    nc.tensor.transpose(ps_ot[:rows, t * 16:(t + 1) * 16],
                                        otf[:, t::NT_J][:, :rows], bident[0:16, :])
                    nc.vector.tensor_copy(out=fo_sb[:rows, t, :],
                                          in_=ps_ot[:rows, t * 16:(t + 1) * 16])
                nc.sync.dma_start(
                    out=fo[b, :].rearrange("(p t e) -> p t e", p=128, e=16),
                    in_=fo_sb)

                # convout
                for blk in range(4):
                    xt = small.tile([128, 49], BF16, tag="xt")
                    nc.sync.dma_start(
                        out=xt,
                        in_=fo[b, ds(blk * 6272, 6272)].rearrange(
                            "(p s) -> p s", p=128))
                    x2p = small.tile([128, 9, 9], BF16, tag="x2p")
                    nc.vector.memset(x2p, 0.0)
                    nc.vector.tensor_copy(
                        out=x2p[:, 1:8, 1:8],
                        in_=xt.rearrange("p (y x) -> p y x", y=7))
                    pc = pp_sm.tile([128, 49], F32, tag="sm")
                    for tap in range(9):
                        ky, kx = tap // 3, tap % 3
                        nc.tensor.matmul(
                            out=pc[:, :], lhsT=wqs[:, tap, blk, :],
                            rhs=x2p[:, ky:ky + 7, kx:kx + 7],
                            start=(tap == 0), stop=(tap == 8))
                    x3c = small.tile([128, 49], F32, tag="x3c")
                    nc.vector.tensor_copy(out=x3c, in_=pc)
                    nc.sync.dma_start(
                        out=f3[b, ds(blk * 6272, 6272)].rearrange("(p s) -> p s", p=128),
                        in_=x3c)

                # LayerNorm over 16-elem groups of flat f3
                nc.sync.dma_start(
                    out=f3[b, ds(J * 16, 1536)].rearrange("(p e) -> p e", p=96),
                    in_=zpad)
                y = perb.tile([128, NT_J, 16], F32, tag="y")
                nc.sync.dma_start(
                    out=y, in_=f3[b, :].rearrange("(p t e) -> p t e", p=128, e=16))
                sums = small.tile([128, NT_J], F32, tag="sums")
                nc.vector.tensor_reduce(out=sums, in_=y, axis=mybir.AxisListType.X,
                                        op=ALU.add)
                sq = perb.tile([128, NT_J, 16], F32, tag="sq")
                nc.vector.tensor_mul(out=sq, in0=y, in1=y)
                sqs = small.tile([128, NT_J], F32, tag="sqs")
                nc.vector.tensor_reduce(out=sqs, in_=sq, axis=mybir.AxisListType.X,
                                        op=ALU.add)
                mu = small.tile([128, NT_J], F32, tag="mu")
                nc.vector.tensor_scalar_mul(out=mu, in0=sums, scalar1=1.0 / 16)
                msq = small.tile([128, NT_J], F32, tag="msq")
                nc.vector.tensor_mul(out=msq, in0=mu, in1=mu)
                var = small.tile([128, NT_J], F32, tag="var")
                nc.vector.scalar_tensor_tensor(out=var, in0=sqs, scalar=1.0 / 16,
                                               in1=msq, op0=ALU.mult,
                                               op1=ALU.subtract)
                # rstd = 1/sqrt(var+eps): bit-trick init + 2 Newton iters (DVE)
                vpe = small.tile([128, NT_J], F32, tag="vpe")
                nc.vector.tensor_scalar_add(out=vpe, in0=var, scalar1=EPS)
                rstd = small.tile([128, NT_J], F32, tag="rstd")
                ri = rstd[:, :].bitcast(mybir.dt.int32)
                nc.vector.tensor_scalar(
                    out=ri, in0=vpe[:, :].bitcast(mybir.dt.int32), scalar1=1,
                    scalar2=None, op0=ALU.logical_shift_right)
                nc.vector.tensor_scalar(
                    out=ri, in0=ri, scalar1=-1, scalar2=0x5F3759DF,
                    op0=ALU.mult, op1=ALU.add)
                tnw = small.tile([128, NT_J], F32, tag="tnw")
                for _ in range(2):
                    nc.vector.tensor_mul(out=tnw, in0=rstd, in1=rstd)
                    nc.vector.tensor_mul(out=tnw, in0=tnw, in1=vpe)
                    nc.vector.tensor_scalar(
                        out=tnw, in0=tnw, scalar1=-0.5, scalar2=1.5,
                        op0=ALU.mult, op1=ALU.add)
                    nc.vector.tensor_mul(out=rstd, in0=rstd, in1=tnw)
                yn = perb.tile([128, NT_J, 16], F32, tag="yn")
                for jt in range(NT_J):
                    nc.vector.tensor_scalar(
                        out=yn[:, jt, :], in0=y[:, jt, :],
                        scalar1=mu[:, jt:jt + 1], scalar2=rstd[:, jt:jt + 1],
                        op0=ALU.subtract, op1=ALU.mult)
                nc.vector.tensor_mul(out=yn, in0=yn, in1=gam_b)
                nc.vector.tensor_add(out=yn, in0=yn, in1=bet_b)
                nc.sync.dma_start(
                    out=outy[b, :].rearrange("(p t e) -> p t e", p=128, e=16),
                    in_=yn)

    nc.compile()
    return nc


def _blockdiag(w):
    out = np.zeros((9, 4, 128, 128), np.float32)
    for blk in range(4):
        for g in range(8):
            grp = blk * 8 + g
            for ky in range(3):
                for kx in range(3):
                    out[ky * 3 + kx, blk, g * 16:(g + 1) * 16, g * 16:(g + 1) * 16] = \
                        w[grp * 16:(grp + 1) * 16, :, ky, kx].T
    return out


def kernel(current_pose, next_pose, current_w, next_w, E_proj, rel_embedd,
           ln_gamma, ln_beta, num_iter=None):
    global _PROG
    if _PROG is None:
        _PROG = _build()

    bf = ml_dtypes.bfloat16
    cp_raw = np.ascontiguousarray(
        np.asarray(current_pose, np.float32).transpose(0, 1, 4, 2, 3)
    ).reshape(B, C, H, H)
    cp_img = np.zeros((B, C, 16, 16), np.float32)
    cp_img[:, :, 1:15, 1:15] = cp_raw
    cp_img = cp_img.astype(bf)
    qp_raw = np.ascontiguousarray(
        np.asarray(next_pose, np.float32).transpose(0, 1, 4, 2, 3)
    ).reshape(B, C, HO, HO)
    qp_img = np.zeros((B, C, 9, 9), np.float32)
    qp_img[:, :, 1:8, 1:8] = qp_raw
    qp_img = qp_img.astype(bf)
    w1bd = _blockdiag(np.asarray(current_w, np.float32)).astype(bf)
    wqbd = _blockdiag(np.asarray(next_w, np.float32)).astype(bf)
    ep = np.asarray(E_proj, np.float32).astype(bf)
    rel = np.asarray(rel_embedd, np.float32)
    ident = np.eye(128, dtype=np.float32).astype(bf)
    bident = np.zeros((128, 16), np.float32)
    for p in range(128):
        if p % 32 < 16:
            bident[p, p % 32] = 1.0
    bident = bident.astype(bf)

    common = {
        "w1bd": w1bd, "wqbd": wqbd, "ep": ep,
        "rel_k": np.ascontiguousarray(rel.T).astype(np.float32),
        "rel_kt": rel.astype(bf),
        "gam": np.asarray(ln_gamma, np.float32),
        "bet": np.asarray(ln_beta, np.float32),
        "ident": ident, "bident": bident,
    }
    core_ids = list(range(8))
    in_maps = []
    for c in core_ids:
        sl = slice(c * NB, (c + 1) * NB)
        in_maps.append({**common, "cp_img": np.ascontiguousarray(cp_img[sl]),
                        "qp_img": np.ascontiguousarray(qp_img[sl])})

    res = run_bass_kernel_spmd(_PROG, in_maps, core_ids)
    out = np.empty((B, J * 16), np.float32)
    for c in core_ids:
        out[c * NB:(c + 1) * NB] = res.results[c]["outy"][:, :J * 16]
    return out.reshape(B, OUT_N, HO, HO, OUT_D)


if __name__ == "__main__":
    import reference as ref
    inputs = ref.setup_inputs()
    expected = np.asarray(ref.reference(**inputs))
    actual = kernel(**{k: np.asarray(v) if not np.isscalar(v) else v
                       for k, v in inputs.items()})
    err = np.abs(actual - expected)
    sc = np.abs(expected).max()
    print("absmax err:", err.max(), "scale:", sc, "rel:", err.max() / sc)



# revision 2
# speedup vs baseline: 1.5094x; 1.5094x over previous
"""Trainium2 Bass kernel for nn_BilinearLinformerCapsuleFC (v2).

Data-parallel over batch (32 -> 4 per core x 8 cores). Single-core program:
grouped convs as block-diag matmuls (9 shifted taps, PSUM-accumulated),
Linformer key projection, column-softmax attention computed in
S^T [keys, queries] layout. The softmax exp runs as ONE whole-row
Activation instruction per i-tile (with accum_out row-sum normalizer),
double-buffered across two 4-bank PSUM halves so the Act engine (the
roofline for this problem) streams back-to-back. Out^T is accumulated
j-major (13 matmuls of 16-wide output each, nearly free on PE) directly
into a spare PSUM region, which makes the output relayout a single copy +
DMA. Prep for batches 0/1 runs up front in the still-free S-buffer banks;
prep for batches 2/3 is spread at half density over two attention windows
each; conv/LayerNorm post-processing trails one batch behind.
"""
import numpy as np
import ml_dtypes

import concourse.bass as bass
import concourse.mybir as mybir
import concourse.tile as tile
from concourse import bacc
from concourse.bass import ds
from concourse.bass_utils import run_bass_kernel_spmd

BF16 = mybir.dt.bfloat16
F32 = mybir.dt.float32
AF = mybir.ActivationFunctionType
ALU = mybir.AluOpType

B, IN_N, IN_D, H, OUT_N, OUT_D, HO, HID = 32, 32, 16, 14, 32, 16, 7, 64
C = IN_N * IN_D            # 512
NB = 4                     # batch items per core
NKEY = IN_N * HID + HO * HO  # 2097
J = OUT_N * HO * HO        # 1568
NT_I = 17                  # i tiles (16x128 + 49)
NT_J = 13                  # j tiles (12x128 + 32)
JPAD = NT_J * 2048         # 26624
EPS = 1e-5
SCALE = IN_D ** -0.5
CHUNKS = [(0, 512), (512, 512), (1024, 512), (1536, 32)]

_PROG = None


def _build():
    nc = bacc.Bacc("TRN2", target_bir_lowering=False, debug=False, num_devices=1)

    x0_d = nc.dram_tensor("x0d", [128, NB, 4, 16, 16], BF16, kind="ExternalInput")
    x0q_d = nc.dram_tensor("x0qd", [128, NB, 4, 9, 9], BF16, kind="ExternalInput")
    w1_d = nc.dram_tensor("w1d", [128, 9, 4, 128], BF16, kind="ExternalInput")
    wq_d = nc.dram_tensor("wqd", [128, 9, 4, 128], BF16, kind="ExternalInput")
    eps_d = nc.dram_tensor("epd", [98, 2, IN_N, HID], BF16, kind="ExternalInput")
    rel_k_d = nc.dram_tensor("rel_k", [49, 16], F32, kind="ExternalInput")
    rel_kt_d = nc.dram_tensor("rel_kt", [16, 49], BF16, kind="ExternalInput")
    gam_d = nc.dram_tensor("gam", [16], F32, kind="ExternalInput")
    bet_d = nc.dram_tensor("bet", [16], F32, kind="ExternalInput")
    ident_d = nc.dram_tensor("ident", [128, 128], BF16, kind="ExternalInput")

    import os
    DBG = os.environ.get("K2_DEBUG", "") == "1"
    okind = {"kind": "ExternalOutput"} if DBG else {}
    fq = nc.dram_tensor("fq", [NB, JPAD], BF16)
    fo = nc.dram_tensor("fo", [NB, JPAD], BF16, **okind)
    f3 = nc.dram_tensor("f3", [NB, JPAD], F32, **okind)
    if DBG:
        dbg_kt = nc.dram_tensor("dbg_kt", [128, 2112], BF16, kind="ExternalOutput")
        dbg_qt = nc.dram_tensor("dbg_qt", [128, J], BF16, kind="ExternalOutput")
        dbg_ksb = nc.dram_tensor("dbg_ksb", [128, NB, NT_I, 16], F32,
                                 kind="ExternalOutput")
    outy = nc.dram_tensor("outy", [NB, JPAD], F32, kind="ExternalOutput")

    with tile.TileContext(nc) as tc:
        from contextlib import ExitStack
        with ExitStack() as ctx:
            consts = ctx.enter_context(tc.tile_pool(name="consts", bufs=1))
            sm = ctx.enter_context(tc.tile_pool(name="sm", bufs=4))
            eb = ctx.enter_context(tc.tile_pool(name="eb", bufs=18))
            kpl = ctx.enter_context(tc.tile_pool(name="kpl", bufs=18))
            pb = ctx.enter_context(tc.tile_pool(name="pb", bufs=2))
            pp = ctx.enter_context(tc.tile_pool(name="pp", bufs=1, space="PSUM"))

            # ---- persistent SBUF ----
            ident = consts.tile([128, 128], BF16)
            w1s = consts.tile([128, 9, 4, 128], BF16)
            wqs = consts.tile([128, 9, 4, 128], BF16)
            eps_s = consts.tile([98, 2, IN_N, HID], BF16)
            gamb = consts.tile([128, NT_J, 16], F32)
            betb = consts.tile([128, NT_J, 16], F32)
            x0 = consts.tile([128, NB, 4, 16, 16], BF16)
            x0q = consts.tile([128, NB, 4, 9, 9], BF16)
            x1 = consts.tile([98, NB, 2, C], BF16)
            kt = consts.tile([128, 2112], BF16)   # b at partition 32b, rows 0:16
            qt = consts.tile([128, J], BF16)      # b at partition 32b
            k_sb = consts.tile([128, NB, NT_I, 16], F32)

            # ---- input DMAs (SP + Act queues; HWDGE is shared anyway) ----
            nc.sync.dma_start(out=x0[:, 0:2, :, :, :], in_=x0_d[:, 0:2, :, :, :])
            nc.scalar.dma_start(out=w1s, in_=w1_d[:, :, :, :])
            nc.sync.dma_start(out=ident, in_=ident_d[:, :])
            nc.sync.dma_start(out=x0q[:, 0:2, :, :, :], in_=x0q_d[:, 0:2, :, :, :])
            nc.sync.dma_start(out=wqs, in_=wq_d[:, :, :, :])
            nc.scalar.dma_start(out=eps_s, in_=eps_d[:, :, :, :])
            nc.sync.dma_start(out=x0[:, 2:, :, :, :], in_=x0_d[:, 2:, :, :, :])
            nc.sync.dma_start(out=x0q[:, 2:, :, :, :], in_=x0q_d[:, 2:, :, :, :])
            nc.sync.dma_start(out=k_sb[0:49, :, 16, :], in_=bass.AP(
                tensor=rel_k_d, offset=0, ap=[[16, 49], [0, NB], [1, 16]]))
            for b in range(NB):
                nc.sync.dma_start(out=kt[32 * b:32 * b + 16, 2048:2097],
                                  in_=rel_kt_d[:, :])
            nc.sync.dma_start(out=gamb, in_=bass.AP(
                tensor=gam_d, offset=0, ap=[[0, 128], [0, NT_J], [1, 16]]))
            nc.sync.dma_start(out=betb, in_=bass.AP(
                tensor=bet_d, offset=0, ap=[[0, 128], [0, NT_J], [1, 16]]))

            # ---- PSUM map: 8 banks total ----
            SA = pp.tile([128, 2048], F32)   # banks 0-3
            SB = pp.tile([128, 2048], F32)   # banks 4-7
            PO = SA[:, 1792:2000]            # Out^T accumulator (bank 3 spare)
            rot = [SB[:, 1568:1764], SB[:, 1764:1960]]    # conv scratch (bank 7)
            pco_r = SB[:, 1960:2009]                      # convout scratch
            ptx = [SA[:, 1568:1632].bitcast(BF16),        # transpose scratch
                   SA[:, 1632:1696].bitcast(BF16),
                   SA[:, 1696:1760].bitcast(BF16)]

            # ---- prep stages (per batch) ----
            kbf_st = {}
            xqc_st = {}
            x1c_st = {}

            def conv1_mm(b, blk, pc=None):
                if pc is None:
                    pc = rot[0]
                for tap in range(9):
                    ky, kx = tap // 3, tap % 3
                    nc.tensor.matmul(out=pc, lhsT=w1s[:, tap, blk, :],
                                     rhs=x0[:, b, blk, ky:ky + 14, kx:kx + 14],
                                     start=(tap == 0), stop=(tap == 8),
                                     skip_group_check=True)
                x1c = sm.tile([128, 196], BF16, tag="x1c")
                nc.vector.tensor_copy(out=x1c, in_=pc)
                x1c_st[(b, blk)] = x1c

            def conv1_tr(b, blk):
                x1c = x1c_st[(b, blk)]
                for hf in range(2):
                    pt = ptx[hf][0:98, :]
                    nc.tensor.transpose(pt, x1c[:, hf * 98:(hf + 1) * 98], ident)
                    nc.vector.tensor_copy(
                        out=x1[:, b, hf, blk * 128:(blk + 1) * 128], in_=pt)

            def kproj_h(b, m, half, pkf=None):
                if pkf is None:
                    pkf = rot[m % 2]
                for q in range(half * 8, half * 8 + 8):
                    n2 = m * 16 + q
                    po = 64 * (q % 2)
                    psl = pkf[po:po + 64, (q // 2) * 16:(q // 2) * 16 + 16]
                    tp = (0, 64) if (q % 2) else (0, 0)
                    for hf in range(2):
                        nc.tensor.matmul(out=psl, lhsT=eps_s[:, hf, n2, :],
                                         rhs=x1[:, b, hf, n2::32],
                                         start=(hf == 0), stop=(hf == 1),
                                         tile_position=tp, skip_group_check=True)
                if half == 1:
                    nc.vector.tensor_copy(
                        out=k_sb[:, b, m * 8:(m + 1) * 8, :],
                        in_=pkf[:, 0:128].rearrange("p (t e) -> p t e", e=16))

            def kbf_stage(b, h):
                kbf = sm.tile([128, 8, 16], BF16, tag="kbf")
                nc.vector.tensor_copy(
                    out=kbf, in_=k_sb[:, b, h * 8:(h + 1) * 8, :])
                kbf_st[(b, h)] = kbf

            def ktT_run(b, t0, nt):
                for t in range(t0, t0 + nt):
                    h, u = divmod(t, 8)
                    if (b, h) not in kbf_st:
                        kbf_stage(b, h)
                    kbf = kbf_st[(b, h)]
                    pt = ptx[t % 3][0:16, :]
                    nc.tensor.transpose(pt, kbf[:, u, :], ident)
                    nc.vector.tensor_copy(
                        out=kt[32 * b:32 * b + 16, t * 128:(t + 1) * 128], in_=pt)

            def convq_conv(b, blk, pcq=None):
                if pcq is None:
                    pcq = rot[blk % 2][:, 0:49]
                for tap in range(9):
                    ky, kx = tap // 3, tap % 3
                    nc.tensor.matmul(out=pcq, lhsT=wqs[:, tap, blk, :],
                                     rhs=x0q[:, b, blk, ky:ky + 7, kx:kx + 7],
                                     start=(tap == 0), stop=(tap == 8),
                                     skip_group_check=True)
                xqc = sm.tile([128, 49], BF16, tag="xqc")
                nc.vector.tensor_copy(out=xqc, in_=pcq)
                nc.sync.dma_start(
                    out=fq[b, ds(blk * 6272, 6272)].rearrange(
                        "(p s) -> p s", p=128),
                    in_=xqc)
                xqc_st[(b, blk)] = xqc

            qall_st = {}

            def qt_load(b):
                q_all = pb.tile([128, NT_J, 16], BF16, tag="qall")
                nc.sync.dma_start(
                    out=q_all,
                    in_=fq[b, :].rearrange("(p t e) -> p t e", p=128, e=16))
                qall_st[b] = q_all

            def qt_tr(b, t0, nt):
                q_all = qall_st[b]
                for t in range(t0, t0 + nt):
                    rows = (J - 1 - t) // NT_J + 1
                    pt = ptx[t % 3][0:16, :]
                    nc.tensor.transpose(pt[:, :rows], q_all[:rows, t, :],
                                        ident[:rows, :rows])
                    dst = qt[32 * b:32 * b + 16, t::NT_J]
                    nc.vector.tensor_copy(out=dst[:, :rows], in_=pt[:, :rows])

            # ---- post stages (convout + LN for batch b) ----
            post_state = {}

            def xt_load(b):
                x2p = pb.tile([128, 4, 9, 9], BF16, tag="x2p")
                nc.gpsimd.memset(x2p, 0.0)
                for blk in range(4):
                    nc.sync.dma_start(out=x2p[:, blk, 1:8, 1:8], in_=bass.AP(
                        tensor=fo, offset=b * JPAD + blk * 6272,
                        ap=[[49, 128], [7, 7], [1, 7]]))
                x3c = pb.tile([128, 4, 49], F32, tag="x3c")
                post_state[b] = {"x2p": x2p, "x3c": x3c}

            def convout_blk(b, blk):
                st = post_state[b]
                for tap in range(9):
                    ky, kx = tap // 3, tap % 3
                    nc.tensor.matmul(out=pco_r, lhsT=wqs[:, tap, blk, :],
                                     rhs=st["x2p"][:, blk, ky:ky + 7, kx:kx + 7],
                                     start=(tap == 0), stop=(tap == 8),
                                     skip_group_check=True)
                nc.vector.tensor_copy(out=st["x3c"][:, blk, :], in_=pco_r)

            def f3_roundtrip(b):
                st = post_state[b]
                nc.sync.dma_start(out=bass.AP(
                    tensor=f3, offset=b * JPAD, ap=[[49, 128], [6272, 4], [1, 49]]),
                    in_=st["x3c"])
                y = pb.tile([128, NT_J, 16], F32, tag="y")
                nc.sync.dma_start(out=y, in_=f3[b, :].rearrange(
                    "(p t e) -> p t e", p=128, e=16))
                st["y"] = y

            def ln_a(b):
                st = post_state[b]
                y = st["y"]
                sums = pb.tile([128, NT_J], F32, tag="sums")
                nc.vector.tensor_reduce(out=sums, in_=y,
                                        axis=mybir.AxisListType.X, op=ALU.add)
                sq = pb.tile([128, NT_J, 16], F32, tag="sq")
                nc.gpsimd.tensor_mul(out=sq, in0=y, in1=y)
                sqs = pb.tile([128, NT_J], F32, tag="sqs")
                nc.vector.tensor_reduce(out=sqs, in_=sq,
                                        axis=mybir.AxisListType.X, op=ALU.add)
                st["sums"], st["sqs"] = sums, sqs

            def ln_b(b):
                st = post_state[b]
                mu = pb.tile([128, NT_J], F32, tag="mu")
                nc.vector.tensor_scalar_mul(out=mu, in0=st["sums"],
                                            scalar1=1.0 / 16)
                msq = pb.tile([128, NT_J], F32, tag="msq")
                nc.vector.tensor_mul(out=msq, in0=mu, in1=mu)
                vpe = pb.tile([128, NT_J], F32, tag="vpe")
                nc.vector.scalar_tensor_tensor(out=vpe, in0=st["sqs"],
                                               scalar=1.0 / 16, in1=msq,
                                               op0=ALU.mult, op1=ALU.subtract)
                nc.vector.tensor_scalar_add(out=vpe, in0=vpe, scalar1=EPS)
                rstd = pb.tile([128, NT_J], F32, tag="rstd")
                ri = rstd[:, :].bitcast(mybir.dt.int32)
                nc.vector.tensor_scalar(
                    out=ri, in0=vpe[:, :].bitcast(mybir.dt.int32), scalar1=1,
                    scalar2=None, op0=ALU.logical_shift_right)
                nc.vector.tensor_scalar(
                    out=ri, in0=ri, scalar1=-1, scalar2=0x5F3759DF,
                    op0=ALU.mult, op1=ALU.add)
                tnw = pb.tile([128, NT_J], F32, tag="tnw")
                for _ in range(2):
                    nc.vector.tensor_mul(out=tnw, in0=rstd, in1=rstd)
                    nc.vector.tensor_mul(out=tnw, in0=tnw, in1=vpe)
                    nc.vector.tensor_scalar(
                        out=tnw, in0=tnw, scalar1=-0.5, scalar2=1.5,
                        op0=ALU.mult, op1=ALU.add)
                    nc.vector.tensor_mul(out=rstd, in0=rstd, in1=tnw)
                st["mu"], st["rstd"] = mu, rstd

            def ln_c(b):
                st = post_state[b]
                yn = pb.tile([128, NT_J, 16], F32, tag="yn")
                for jt in range(NT_J):
                    eng = nc.gpsimd
                    eng.tensor_scalar(
                        out=yn[:, jt, :], in0=st["y"][:, jt, :],
                        scalar1=st["mu"][:, jt:jt + 1],
                        scalar2=st["rstd"][:, jt:jt + 1],
                        op0=ALU.subtract, op1=ALU.mult)
                st["yn"] = yn

            def ln_d(b):
                st = post_state[b]
                yn = st["yn"]
                nc.vector.tensor_mul(out=yn, in0=yn, in1=gamb)
                nc.vector.tensor_add(out=yn, in0=yn, in1=betb)
                nc.sync.dma_start(
                    out=outy[b, :].rearrange("(p t e) -> p t e", p=128, e=16),
                    in_=yn)

            def post_slots(b):
                return {1: [lambda: xt_load(b)],
                        3: [lambda: convout_blk(b, 0)],
                        5: [lambda: convout_blk(b, 1)],
                        7: [lambda: convout_blk(b, 2)],
                        9: [lambda: convout_blk(b, 3)],
                        11: [lambda: f3_roundtrip(b)],
                        12: [lambda: ln_a(b)],
                        13: [lambda: ln_b(b)],
                        14: [lambda: ln_c(b)],
                        15: [lambda: ln_d(b)]}

            def prep_slots(b):
                return {1: [lambda: conv1_mm(b, 0), lambda: conv1_tr(b, 0)],
                        2: [lambda: conv1_mm(b, 1), lambda: conv1_tr(b, 1),
                            lambda: convq_conv(b, 0)],
                        3: [lambda: conv1_mm(b, 2), lambda: conv1_tr(b, 2),
                            lambda: convq_conv(b, 1)],
                        4: [lambda: conv1_mm(b, 3), lambda: conv1_tr(b, 3),
                            lambda: convq_conv(b, 2)],
                        5: [lambda: kproj_h(b, 0, 0), lambda: kproj_h(b, 0, 1),
                            lambda: convq_conv(b, 3)],
                        6: [lambda: kproj_h(b, 1, 0), lambda: kproj_h(b, 1, 1),
                            lambda: qt_load(b)],
                        7: [lambda: ktT_run(b, 0, 2)],
                        8: [lambda: ktT_run(b, 2, 2)],
                        9: [lambda: qt_tr(b, 0, 5)],
                        10: [lambda: qt_tr(b, 5, 5)],
                        11: [lambda: qt_tr(b, 10, 3)],
                        12: [lambda: ktT_run(b, 4, 2)],
                        13: [lambda: ktT_run(b, 6, 2)],
                        14: [lambda: ktT_run(b, 8, 4)],
                        15: [lambda: ktT_run(b, 12, 4)]}

            # ---- attention ----
            def attention(b, inject, carry):
                e_t, kp_t = {}, {}
                oa_box = {}

                def emit_out_block(it0, it1, first, last):
                    # contiguous in the PE stream => the PSUM zero-region
                    # cannot be poisoned mid-accumulation
                    for it in range(it0, it1 + 1):
                        e, rows = e_t[it]
                        kp = kp_t[it]
                        for t in range(NT_J):
                            cols = 128 if t < 12 else 32
                            nc.tensor.matmul(
                                out=PO[0:cols, t * 16:(t + 1) * 16],
                                lhsT=e[:rows, t * 128:t * 128 + cols],
                                rhs=kp[:rows, :],
                                start=(it == it0 and t == 0 and first),
                                stop=(it == it1 and last),
                                skip_group_check=True)

                def emit_s(it):
                    rows = 128 if it < 16 else 49
                    X = SA if (b * NT_I + it) % 2 == 0 else SB
                    for (c0, w) in CHUNKS:
                        nc.tensor.matmul(
                            out=X[:rows, c0:c0 + w],
                            lhsT=kt[32 * b:32 * b + 16, it * 128:it * 128 + rows],
                            rhs=qt[32 * b:32 * b + 16, c0:c0 + w],
                            start=True, stop=True,
                            tile_position=(32 * b, 0), skip_group_check=True)
                    return X, rows

                Xr = {0: emit_s(0)}
                for it in range(NT_I):
                    if it + 1 < NT_I:
                        Xr[it + 1] = emit_s(it + 1)
                    if it == 0 and carry is not None:
                        carry()
                    X, rows = Xr[it]
                    e = eb.tile([128, J], BF16, tag="e")
                    z = sm.tile([128, 1], F32, tag="z")
                    nc.scalar.activation(out=e[:rows, :], in_=X[:rows, 0:1568],
                                         func=AF.Exp, scale=SCALE,
                                         accum_out=z[:rows, :])
                    r = sm.tile([128, 1], F32, tag="r")
                    nc.vector.reciprocal(out=r[:rows, :], in_=z[:rows, :])
                    kp = kpl.tile([128, 16], BF16, tag="kp")
                    nc.vector.tensor_scalar_mul(out=kp[:rows, :],
                                                in0=k_sb[:rows, b, it, :],
                                                scalar1=r[:rows, :])
                    e_t[it] = (e, rows)
                    kp_t[it] = kp
                    if it == 9:
                        emit_out_block(0, 8, True, True)
                        oa = pb.tile([128, 208], F32, tag="oa")
                        nc.vector.tensor_copy(out=oa, in_=PO)
                        oa_box["oa"] = oa
                    if it > 0:
                        for th in inject.get(it - 1, []):
                            th()
                for th in inject.get(NT_I - 1, []):
                    th()

                def _carry():
                    emit_out_block(9, NT_I - 1, True, True)
                    fo_sb = pb.tile([128, NT_J, 16], BF16, tag="fos")
                    nc.vector.tensor_add(
                        out=fo_sb,
                        in0=oa_box["oa"].rearrange("p (t e) -> p t e", e=16),
                        in1=PO.rearrange("p (t e) -> p t e", e=16))
                    nc.sync.dma_start(
                        out=fo[b, :].rearrange("(t p e) -> p t e", p=128, e=16),
                        in_=fo_sb)
                return _carry

            # ---- schedule ----
            for blk in range(4):
                conv1_mm(0, blk)
                conv1_tr(0, blk)
            for blk in range(4):
                convq_conv(0, blk)
            kproj_h(0, 0, 0)
            kproj_h(0, 0, 1)
            kproj_h(0, 1, 0)
            kproj_h(0, 1, 1)
            qt_load(0)
            ktT_run(0, 0, 3)
            qt_tr(0, 0, 7)
            qt_tr(0, 7, 6)

            carry = None
            for b in range(NB):
                inject = {}
                if b == 0:
                    for i in range(6):
                        inject[i] = [lambda p=i: ktT_run(0, 3 + 2 * p, 2)]
                    inject[6] = [lambda: ktT_run(0, 15, 1)]
                if b + 1 < NB:
                    for k, v in prep_slots(b + 1).items():
                        inject.setdefault(k, []).extend(v)
                if b >= 1:
                    for k, v in post_slots(b - 1).items():
                        inject.setdefault(k, []).extend(v)
                carry = attention(b, inject, carry)
            carry()
            xt_load(NB - 1)
            for blk in range(4):
                convout_blk(NB - 1, blk)
            f3_roundtrip(NB - 1)
            ln_a(NB - 1)
            ln_b(NB - 1)
            ln_c(NB - 1)
            ln_d(NB - 1)
            if DBG:
                nc.sync.dma_start(out=dbg_kt[:, :], in_=kt)
                nc.sync.dma_start(out=dbg_qt[:, :], in_=qt)
                nc.sync.dma_start(out=dbg_ksb[:, :, :, :], in_=k_sb)

    nc.compile()
    return nc


def _blockdiag_pm(w):
    # torch OIHW grouped weights -> partition-major block-diag [128, 9, 4, 128]
    out = np.zeros((128, 9, 4, 128), np.float32)
    for blk in range(4):
        for g in range(8):
            grp = blk * 8 + g
            for ky in range(3):
                for kx in range(3):
                    out[g * 16:(g + 1) * 16, ky * 3 + kx, blk,
                        g * 16:(g + 1) * 16] = w[grp * 16:(grp + 1) * 16,
                                                 :, ky, kx].T
    return out


def kernel(current_pose, next_pose, current_w, next_w, E_proj, rel_embedd,
           ln_gamma, ln_beta, num_iter=None):
    global _PROG
    if _PROG is None:
        _PROG = _build()

    bf = ml_dtypes.bfloat16
    cp_raw = np.ascontiguousarray(
        np.asarray(current_pose, np.float32).transpose(0, 1, 4, 2, 3)
    ).reshape(B, C, H, H)
    cp_img = np.zeros((B, C, 16, 16), np.float32)
    cp_img[:, :, 1:15, 1:15] = cp_raw
    x0_h = np.ascontiguousarray(
        cp_img.reshape(B, 4, 128, 16, 16).transpose(2, 0, 1, 3, 4)).astype(bf)
    qp_raw = np.ascontiguousarray(
        np.asarray(next_pose, np.float32).transpose(0, 1, 4, 2, 3)
    ).reshape(B, C, HO, HO)
    qp_img = np.zeros((B, C, 9, 9), np.float32)
    qp_img[:, :, 1:8, 1:8] = qp_raw
    x0q_h = np.ascontiguousarray(
        qp_img.reshape(B, 4, 128, 9, 9).transpose(2, 0, 1, 3, 4)).astype(bf)

    w1_h = _blockdiag_pm(np.asarray(current_w, np.float32)).astype(bf)
    wq_h = _blockdiag_pm(np.asarray(next_w, np.float32)).astype(bf)
    ep_h = np.ascontiguousarray(
        np.asarray(E_proj, np.float32).reshape(IN_N, 2, 98, HID)
        .transpose(2, 1, 0, 3)).astype(bf)
    rel = np.asarray(rel_embedd, np.float32)
    ident = np.eye(128, dtype=np.float32).astype(bf)

    common = {
        "w1d": w1_h, "wqd": wq_h, "epd": ep_h,
        "rel_k": np.ascontiguousarray(rel.T).astype(np.float32),
        "rel_kt": rel.astype(bf),
        "gam": np.asarray(ln_gamma, np.float32),
        "bet": np.asarray(ln_beta, np.float32),
        "ident": ident,
    }
    core_ids = list(range(8))
    in_maps = []
    for c in core_ids:
        sl = slice(c * NB, (c + 1) * NB)
        in_maps.append({**common,
                        "x0d": np.ascontiguousarray(x0_h[:, sl]),
                        "x0qd": np.ascontiguousarray(x0q_h[:, sl])})

    res = run_bass_kernel_spmd(_PROG, in_maps, core_ids)
    out = np.empty((B, J * 16), np.float32)
    for c in core_ids:
        out[c * NB:(c + 1) * NB] = res.results[c]["outy"][:, :J * 16]
    return out.reshape(B, OUT_N, HO, HO, OUT_D)


if __name__ == "__main__":
    import reference as ref
    inputs = ref.setup_inputs()
    expected = np.asarray(ref.reference(**inputs))
    actual = kernel(**{k: np.asarray(v) if not np.isscalar(v) else v
                       for k, v in inputs.items()})
    err = np.abs(actual - expected)
    sc = np.abs(expected).max()
    print("absmax err:", err.max(), "scale:", sc, "rel:", err.max() / sc)


# revision 3
# speedup vs baseline: 1.7536x; 1.1618x over previous
"""Trainium2 Bass kernel for nn_BilinearLinformerCapsuleFC (v2).

Data-parallel over batch (32 -> 4 per core x 8 cores). Single-core program:
grouped convs as block-diag matmuls (9 shifted taps, PSUM-accumulated),
Linformer key projection, column-softmax attention computed in
S^T [keys, queries] layout. The softmax exp runs as ONE whole-row
Activation instruction per i-tile (with accum_out row-sum normalizer),
double-buffered across two 4-bank PSUM halves so the Act engine (the
roofline for this problem) streams back-to-back. Out^T is accumulated
j-major (13 matmuls of 16-wide output each, nearly free on PE) directly
into a spare PSUM region, which makes the output relayout a single copy +
DMA. Prep for batches 0/1 runs up front in the still-free S-buffer banks;
prep for batches 2/3 is spread at half density over two attention windows
each; conv/LayerNorm post-processing trails one batch behind.
"""
import numpy as np
import ml_dtypes

import concourse.bass as bass
import concourse.mybir as mybir
import concourse.tile as tile
from concourse import bacc
from concourse.bass import ds
from concourse.bass_utils import run_bass_kernel_spmd

BF16 = mybir.dt.bfloat16
F32 = mybir.dt.float32
AF = mybir.ActivationFunctionType
ALU = mybir.AluOpType

B, IN_N, IN_D, H, OUT_N, OUT_D, HO, HID = 32, 32, 16, 14, 32, 16, 7, 64
C = IN_N * IN_D            # 512
NB = 4                     # batch items per core
NKEY = IN_N * HID + HO * HO  # 2097
J = OUT_N * HO * HO        # 1568
NT_I = 17                  # i tiles (16x128 + 49)
NT_J = 13                  # j tiles (12x128 + 32)
JPAD = NT_J * 2048         # 26624
EPS = 1e-5
SCALE = IN_D ** -0.5
CHUNKS = [(0, 512), (512, 512), (1024, 512), (1536, 32)]

_PROG = None


def _build():
    nc = bacc.Bacc("TRN2", target_bir_lowering=False, debug=False, num_devices=1)

    x0_d = nc.dram_tensor("x0d", [128, NB, 4, 16, 16], BF16, kind="ExternalInput")
    x0q_d = nc.dram_tensor("x0qd", [128, NB, 4, 9, 9], BF16, kind="ExternalInput")
    w1_d = nc.dram_tensor("w1d", [128, 9, 4, 128], BF16, kind="ExternalInput")
    wq_d = nc.dram_tensor("wqd", [128, 9, 4, 128], BF16, kind="ExternalInput")
    eps_d = nc.dram_tensor("epd", [98, 2, IN_N, HID], BF16, kind="ExternalInput")
    rel_k_d = nc.dram_tensor("rel_k", [49, 16], F32, kind="ExternalInput")
    rel_kt_d = nc.dram_tensor("rel_kt", [16, 49], BF16, kind="ExternalInput")
    gam_d = nc.dram_tensor("gam", [16], F32, kind="ExternalInput")
    bet_d = nc.dram_tensor("bet", [16], F32, kind="ExternalInput")
    ident_d = nc.dram_tensor("ident", [128, 128], BF16, kind="ExternalInput")

    import os
    DBG = os.environ.get("K2_DEBUG", "") == "1"
    okind = {"kind": "ExternalOutput"} if DBG else {}
    fq = nc.dram_tensor("fq", [NB, JPAD], BF16)
    fo = nc.dram_tensor("fo", [NB, JPAD], BF16, **okind)
    f3 = nc.dram_tensor("f3", [NB, JPAD], F32, **okind)
    if DBG:
        dbg_kt = nc.dram_tensor("dbg_kt", [128, 2112], BF16, kind="ExternalOutput")
        dbg_qt = nc.dram_tensor("dbg_qt", [128, J], BF16, kind="ExternalOutput")
        dbg_ksb = nc.dram_tensor("dbg_ksb", [128, NB, NT_I, 16], F32,
                                 kind="ExternalOutput")
    outy = nc.dram_tensor("outy", [NB, JPAD], F32, kind="ExternalOutput")

    with tile.TileContext(nc) as tc:
        from contextlib import ExitStack
        with ExitStack() as ctx:
            consts = ctx.enter_context(tc.tile_pool(name="consts", bufs=1))
            sm = ctx.enter_context(tc.tile_pool(name="sm", bufs=4))
            eb = ctx.enter_context(tc.tile_pool(name="eb", bufs=18))
            kpl = ctx.enter_context(tc.tile_pool(name="kpl", bufs=18))
            pb = ctx.enter_context(tc.tile_pool(name="pb", bufs=2))
            pp = ctx.enter_context(tc.tile_pool(name="pp", bufs=1, space="PSUM"))

            # ---- persistent SBUF ----
            ident = consts.tile([128, 128], BF16)
            w1s = consts.tile([128, 9, 4, 128], BF16)
            wqs = consts.tile([128, 9, 4, 128], BF16)
            eps_s = consts.tile([98, 2, IN_N, HID], BF16)
            gamb = consts.tile([128, NT_J, 16], F32)
            betb = consts.tile([128, NT_J, 16], F32)
            x0 = consts.tile([128, NB, 4, 16, 16], BF16)
            x0q = consts.tile([128, NB, 4, 9, 9], BF16)
            x1 = consts.tile([98, NB, 2, C], BF16)
            kt = consts.tile([128, 2112], BF16)   # b at partition 32b, rows 0:16
            qt = consts.tile([128, J], BF16)      # b at partition 32b
            k_sb = consts.tile([128, NB, NT_I, 16], F32)

            # ---- input DMAs (SP + Act queues; HWDGE is shared anyway) ----
            nc.sync.dma_start(out=x0[:, 0:2, :, :, :], in_=x0_d[:, 0:2, :, :, :])
            nc.scalar.dma_start(out=w1s, in_=w1_d[:, :, :, :])
            nc.sync.dma_start(out=ident, in_=ident_d[:, :])
            nc.sync.dma_start(out=x0q[:, 0:2, :, :, :], in_=x0q_d[:, 0:2, :, :, :])
            nc.sync.dma_start(out=wqs, in_=wq_d[:, :, :, :])
            nc.scalar.dma_start(out=eps_s, in_=eps_d[:, :, :, :])
            nc.sync.dma_start(out=x0[:, 2:, :, :, :], in_=x0_d[:, 2:, :, :, :])
            nc.sync.dma_start(out=x0q[:, 2:, :, :, :], in_=x0q_d[:, 2:, :, :, :])
            nc.sync.dma_start(out=k_sb[0:49, :, 16, :], in_=bass.AP(
                tensor=rel_k_d, offset=0, ap=[[16, 49], [0, NB], [1, 16]]))
            for b in range(NB):
                nc.sync.dma_start(out=kt[32 * b:32 * b + 16, 2048:2097],
                                  in_=rel_kt_d[:, :])
            nc.sync.dma_start(out=gamb, in_=bass.AP(
                tensor=gam_d, offset=0, ap=[[0, 128], [0, NT_J], [1, 16]]))
            nc.sync.dma_start(out=betb, in_=bass.AP(
                tensor=bet_d, offset=0, ap=[[0, 128], [0, NT_J], [1, 16]]))

            # ---- PSUM map: 8 banks total ----
            SA = pp.tile([128, 2048], F32)   # banks 0-3
            SB = pp.tile([128, 2048], F32)   # banks 4-7
            PO = SA[:, 1792:2000]            # Out^T accumulator (bank 3 spare)
            rot = [SB[:, 1568:1764], SB[:, 1764:1960]]    # conv scratch (bank 7)
            pco_r = SB[:, 1960:2009]                      # convout scratch
            ptx = [SA[:, 1568:1632].bitcast(BF16),        # transpose scratch
                   SA[:, 1632:1696].bitcast(BF16),
                   SA[:, 1696:1760].bitcast(BF16)]
            ptw = SA[:, 1568:1760].bitcast(BF16)          # all 3, adjacent

            # ---- prep stages (per batch) ----
            kbf_st = {}
            xqc_st = {}
            x1c_st = {}

            def conv1_mm(b, blk, pc=None):
                if pc is None:
                    pc = rot[0]
                for tap in range(9):
                    ky, kx = tap // 3, tap % 3
                    nc.tensor.matmul(out=pc, lhsT=w1s[:, tap, blk, :],
                                     rhs=x0[:, b, blk, ky:ky + 14, kx:kx + 14],
                                     start=(tap == 0), stop=(tap == 8),
                                     skip_group_check=True)
                x1c = sm.tile([128, 196], BF16, tag="x1c")
                nc.vector.tensor_copy(out=x1c, in_=pc)
                x1c_st[(b, blk)] = x1c

            def conv1_tr(b, blk):
                x1c = x1c_st[(b, blk)]
                for hf in range(2):
                    pt = ptx[(2 * blk + hf) % 3][0:98, :]
                    nc.tensor.transpose(pt, x1c[:, hf * 98:(hf + 1) * 98], ident)
                    nc.vector.tensor_copy(
                        out=x1[:, b, hf, blk * 128:(blk + 1) * 128], in_=pt)

            def kproj_h(b, m, half, pkf=None):
                if pkf is None:
                    pkf = rot[m % 2]
                for q in range(half * 8, half * 8 + 8):
                    n2 = m * 16 + q
                    po = 64 * (q % 2)
                    psl = pkf[po:po + 64, (q // 2) * 16:(q // 2) * 16 + 16]
                    tp = (0, 64) if (q % 2) else (0, 0)
                    for hf in range(2):
                        nc.tensor.matmul(out=psl, lhsT=eps_s[:, hf, n2, :],
                                         rhs=x1[:, b, hf, n2::32],
                                         start=(hf == 0), stop=(hf == 1),
                                         tile_position=tp, skip_group_check=True)
                if half == 1:
                    nc.vector.tensor_copy(
                        out=k_sb[:, b, m * 8:(m + 1) * 8, :],
                        in_=pkf[:, 0:128].rearrange("p (t e) -> p t e", e=16))

            def kbf_stage(b, h):
                kbf = sm.tile([128, 8, 16], BF16, tag="kbf")
                nc.vector.tensor_copy(
                    out=kbf, in_=k_sb[:, b, h * 8:(h + 1) * 8, :])
                kbf_st[(b, h)] = kbf

            def ktT_run(b, t0, nt):
                # nt <= 3 tiles; one batched copy from the adjacent scratch
                for i, t in enumerate(range(t0, t0 + nt)):
                    h, u = divmod(t, 8)
                    if (b, h) not in kbf_st:
                        kbf_stage(b, h)
                    kbf = kbf_st[(b, h)]
                    pt = ptx[i][0:16, :]
                    nc.tensor.transpose(pt, kbf[:, u, :], ident)
                nc.vector.tensor_copy(
                    out=kt[32 * b:32 * b + 16, t0 * 128:(t0 + nt) * 128],
                    in_=ptw[0:16, 0:nt * 128])

            def convq_conv(b, blk, pcq=None):
                if pcq is None:
                    pcq = rot[blk % 2][:, 0:49]
                for tap in range(9):
                    ky, kx = tap // 3, tap % 3
                    nc.tensor.matmul(out=pcq, lhsT=wqs[:, tap, blk, :],
                                     rhs=x0q[:, b, blk, ky:ky + 7, kx:kx + 7],
                                     start=(tap == 0), stop=(tap == 8),
                                     skip_group_check=True)
                xqc = sm.tile([128, 49], BF16, tag="xqc")
                nc.vector.tensor_copy(out=xqc, in_=pcq)
                nc.sync.dma_start(
                    out=fq[b, ds(blk * 6272, 6272)].rearrange(
                        "(p s) -> p s", p=128),
                    in_=xqc)
                xqc_st[(b, blk)] = xqc

            qall_st = {}

            def qt_load(b):
                q_all = pb.tile([128, NT_J, 16], BF16, tag="qall")
                nc.sync.dma_start(
                    out=q_all,
                    in_=fq[b, :].rearrange("(p t e) -> p t e", p=128, e=16))
                qall_st[b] = q_all

            def qt_tr(b, t0, nt):
                # nt <= 3 tiles with equal row counts; one strided copy
                q_all = qall_st[b]
                rows = (J - 1 - t0) // NT_J + 1
                for i, t in enumerate(range(t0, t0 + nt)):
                    pt = ptx[i][0:16, :]
                    nc.tensor.transpose(pt[:, :rows], q_all[:rows, t, :],
                                        ident[:rows, :rows])
                src_v = ptw[0:16, 0:3 * 128].rearrange(
                    "p (i c) -> p i c", i=3)[:, 0:nt, 0:rows]
                dst = qt[32 * b:32 * b + 16, t0:]
                dst_v = bass.AP(tensor=dst.tensor, offset=dst.offset,
                                ap=[dst.ap[0], [1, nt], [NT_J, rows]])
                nc.vector.tensor_copy(out=dst_v, in_=src_v)

            # ---- post stages (convout + LN for batch b) ----
            post_state = {}

            def xt_load(b):
                x2p = pb.tile([128, 4, 9, 9], BF16, tag="x2p")
                nc.gpsimd.memset(x2p, 0.0)
                for blk in range(4):
                    nc.sync.dma_start(out=x2p[:, blk, 1:8, 1:8], in_=bass.AP(
                        tensor=fo, offset=b * JPAD + blk * 6272,
                        ap=[[49, 128], [7, 7], [1, 7]]))
                x3c = pb.tile([128, 4, 49], F32, tag="x3c")
                post_state[b] = {"x2p": x2p, "x3c": x3c}

            def convout_blk(b, blk):
                st = post_state[b]
                for tap in range(9):
                    ky, kx = tap // 3, tap % 3
                    nc.tensor.matmul(out=pco_r, lhsT=wqs[:, tap, blk, :],
                                     rhs=st["x2p"][:, blk, ky:ky + 7, kx:kx + 7],
                                     start=(tap == 0), stop=(tap == 8),
                                     skip_group_check=True)
                nc.vector.tensor_copy(out=st["x3c"][:, blk, :], in_=pco_r)

            def f3_roundtrip(b):
                st = post_state[b]
                nc.sync.dma_start(out=bass.AP(
                    tensor=f3, offset=b * JPAD, ap=[[49, 128], [6272, 4], [1, 49]]),
                    in_=st["x3c"])
                y = pb.tile([128, NT_J, 16], F32, tag="y")
                nc.sync.dma_start(out=y, in_=f3[b, :].rearrange(
                    "(p t e) -> p t e", p=128, e=16))
                st["y"] = y

            def ln_a(b):
                st = post_state[b]
                y = st["y"]
                sums = pb.tile([128, NT_J], F32, tag="sums")
                nc.vector.tensor_reduce(out=sums, in_=y,
                                        axis=mybir.AxisListType.X, op=ALU.add)
                sq = pb.tile([128, NT_J, 16], F32, tag="sq")
                nc.gpsimd.tensor_mul(out=sq, in0=y, in1=y)
                sqs = pb.tile([128, NT_J], F32, tag="sqs")
                nc.vector.tensor_reduce(out=sqs, in_=sq,
                                        axis=mybir.AxisListType.X, op=ALU.add)
                st["sums"], st["sqs"] = sums, sqs

            def ln_b(b):
                st = post_state[b]
                mu = pb.tile([128, NT_J], F32, tag="mu")
                nc.vector.tensor_scalar_mul(out=mu, in0=st["sums"],
                                            scalar1=1.0 / 16)
                msq = pb.tile([128, NT_J], F32, tag="msq")
                nc.vector.tensor_mul(out=msq, in0=mu, in1=mu)
                vpe = pb.tile([128, NT_J], F32, tag="vpe")
                nc.vector.scalar_tensor_tensor(out=vpe, in0=st["sqs"],
                                               scalar=1.0 / 16, in1=msq,
                                               op0=ALU.mult, op1=ALU.subtract)
                nc.vector.tensor_scalar_add(out=vpe, in0=vpe, scalar1=EPS)
                rstd = pb.tile([128, NT_J], F32, tag="rstd")
                ri = rstd[:, :].bitcast(mybir.dt.int32)
                nc.vector.tensor_scalar(
                    out=ri, in0=vpe[:, :].bitcast(mybir.dt.int32), scalar1=1,
                    scalar2=None, op0=ALU.logical_shift_right)
                nc.vector.tensor_scalar(
                    out=ri, in0=ri, scalar1=-1, scalar2=0x5F3759DF,
                    op0=ALU.mult, op1=ALU.add)
                tnw = pb.tile([128, NT_J], F32, tag="tnw")
                for _ in range(2):
                    nc.vector.tensor_mul(out=tnw, in0=rstd, in1=rstd)
                    nc.vector.tensor_mul(out=tnw, in0=tnw, in1=vpe)
                    nc.vector.tensor_scalar(
                        out=tnw, in0=tnw, scalar1=-0.5, scalar2=1.5,
                        op0=ALU.mult, op1=ALU.add)
                    nc.vector.tensor_mul(out=rstd, in0=rstd, in1=tnw)
                st["mu"], st["rstd"] = mu, rstd

            def ln_c(b):
                st = post_state[b]
                yn = pb.tile([128, NT_J, 16], F32, tag="yn")
                for jt in range(NT_J):
                    eng = nc.gpsimd
                    eng.tensor_scalar(
                        out=yn[:, jt, :], in0=st["y"][:, jt, :],
                        scalar1=st["mu"][:, jt:jt + 1],
                        scalar2=st["rstd"][:, jt:jt + 1],
                        op0=ALU.subtract, op1=ALU.mult)
                st["yn"] = yn

            def ln_d(b):
                st = post_state[b]
                yn = st["yn"]
                nc.vector.tensor_mul(out=yn, in0=yn, in1=gamb)
                nc.vector.tensor_add(out=yn, in0=yn, in1=betb)
                nc.sync.dma_start(
                    out=outy[b, :].rearrange("(p t e) -> p t e", p=128, e=16),
                    in_=yn)

            def post_slots(b):
                return {0: [lambda: xt_load(b)],
                        2: [lambda: convout_blk(b, 0)],
                        4: [lambda: convout_blk(b, 1)],
                        6: [lambda: convout_blk(b, 2)],
                        8: [lambda: convout_blk(b, 3)],
                        11: [lambda: f3_roundtrip(b)],
                        12: [lambda: ln_a(b)],
                        13: [lambda: ln_b(b)],
                        14: [lambda: ln_c(b)],
                        15: [lambda: ln_d(b)]}

            def prep_slots(b):
                return {0: [lambda: conv1_mm(b, 0)],
                        1: [lambda: conv1_tr(b, 0), lambda: convq_conv(b, 0)],
                        2: [lambda: conv1_mm(b, 1), lambda: convq_conv(b, 1)],
                        3: [lambda: conv1_tr(b, 1), lambda: convq_conv(b, 2)],
                        4: [lambda: conv1_mm(b, 2), lambda: convq_conv(b, 3)],
                        5: [lambda: conv1_tr(b, 2), lambda: qt_load(b)],
                        6: [lambda: conv1_mm(b, 3)],
                        7: [lambda: conv1_tr(b, 3), lambda: qt_tr(b, 0, 3)],
                        8: [lambda: kproj_h(b, 0, 0), lambda: kproj_h(b, 0, 1),
                            lambda: qt_tr(b, 3, 3)],
                        9: [lambda: qt_tr(b, 6, 2), lambda: qt_tr(b, 8, 3)],
                        10: [lambda: kproj_h(b, 1, 0), lambda: kproj_h(b, 1, 1),
                             lambda: qt_tr(b, 11, 2)],
                        11: [lambda: ktT_run(b, 0, 3)],
                        12: [lambda: ktT_run(b, 3, 3)],
                        13: [lambda: ktT_run(b, 6, 3)],
                        14: [lambda: ktT_run(b, 9, 3)],
                        15: [lambda: ktT_run(b, 12, 3), lambda: ktT_run(b, 15, 1)]}

            # ---- attention ----
            def attention(b, inject, carry):
                e_t, kp_t = {}, {}
                oa_box = {}

                def emit_out_block(it0, it1, first, last):
                    # contiguous in the PE stream => the PSUM zero-region
                    # cannot be poisoned mid-accumulation
                    for it in range(it0, it1 + 1):
                        e, rows = e_t[it]
                        kp = kp_t[it]
                        for t in range(NT_J):
                            cols = 128 if t < 12 else 32
                            nc.tensor.matmul(
                                out=PO[0:cols, t * 16:(t + 1) * 16],
                                lhsT=e[:rows, t * 128:t * 128 + cols],
                                rhs=kp[:rows, :],
                                start=(it == it0 and t == 0 and first),
                                stop=(it == it1 and last),
                                skip_group_check=True)

                def emit_s(it):
                    rows = 128 if it < 16 else 49
                    X = SA if (b * NT_I + it) % 2 == 0 else SB
                    for (c0, w) in CHUNKS:
                        nc.tensor.matmul(
                            out=X[:rows, c0:c0 + w],
                            lhsT=kt[32 * b:32 * b + 16, it * 128:it * 128 + rows],
                            rhs=qt[32 * b:32 * b + 16, c0:c0 + w],
                            start=True, stop=True,
                            tile_position=(32 * b, 0), skip_group_check=True)
                    return X, rows

                Xr = {0: emit_s(0)}
                for it in range(NT_I):
                    if it + 1 < NT_I:
                        Xr[it + 1] = emit_s(it + 1)
                    if it == 0 and carry is not None:
                        carry()
                    X, rows = Xr[it]
                    e = eb.tile([128, J], BF16, tag="e")
                    z = sm.tile([128, 1], F32, tag="z")
                    nc.scalar.activation(out=e[:rows, :], in_=X[:rows, 0:1568],
                                         func=AF.Exp, scale=SCALE,
                                         accum_out=z[:rows, :])
                    r = sm.tile([128, 1], F32, tag="r")
                    nc.vector.reciprocal(out=r[:rows, :], in_=z[:rows, :])
                    kp = kpl.tile([128, 16], BF16, tag="kp")
                    nc.vector.tensor_scalar_mul(out=kp[:rows, :],
                                                in0=k_sb[:rows, b, it, :],
                                                scalar1=r[:rows, :])
                    e_t[it] = (e, rows)
                    kp_t[it] = kp
                    if it == 9:
                        emit_out_block(0, 8, True, True)
                        oa = pb.tile([128, 208], F32, tag="oa")
                        nc.vector.tensor_copy(out=oa, in_=PO)
                        oa_box["oa"] = oa
                    if it > 0:
                        for th in inject.get(it - 1, []):
                            th()
                for th in inject.get(NT_I - 1, []):
                    th()

                def _carry():
                    emit_out_block(9, NT_I - 1, True, True)
                    fo_sb = pb.tile([128, NT_J, 16], BF16, tag="fos")
                    nc.vector.tensor_add(
                        out=fo_sb,
                        in0=oa_box["oa"].rearrange("p (t e) -> p t e", e=16),
                        in1=PO.rearrange("p (t e) -> p t e", e=16))
                    nc.sync.dma_start(
                        out=fo[b, :].rearrange("(t p e) -> p t e", p=128, e=16),
                        in_=fo_sb)
                return _carry

            # ---- schedule ----
            for blk in range(4):
                conv1_mm(0, blk)
                conv1_tr(0, blk)
            for blk in range(4):
                convq_conv(0, blk)
            kproj_h(0, 0, 0)
            kproj_h(0, 0, 1)
            kproj_h(0, 1, 0)
            kproj_h(0, 1, 1)
            qt_load(0)
            ktT_run(0, 0, 3)
            qt_tr(0, 0, 3)
            qt_tr(0, 3, 3)
            qt_tr(0, 6, 2)
            qt_tr(0, 8, 3)
            qt_tr(0, 11, 2)

            carry = None
            for b in range(NB):
                inject = {}
                if b == 0:
                    inject[0] = [lambda: ktT_run(0, 3, 3)]
                    inject[1] = [lambda: ktT_run(0, 6, 3)]
                    inject[2] = [lambda: ktT_run(0, 9, 3)]
                    inject[3] = [lambda: ktT_run(0, 12, 3)]
                    inject[4] = [lambda: ktT_run(0, 15, 1)]
                if b + 1 < NB:
                    for k, v in prep_slots(b + 1).items():
                        inject.setdefault(k, []).extend(v)
                if b >= 1:
                    for k, v in post_slots(b - 1).items():
                        inject.setdefault(k, []).extend(v)
                carry = attention(b, inject, carry)
            carry()
            xt_load(NB - 1)
            for blk in range(4):
                convout_blk(NB - 1, blk)
            f3_roundtrip(NB - 1)
            ln_a(NB - 1)
            ln_b(NB - 1)
            ln_c(NB - 1)
            ln_d(NB - 1)
            if DBG:
                nc.sync.dma_start(out=dbg_kt[:, :], in_=kt)
                nc.sync.dma_start(out=dbg_qt[:, :], in_=qt)
                nc.sync.dma_start(out=dbg_ksb[:, :, :, :], in_=k_sb)

    nc.compile()
    return nc


def _blockdiag_pm(w):
    # torch OIHW grouped weights -> partition-major block-diag [128, 9, 4, 128]
    out = np.zeros((128, 9, 4, 128), np.float32)
    for blk in range(4):
        for g in range(8):
            grp = blk * 8 + g
            for ky in range(3):
                for kx in range(3):
                    out[g * 16:(g + 1) * 16, ky * 3 + kx, blk,
                        g * 16:(g + 1) * 16] = w[grp * 16:(grp + 1) * 16,
                                                 :, ky, kx].T
    return out


def kernel(current_pose, next_pose, current_w, next_w, E_proj, rel_embedd,
           ln_gamma, ln_beta, num_iter=None):
    global _PROG
    if _PROG is None:
        _PROG = _build()

    bf = ml_dtypes.bfloat16
    cp_raw = np.ascontiguousarray(
        np.asarray(current_pose, np.float32).transpose(0, 1, 4, 2, 3)
    ).reshape(B, C, H, H)
    cp_img = np.zeros((B, C, 16, 16), np.float32)
    cp_img[:, :, 1:15, 1:15] = cp_raw
    x0_h = np.ascontiguousarray(
        cp_img.reshape(B, 4, 128, 16, 16).transpose(2, 0, 1, 3, 4)).astype(bf)
    qp_raw = np.ascontiguousarray(
        np.asarray(next_pose, np.float32).transpose(0, 1, 4, 2, 3)
    ).reshape(B, C, HO, HO)
    qp_img = np.zeros((B, C, 9, 9), np.float32)
    qp_img[:, :, 1:8, 1:8] = qp_raw
    x0q_h = np.ascontiguousarray(
        qp_img.reshape(B, 4, 128, 9, 9).transpose(2, 0, 1, 3, 4)).astype(bf)

    w1_h = _blockdiag_pm(np.asarray(current_w, np.float32)).astype(bf)
    wq_h = _blockdiag_pm(np.asarray(next_w, np.float32)).astype(bf)
    ep_h = np.ascontiguousarray(
        np.asarray(E_proj, np.float32).reshape(IN_N, 2, 98, HID)
        .transpose(2, 1, 0, 3)).astype(bf)
    rel = np.asarray(rel_embedd, np.float32)
    ident = np.eye(128, dtype=np.float32).astype(bf)

    common = {
        "w1d": w1_h, "wqd": wq_h, "epd": ep_h,
        "rel_k": np.ascontiguousarray(rel.T).astype(np.float32),
        "rel_kt": rel.astype(bf),
        "gam": np.asarray(ln_gamma, np.float32),
        "bet": np.asarray(ln_beta, np.float32),
        "ident": ident,
    }
    core_ids = list(range(8))
    in_maps = []
    for c in core_ids:
        sl = slice(c * NB, (c + 1) * NB)
        in_maps.append({**common,
                        "x0d": np.ascontiguousarray(x0_h[:, sl]),
                        "x0qd": np.ascontiguousarray(x0q_h[:, sl])})

    res = run_bass_kernel_spmd(_PROG, in_maps, core_ids)
    out = np.empty((B, J * 16), np.float32)
    for c in core_ids:
        out[c * NB:(c + 1) * NB] = res.results[c]["outy"][:, :J * 16]
    return out.reshape(B, OUT_N, HO, HO, OUT_D)


if __name__ == "__main__":
    import reference as ref
    inputs = ref.setup_inputs()
    expected = np.asarray(ref.reference(**inputs))
    actual = kernel(**{k: np.asarray(v) if not np.isscalar(v) else v
                       for k, v in inputs.items()})
    err = np.abs(actual - expected)
    sc = np.abs(expected).max()
    print("absmax err:", err.max(), "scale:", sc, "rel:", err.max() / sc)


# revision 4
# speedup vs baseline: 1.7643x; 1.0061x over previous
"""Trainium2 Bass kernel for nn_BilinearLinformerCapsuleFC (v2).

Data-parallel over batch (32 -> 4 per core x 8 cores). Single-core program:
grouped convs as block-diag matmuls (9 shifted taps, PSUM-accumulated),
Linformer key projection, column-softmax attention computed in
S^T [keys, queries] layout. The softmax exp runs as ONE whole-row
Activation instruction per i-tile (with accum_out row-sum normalizer),
double-buffered across two 4-bank PSUM halves so the Act engine (the
roofline for this problem) streams back-to-back. Out^T is accumulated
j-major (13 matmuls of 16-wide output each, nearly free on PE) directly
into a spare PSUM region, which makes the output relayout a single copy +
DMA. Prep for batches 0/1 runs up front in the still-free S-buffer banks;
prep for batches 2/3 is spread at half density over two attention windows
each; conv/LayerNorm post-processing trails one batch behind.
"""
import numpy as np
import ml_dtypes

import concourse.bass as bass
import concourse.mybir as mybir
import concourse.tile as tile
from concourse import bacc
from concourse.bass import ds
from concourse.bass_utils import run_bass_kernel_spmd

BF16 = mybir.dt.bfloat16
F32 = mybir.dt.float32
AF = mybir.ActivationFunctionType
ALU = mybir.AluOpType

B, IN_N, IN_D, H, OUT_N, OUT_D, HO, HID = 32, 32, 16, 14, 32, 16, 7, 64
C = IN_N * IN_D            # 512
NB = 4                     # batch items per core
NKEY = IN_N * HID + HO * HO  # 2097
J = OUT_N * HO * HO        # 1568
NT_I = 17                  # i tiles (16x128 + 49)
NT_J = 13                  # j tiles (12x128 + 32)
JPAD = NT_J * 2048         # 26624
EPS = 1e-5
SCALE = IN_D ** -0.5
CHUNKS = [(0, 512), (512, 512), (1024, 512), (1536, 32)]

_PROG = None


def _build():
    nc = bacc.Bacc("TRN2", target_bir_lowering=False, debug=False, num_devices=1)

    x0_d = nc.dram_tensor("x0d", [128, NB, 4, 16, 16], BF16, kind="ExternalInput")
    x0q_d = nc.dram_tensor("x0qd", [128, NB, 4, 9, 9], BF16, kind="ExternalInput")
    w1_d = nc.dram_tensor("w1d", [128, 9, 4, 128], BF16, kind="ExternalInput")
    wq_d = nc.dram_tensor("wqd", [128, 9, 4, 128], BF16, kind="ExternalInput")
    eps_d = nc.dram_tensor("epd", [98, 2, IN_N, HID], BF16, kind="ExternalInput")
    rel_k_d = nc.dram_tensor("rel_k", [49, 16], F32, kind="ExternalInput")
    rel_kt_d = nc.dram_tensor("rel_kt", [16, 49], BF16, kind="ExternalInput")
    gam_d = nc.dram_tensor("gam", [16], F32, kind="ExternalInput")
    bet_d = nc.dram_tensor("bet", [16], F32, kind="ExternalInput")
    ident_d = nc.dram_tensor("ident", [128, 128], BF16, kind="ExternalInput")

    import os
    DBG = os.environ.get("K2_DEBUG", "") == "1"
    okind = {"kind": "ExternalOutput"} if DBG else {}
    fq = nc.dram_tensor("fq", [NB, JPAD], BF16)
    fo = nc.dram_tensor("fo", [NB, JPAD], BF16, **okind)
    f3 = nc.dram_tensor("f3", [NB, JPAD], F32, **okind)
    if DBG:
        dbg_kt = nc.dram_tensor("dbg_kt", [128, 2112], BF16, kind="ExternalOutput")
        dbg_qt = nc.dram_tensor("dbg_qt", [128, J], BF16, kind="ExternalOutput")
        dbg_ksb = nc.dram_tensor("dbg_ksb", [128, NB, NT_I, 16], F32,
                                 kind="ExternalOutput")
    outy = nc.dram_tensor("outy", [NB, JPAD], F32, kind="ExternalOutput")

    with tile.TileContext(nc) as tc:
        from contextlib import ExitStack
        with ExitStack() as ctx:
            consts = ctx.enter_context(tc.tile_pool(name="consts", bufs=1))
            sm = ctx.enter_context(tc.tile_pool(name="sm", bufs=6))
            eb = ctx.enter_context(tc.tile_pool(name="eb", bufs=18))
            kpl = ctx.enter_context(tc.tile_pool(name="kpl", bufs=18))
            pb = ctx.enter_context(tc.tile_pool(name="pb", bufs=3))
            pp = ctx.enter_context(tc.tile_pool(name="pp", bufs=1, space="PSUM"))

            # ---- persistent SBUF ----
            ident = consts.tile([128, 128], BF16)
            w1s = consts.tile([128, 9, 4, 128], BF16)
            wqs = consts.tile([128, 9, 4, 128], BF16)
            eps_s = consts.tile([98, 2, IN_N, HID], BF16)
            gamb = consts.tile([128, NT_J, 16], F32)
            betb = consts.tile([128, NT_J, 16], F32)
            x0 = consts.tile([128, NB, 4, 16, 16], BF16)
            x0q = consts.tile([128, NB, 4, 9, 9], BF16)
            x1 = consts.tile([98, NB, 2, C], BF16)
            kt = consts.tile([128, 2112], BF16)   # b at partition 32b, rows 0:16
            qt = consts.tile([128, J], BF16)      # b at partition 32b
            k_sb = consts.tile([128, NB, NT_I, 16], F32)

            # ---- input DMAs (SP + Act queues; HWDGE is shared anyway) ----
            nc.sync.dma_start(out=x0[:, 0:2, :, :, :], in_=x0_d[:, 0:2, :, :, :])
            nc.scalar.dma_start(out=w1s, in_=w1_d[:, :, :, :])
            nc.sync.dma_start(out=ident, in_=ident_d[:, :])
            nc.sync.dma_start(out=x0q[:, 0:2, :, :, :], in_=x0q_d[:, 0:2, :, :, :])
            nc.sync.dma_start(out=wqs, in_=wq_d[:, :, :, :])
            nc.scalar.dma_start(out=eps_s, in_=eps_d[:, :, :, :])
            nc.sync.dma_start(out=x0[:, 2:, :, :, :], in_=x0_d[:, 2:, :, :, :])
            nc.sync.dma_start(out=x0q[:, 2:, :, :, :], in_=x0q_d[:, 2:, :, :, :])
            nc.sync.dma_start(out=k_sb[0:49, :, 16, :], in_=bass.AP(
                tensor=rel_k_d, offset=0, ap=[[16, 49], [0, NB], [1, 16]]))
            for b in range(NB):
                nc.sync.dma_start(out=kt[32 * b:32 * b + 16, 2048:2097],
                                  in_=rel_kt_d[:, :])
            nc.sync.dma_start(out=gamb, in_=bass.AP(
                tensor=gam_d, offset=0, ap=[[0, 128], [0, NT_J], [1, 16]]))
            nc.sync.dma_start(out=betb, in_=bass.AP(
                tensor=bet_d, offset=0, ap=[[0, 128], [0, NT_J], [1, 16]]))

            # ---- PSUM map: 8 banks total ----
            SA = pp.tile([128, 2048], F32)   # banks 0-3
            SB = pp.tile([128, 2048], F32)   # banks 4-7
            PO = SA[:, 1792:2000]            # Out^T accumulator (bank 3 spare)
            rot = [SB[:, 1568:1764], SB[:, 1764:1960]]    # conv scratch (bank 7)
            pco_r = SB[:, 1960:2009]                      # convout scratch
            ptx = [SA[:, 1568:1632].bitcast(BF16),        # transpose scratch
                   SA[:, 1632:1696].bitcast(BF16),
                   SA[:, 1696:1760].bitcast(BF16)]
            ptw = SA[:, 1568:1760].bitcast(BF16)          # all 3, adjacent

            # ---- prep stages (per batch) ----
            kbf_st = {}
            xqc_st = {}
            x1c_st = {}

            def conv1_mm(b, blk, pc=None):
                if pc is None:
                    pc = rot[0]
                for tap in range(9):
                    ky, kx = tap // 3, tap % 3
                    nc.tensor.matmul(out=pc, lhsT=w1s[:, tap, blk, :],
                                     rhs=x0[:, b, blk, ky:ky + 14, kx:kx + 14],
                                     start=(tap == 0), stop=(tap == 8),
                                     skip_group_check=True)
                x1c = sm.tile([128, 196], BF16, tag="x1c")
                nc.vector.tensor_copy(out=x1c, in_=pc)
                x1c_st[(b, blk)] = x1c

            def conv1_tr(b, blk):
                x1c = x1c_st[(b, blk)]
                for hf in range(2):
                    pt = ptx[hf][0:98, :]
                    nc.tensor.transpose(pt, x1c[:, hf * 98:(hf + 1) * 98], ident)
                nc.vector.tensor_copy(
                    out=x1[:, b, :, blk * 128:(blk + 1) * 128],
                    in_=ptw[0:98, 0:256].rearrange("p (h c) -> p h c", h=2))

            def kproj_h(b, m, half, pkf=None):
                if pkf is None:
                    pkf = rot[m % 2]
                for q in range(half * 8, half * 8 + 8):
                    n2 = m * 16 + q
                    po = 64 * (q % 2)
                    psl = pkf[po:po + 64, (q // 2) * 16:(q // 2) * 16 + 16]
                    tp = (0, 64) if (q % 2) else (0, 0)
                    for hf in range(2):
                        nc.tensor.matmul(out=psl, lhsT=eps_s[:, hf, n2, :],
                                         rhs=x1[:, b, hf, n2::32],
                                         start=(hf == 0), stop=(hf == 1),
                                         tile_position=tp, skip_group_check=True)
                if half == 1:
                    nc.vector.tensor_copy(
                        out=k_sb[:, b, m * 8:(m + 1) * 8, :],
                        in_=pkf[:, 0:128].rearrange("p (t e) -> p t e", e=16))

            def kbf_stage(b, h):
                kbf = sm.tile([128, 8, 16], BF16, tag="kbf")
                nc.vector.tensor_copy(
                    out=kbf, in_=k_sb[:, b, h * 8:(h + 1) * 8, :])
                kbf_st[(b, h)] = kbf

            def ktT_run(b, t0, nt):
                # nt <= 3 tiles; one batched copy from the adjacent scratch
                for i, t in enumerate(range(t0, t0 + nt)):
                    h, u = divmod(t, 8)
                    if (b, h) not in kbf_st:
                        kbf_stage(b, h)
                    kbf = kbf_st[(b, h)]
                    pt = ptx[i][0:16, :]
                    nc.tensor.transpose(pt, kbf[:, u, :], ident)
                nc.vector.tensor_copy(
                    out=kt[32 * b:32 * b + 16, t0 * 128:(t0 + nt) * 128],
                    in_=ptw[0:16, 0:nt * 128])

            def convq_conv(b, blk, pcq=None):
                if pcq is None:
                    pcq = rot[blk % 2][:, 0:49]
                for tap in range(9):
                    ky, kx = tap // 3, tap % 3
                    nc.tensor.matmul(out=pcq, lhsT=wqs[:, tap, blk, :],
                                     rhs=x0q[:, b, blk, ky:ky + 7, kx:kx + 7],
                                     start=(tap == 0), stop=(tap == 8),
                                     skip_group_check=True)
                xqc = sm.tile([128, 49], BF16, tag="xqc")
                nc.vector.tensor_copy(out=xqc, in_=pcq)
                nc.sync.dma_start(
                    out=fq[b, ds(blk * 6272, 6272)].rearrange(
                        "(p s) -> p s", p=128),
                    in_=xqc)
                xqc_st[(b, blk)] = xqc

            qall_st = {}

            def qt_load(b):
                q_all = pb.tile([128, NT_J, 16], BF16, tag="qall")
                nc.sync.dma_start(
                    out=q_all,
                    in_=fq[b, :].rearrange("(p t e) -> p t e", p=128, e=16))
                qall_st[b] = q_all

            def qt_tr(b, t0, nt):
                # nt <= 3 tiles with equal row counts; one strided copy
                q_all = qall_st[b]
                rows = (J - 1 - t0) // NT_J + 1
                for i, t in enumerate(range(t0, t0 + nt)):
                    pt = ptx[i][0:16, :]
                    nc.tensor.transpose(pt[:, :rows], q_all[:rows, t, :],
                                        ident[:rows, :rows])
                src_v = ptw[0:16, 0:3 * 128].rearrange(
                    "p (i c) -> p i c", i=3)[:, 0:nt, 0:rows]
                dst = qt[32 * b:32 * b + 16, t0:]
                dst_v = bass.AP(tensor=dst.tensor, offset=dst.offset,
                                ap=[dst.ap[0], [1, nt], [NT_J, rows]])
                nc.vector.tensor_copy(out=dst_v, in_=src_v)

            # ---- post stages (convout + LN for batch b) ----
            post_state = {}

            def xt_load(b):
                x2p = pb.tile([128, 4, 9, 9], BF16, tag="x2p")
                nc.gpsimd.memset(x2p, 0.0)
                for blk in range(4):
                    nc.sync.dma_start(out=x2p[:, blk, 1:8, 1:8], in_=bass.AP(
                        tensor=fo, offset=b * JPAD + blk * 6272,
                        ap=[[49, 128], [7, 7], [1, 7]]))
                x3c = pb.tile([128, 4, 49], F32, tag="x3c")
                post_state[b] = {"x2p": x2p, "x3c": x3c}

            def convout_blk(b, blk):
                st = post_state[b]
                for tap in range(9):
                    ky, kx = tap // 3, tap % 3
                    nc.tensor.matmul(out=pco_r, lhsT=wqs[:, tap, blk, :],
                                     rhs=st["x2p"][:, blk, ky:ky + 7, kx:kx + 7],
                                     start=(tap == 0), stop=(tap == 8),
                                     skip_group_check=True)
                nc.vector.tensor_copy(out=st["x3c"][:, blk, :], in_=pco_r)

            def f3_roundtrip(b):
                st = post_state[b]
                nc.sync.dma_start(out=bass.AP(
                    tensor=f3, offset=b * JPAD, ap=[[49, 128], [6272, 4], [1, 49]]),
                    in_=st["x3c"])
                y = pb.tile([128, NT_J, 16], F32, tag="y")
                nc.sync.dma_start(out=y, in_=f3[b, :].rearrange(
                    "(p t e) -> p t e", p=128, e=16))
                st["y"] = y

            def ln_a(b):
                st = post_state[b]
                y = st["y"]
                sums = pb.tile([128, NT_J], F32, tag="sums")
                nc.vector.tensor_reduce(out=sums, in_=y,
                                        axis=mybir.AxisListType.X, op=ALU.add)
                sq = pb.tile([128, NT_J, 16], F32, tag="sq")
                nc.gpsimd.tensor_mul(out=sq, in0=y, in1=y)
                sqs = pb.tile([128, NT_J], F32, tag="sqs")
                nc.vector.tensor_reduce(out=sqs, in_=sq,
                                        axis=mybir.AxisListType.X, op=ALU.add)
                st["sums"], st["sqs"] = sums, sqs

            def ln_b(b):
                st = post_state[b]
                mu = pb.tile([128, NT_J], F32, tag="mu")
                nc.vector.tensor_scalar_mul(out=mu, in0=st["sums"],
                                            scalar1=1.0 / 16)
                msq = pb.tile([128, NT_J], F32, tag="msq")
                nc.vector.tensor_mul(out=msq, in0=mu, in1=mu)
                vpe = pb.tile([128, NT_J], F32, tag="vpe")
                nc.vector.scalar_tensor_tensor(out=vpe, in0=st["sqs"],
                                               scalar=1.0 / 16, in1=msq,
                                               op0=ALU.mult, op1=ALU.subtract)
                nc.vector.tensor_scalar_add(out=vpe, in0=vpe, scalar1=EPS)
                rstd = pb.tile([128, NT_J], F32, tag="rstd")
                ri = rstd[:, :].bitcast(mybir.dt.int32)
                nc.vector.tensor_scalar(
                    out=ri, in0=vpe[:, :].bitcast(mybir.dt.int32), scalar1=1,
                    scalar2=None, op0=ALU.logical_shift_right)
                nc.vector.tensor_scalar(
                    out=ri, in0=ri, scalar1=-1, scalar2=0x5F3759DF,
                    op0=ALU.mult, op1=ALU.add)
                tnw = pb.tile([128, NT_J], F32, tag="tnw")
                for _ in range(2):
                    nc.vector.tensor_mul(out=tnw, in0=rstd, in1=rstd)
                    nc.vector.tensor_mul(out=tnw, in0=tnw, in1=vpe)
                    nc.vector.tensor_scalar(
                        out=tnw, in0=tnw, scalar1=-0.5, scalar2=1.5,
                        op0=ALU.mult, op1=ALU.add)
                    nc.vector.tensor_mul(out=rstd, in0=rstd, in1=tnw)
                st["mu"], st["rstd"] = mu, rstd

            def ln_c(b):
                st = post_state[b]
                yn = pb.tile([128, NT_J, 16], F32, tag="yn")
                for jt in range(NT_J):
                    eng = nc.gpsimd
                    eng.tensor_scalar(
                        out=yn[:, jt, :], in0=st["y"][:, jt, :],
                        scalar1=st["mu"][:, jt:jt + 1],
                        scalar2=st["rstd"][:, jt:jt + 1],
                        op0=ALU.subtract, op1=ALU.mult)
                st["yn"] = yn

            def ln_d(b):
                st = post_state[b]
                yn = st["yn"]
                nc.vector.tensor_mul(out=yn, in0=yn, in1=gamb)
                nc.vector.tensor_add(out=yn, in0=yn, in1=betb)
                nc.sync.dma_start(
                    out=outy[b, :].rearrange("(p t e) -> p t e", p=128, e=16),
                    in_=yn)

            def post_slots(b):
                return {0: [lambda: xt_load(b)],
                        2: [lambda: convout_blk(b, 0)],
                        4: [lambda: convout_blk(b, 1)],
                        6: [lambda: convout_blk(b, 2)],
                        8: [lambda: convout_blk(b, 3)],
                        11: [lambda: f3_roundtrip(b)],
                        12: [lambda: ln_a(b)],
                        13: [lambda: ln_b(b)],
                        14: [lambda: ln_c(b)],
                        15: [lambda: ln_d(b)]}

            def prep_slots(b):
                return {0: [lambda: conv1_mm(b, 0)],
                        1: [lambda: conv1_tr(b, 0), lambda: convq_conv(b, 0)],
                        2: [lambda: conv1_mm(b, 1), lambda: convq_conv(b, 1)],
                        3: [lambda: conv1_tr(b, 1), lambda: convq_conv(b, 2)],
                        4: [lambda: conv1_mm(b, 2), lambda: convq_conv(b, 3)],
                        5: [lambda: conv1_tr(b, 2), lambda: qt_load(b)],
                        6: [lambda: conv1_mm(b, 3)],
                        7: [lambda: conv1_tr(b, 3), lambda: qt_tr(b, 0, 3)],
                        8: [lambda: kproj_h(b, 0, 0), lambda: kproj_h(b, 0, 1),
                            lambda: qt_tr(b, 3, 3)],
                        9: [lambda: qt_tr(b, 6, 2), lambda: qt_tr(b, 8, 3)],
                        10: [lambda: kproj_h(b, 1, 0), lambda: kproj_h(b, 1, 1),
                             lambda: qt_tr(b, 11, 2)],
                        11: [lambda: ktT_run(b, 0, 3)],
                        12: [lambda: ktT_run(b, 3, 3)],
                        13: [lambda: ktT_run(b, 6, 3)],
                        14: [lambda: ktT_run(b, 9, 3)],
                        15: [lambda: ktT_run(b, 12, 3), lambda: ktT_run(b, 15, 1)]}

            # ---- attention ----
            def attention(b, inject, carry):
                e_t, kp_t = {}, {}
                oa_box = {}

                def emit_out_block(it0, it1, first, last):
                    # contiguous in the PE stream => the PSUM zero-region
                    # cannot be poisoned mid-accumulation
                    for it in range(it0, it1 + 1):
                        e, rows = e_t[it]
                        kp = kp_t[it]
                        for t in range(NT_J):
                            cols = 128 if t < 12 else 32
                            nc.tensor.matmul(
                                out=PO[0:cols, t * 16:(t + 1) * 16],
                                lhsT=e[:rows, t * 128:t * 128 + cols],
                                rhs=kp[:rows, :],
                                start=(it == it0 and t == 0 and first),
                                stop=(it == it1 and last),
                                skip_group_check=True)

                def emit_s(it):
                    rows = 128 if it < 16 else 49
                    X = SA if (b * NT_I + it) % 2 == 0 else SB
                    for (c0, w) in CHUNKS:
                        nc.tensor.matmul(
                            out=X[:rows, c0:c0 + w],
                            lhsT=kt[32 * b:32 * b + 16, it * 128:it * 128 + rows],
                            rhs=qt[32 * b:32 * b + 16, c0:c0 + w],
                            start=True, stop=True,
                            tile_position=(32 * b, 0), skip_group_check=True)
                    return X, rows

                Xr = {0: emit_s(0)}
                for it in range(NT_I):
                    if it + 1 < NT_I:
                        Xr[it + 1] = emit_s(it + 1)
                    if it == 0 and carry is not None:
                        carry()
                    X, rows = Xr[it]
                    e = eb.tile([128, J], BF16, tag="e")
                    z = sm.tile([128, 1], F32, tag="z")
                    nc.scalar.activation(out=e[:rows, :], in_=X[:rows, 0:1568],
                                         func=AF.Exp, scale=SCALE,
                                         accum_out=z[:rows, :])
                    r = sm.tile([128, 1], F32, tag="r")
                    nc.vector.reciprocal(out=r[:rows, :], in_=z[:rows, :])
                    kp = kpl.tile([128, 16], BF16, tag="kp")
                    nc.vector.tensor_scalar_mul(out=kp[:rows, :],
                                                in0=k_sb[:rows, b, it, :],
                                                scalar1=r[:rows, :])
                    e_t[it] = (e, rows)
                    kp_t[it] = kp
                    if it == 9:
                        emit_out_block(0, 8, True, True)
                        oa = pb.tile([128, 208], F32, tag="oa")
                        nc.vector.tensor_copy(out=oa, in_=PO)
                        oa_box["oa"] = oa
                    if it > 0:
                        for th in inject.get(it - 1, []):
                            th()
                for th in inject.get(NT_I - 1, []):
                    th()

                def _carry():
                    emit_out_block(9, NT_I - 1, True, True)
                    fo_sb = pb.tile([128, NT_J, 16], BF16, tag="fos")
                    nc.vector.tensor_add(
                        out=fo_sb,
                        in0=oa_box["oa"].rearrange("p (t e) -> p t e", e=16),
                        in1=PO.rearrange("p (t e) -> p t e", e=16))
                    nc.sync.dma_start(
                        out=fo[b, :].rearrange("(t p e) -> p t e", p=128, e=16),
                        in_=fo_sb)
                return _carry

            # ---- schedule ----
            for blk in range(4):
                conv1_mm(0, blk)
                conv1_tr(0, blk)
            for blk in range(4):
                convq_conv(0, blk)
            kproj_h(0, 0, 0)
            kproj_h(0, 0, 1)
            kproj_h(0, 1, 0)
            kproj_h(0, 1, 1)
            qt_load(0)
            ktT_run(0, 0, 3)
            qt_tr(0, 0, 3)
            qt_tr(0, 3, 3)
            qt_tr(0, 6, 2)
            qt_tr(0, 8, 3)
            qt_tr(0, 11, 2)

            carry = None
            for b in range(NB):
                inject = {}
                if b == 0:
                    inject[0] = [lambda: ktT_run(0, 3, 3)]
                    inject[1] = [lambda: ktT_run(0, 6, 3)]
                    inject[2] = [lambda: ktT_run(0, 9, 3)]
                    inject[3] = [lambda: ktT_run(0, 12, 3)]
                    inject[4] = [lambda: ktT_run(0, 15, 1)]
                if b + 1 < NB:
                    for k, v in prep_slots(b + 1).items():
                        inject.setdefault(k, []).extend(v)
                if b >= 1:
                    for k, v in post_slots(b - 1).items():
                        inject.setdefault(k, []).extend(v)
                carry = attention(b, inject, carry)
            carry()
            xt_load(NB - 1)
            for blk in range(4):
                convout_blk(NB - 1, blk)
            f3_roundtrip(NB - 1)
            ln_a(NB - 1)
            ln_b(NB - 1)
            ln_c(NB - 1)
            ln_d(NB - 1)
            if DBG:
                nc.sync.dma_start(out=dbg_kt[:, :], in_=kt)
                nc.sync.dma_start(out=dbg_qt[:, :], in_=qt)
                nc.sync.dma_start(out=dbg_ksb[:, :, :, :], in_=k_sb)

    nc.compile()
    return nc


def _blockdiag_pm(w):
    # torch OIHW grouped weights -> partition-major block-diag [128, 9, 4, 128]
    out = np.zeros((128, 9, 4, 128), np.float32)
    for blk in range(4):
        for g in range(8):
            grp = blk * 8 + g
            for ky in range(3):
                for kx in range(3):
                    out[g * 16:(g + 1) * 16, ky * 3 + kx, blk,
                        g * 16:(g + 1) * 16] = w[grp * 16:(grp + 1) * 16,
                                                 :, ky, kx].T
    return out


def kernel(current_pose, next_pose, current_w, next_w, E_proj, rel_embedd,
           ln_gamma, ln_beta, num_iter=None):
    global _PROG
    if _PROG is None:
        _PROG = _build()

    bf = ml_dtypes.bfloat16
    cp_raw = np.ascontiguousarray(
        np.asarray(current_pose, np.float32).transpose(0, 1, 4, 2, 3)
    ).reshape(B, C, H, H)
    cp_img = np.zeros((B, C, 16, 16), np.float32)
    cp_img[:, :, 1:15, 1:15] = cp_raw
    x0_h = np.ascontiguousarray(
        cp_img.reshape(B, 4, 128, 16, 16).transpose(2, 0, 1, 3, 4)).astype(bf)
    qp_raw = np.ascontiguousarray(
        np.asarray(next_pose, np.float32).transpose(0, 1, 4, 2, 3)
    ).reshape(B, C, HO, HO)
    qp_img = np.zeros((B, C, 9, 9), np.float32)
    qp_img[:, :, 1:8, 1:8] = qp_raw
    x0q_h = np.ascontiguousarray(
        qp_img.reshape(B, 4, 128, 9, 9).transpose(2, 0, 1, 3, 4)).astype(bf)

    w1_h = _blockdiag_pm(np.asarray(current_w, np.float32)).astype(bf)
    wq_h = _blockdiag_pm(np.asarray(next_w, np.float32)).astype(bf)
    ep_h = np.ascontiguousarray(
        np.asarray(E_proj, np.float32).reshape(IN_N, 2, 98, HID)
        .transpose(2, 1, 0, 3)).astype(bf)
    rel = np.asarray(rel_embedd, np.float32)
    ident = np.eye(128, dtype=np.float32).astype(bf)

    common = {
        "w1d": w1_h, "wqd": wq_h, "epd": ep_h,
        "rel_k": np.ascontiguousarray(rel.T).astype(np.float32),
        "rel_kt": rel.astype(bf),
        "gam": np.asarray(ln_gamma, np.float32),
        "bet": np.asarray(ln_beta, np.float32),
        "ident": ident,
    }
    core_ids = list(range(8))
    in_maps = []
    for c in core_ids:
        sl = slice(c * NB, (c + 1) * NB)
        in_maps.append({**common,
                        "x0d": np.ascontiguousarray(x0_h[:, sl]),
                        "x0qd": np.ascontiguousarray(x0q_h[:, sl])})

    res = run_bass_kernel_spmd(_PROG, in_maps, core_ids)
    out = np.empty((B, J * 16), np.float32)
    for c in core_ids:
        out[c * NB:(c + 1) * NB] = res.results[c]["outy"][:, :J * 16]
    return out.reshape(B, OUT_N, HO, HO, OUT_D)


if __name__ == "__main__":
    import reference as ref
    inputs = ref.setup_inputs()
    expected = np.asarray(ref.reference(**inputs))
    actual = kernel(**{k: np.asarray(v) if not np.isscalar(v) else v
                       for k, v in inputs.items()})
    err = np.abs(actual - expected)
    sc = np.abs(expected).max()
    print("absmax err:", err.max(), "scale:", sc, "rel:", err.max() / sc)


# revision 5
# speedup vs baseline: 1.7881x; 1.0135x over previous
"""Trainium2 Bass kernel for nn_BilinearLinformerCapsuleFC (v2).

Data-parallel over batch (32 -> 4 per core x 8 cores). Single-core program:
grouped convs as block-diag matmuls (9 shifted taps, PSUM-accumulated),
Linformer key projection, column-softmax attention computed in
S^T [keys, queries] layout. The softmax exp runs as ONE whole-row
Activation instruction per i-tile (with accum_out row-sum normalizer),
double-buffered across two 4-bank PSUM halves so the Act engine (the
roofline for this problem) streams back-to-back. Out^T is accumulated
j-major (13 matmuls of 16-wide output each, nearly free on PE) directly
into a spare PSUM region, which makes the output relayout a single copy +
DMA. Prep for batches 0/1 runs up front in the still-free S-buffer banks;
prep for batches 2/3 is spread at half density over two attention windows
each; conv/LayerNorm post-processing trails one batch behind.
"""
import numpy as np
import ml_dtypes

import concourse.bass as bass
import concourse.mybir as mybir
import concourse.tile as tile
from concourse import bacc
from concourse.bass import ds
from concourse.bass_utils import run_bass_kernel_spmd

BF16 = mybir.dt.bfloat16
F32 = mybir.dt.float32
AF = mybir.ActivationFunctionType
ALU = mybir.AluOpType

B, IN_N, IN_D, H, OUT_N, OUT_D, HO, HID = 32, 32, 16, 14, 32, 16, 7, 64
C = IN_N * IN_D            # 512
NB = 4                     # batch items per core
NKEY = IN_N * HID + HO * HO  # 2097
J = OUT_N * HO * HO        # 1568
NT_I = 17                  # i tiles (16x128 + 49)
NT_J = 13                  # j tiles (12x128 + 32)
JPAD = NT_J * 2048         # 26624
EPS = 1e-5
SCALE = IN_D ** -0.5
CHUNKS = [(0, 512), (512, 512), (1024, 512), (1536, 32)]

_PROG = None


def _build():
    nc = bacc.Bacc("TRN2", target_bir_lowering=False, debug=False, num_devices=1)

    x0_d = nc.dram_tensor("x0d", [128, NB, 4, 16, 16], BF16, kind="ExternalInput")
    x0q_d = nc.dram_tensor("x0qd", [128, NB, 4, 9, 9], BF16, kind="ExternalInput")
    w1_d = nc.dram_tensor("w1d", [128, 9, 4, 128], BF16, kind="ExternalInput")
    wq_d = nc.dram_tensor("wqd", [128, 9, 4, 128], BF16, kind="ExternalInput")
    eps_d = nc.dram_tensor("epd", [98, 2, IN_N, HID], BF16, kind="ExternalInput")
    rel_k_d = nc.dram_tensor("rel_k", [49, 16], F32, kind="ExternalInput")
    rel_kt_d = nc.dram_tensor("rel_kt", [16, 49], BF16, kind="ExternalInput")
    gam_d = nc.dram_tensor("gam", [16], F32, kind="ExternalInput")
    bet_d = nc.dram_tensor("bet", [16], F32, kind="ExternalInput")
    ident_d = nc.dram_tensor("ident", [128, 128], BF16, kind="ExternalInput")

    import os
    DBG = os.environ.get("K2_DEBUG", "") == "1"
    okind = {"kind": "ExternalOutput"} if DBG else {}
    fq = nc.dram_tensor("fq", [NB, JPAD], BF16)
    fo = nc.dram_tensor("fo", [NB, JPAD], BF16, **okind)
    f3 = nc.dram_tensor("f3", [NB, JPAD], F32, **okind)
    if DBG:
        dbg_kt = nc.dram_tensor("dbg_kt", [128, 2112], BF16, kind="ExternalOutput")
        dbg_qt = nc.dram_tensor("dbg_qt", [128, J], BF16, kind="ExternalOutput")
        dbg_ksb = nc.dram_tensor("dbg_ksb", [128, NB, NT_I, 16], F32,
                                 kind="ExternalOutput")
    outy = nc.dram_tensor("outy", [NB, JPAD], F32, kind="ExternalOutput")

    with tile.TileContext(nc) as tc:
        from contextlib import ExitStack
        with ExitStack() as ctx:
            consts = ctx.enter_context(tc.tile_pool(name="consts", bufs=1))
            sm = ctx.enter_context(tc.tile_pool(name="sm", bufs=6))
            eb = ctx.enter_context(tc.tile_pool(name="eb", bufs=18))
            kpl = ctx.enter_context(tc.tile_pool(name="kpl", bufs=18))
            pb = ctx.enter_context(tc.tile_pool(name="pb", bufs=3))
            pp = ctx.enter_context(tc.tile_pool(name="pp", bufs=1, space="PSUM"))

            # ---- persistent SBUF ----
            ident = consts.tile([128, 128], BF16)
            w1s = consts.tile([128, 9, 4, 128], BF16)
            wqs = consts.tile([128, 9, 4, 128], BF16)
            eps_s = consts.tile([98, 2, IN_N, HID], BF16)
            gamb = consts.tile([128, NT_J, 16], F32)
            betb = consts.tile([128, NT_J, 16], F32)
            x0 = consts.tile([128, NB, 4, 16, 16], BF16)
            x0q = consts.tile([128, NB, 4, 9, 9], BF16)
            x1 = consts.tile([98, NB, 2, C], BF16)
            kt = consts.tile([128, 2112], BF16)   # b at partition 32b, rows 0:16
            qt = consts.tile([128, J], BF16)      # b at partition 32b
            k_sb = consts.tile([128, NB, NT_I, 16], F32)

            # ---- input DMAs (SP + Act queues; HWDGE is shared anyway) ----
            nc.sync.dma_start(out=x0[:, 0:2, :, :, :], in_=x0_d[:, 0:2, :, :, :])
            nc.scalar.dma_start(out=w1s, in_=w1_d[:, :, :, :])
            nc.sync.dma_start(out=ident, in_=ident_d[:, :])
            nc.sync.dma_start(out=x0q[:, 0:2, :, :, :], in_=x0q_d[:, 0:2, :, :, :])
            nc.sync.dma_start(out=wqs, in_=wq_d[:, :, :, :])
            nc.scalar.dma_start(out=eps_s, in_=eps_d[:, :, :, :])
            nc.sync.dma_start(out=x0[:, 2:, :, :, :], in_=x0_d[:, 2:, :, :, :])
            nc.sync.dma_start(out=x0q[:, 2:, :, :, :], in_=x0q_d[:, 2:, :, :, :])
            nc.sync.dma_start(out=k_sb[0:49, :, 16, :], in_=bass.AP(
                tensor=rel_k_d, offset=0, ap=[[16, 49], [0, NB], [1, 16]]))
            for b in range(NB):
                nc.sync.dma_start(out=kt[32 * b:32 * b + 16, 2048:2097],
                                  in_=rel_kt_d[:, :])
            nc.sync.dma_start(out=gamb, in_=bass.AP(
                tensor=gam_d, offset=0, ap=[[0, 128], [0, NT_J], [1, 16]]))
            nc.sync.dma_start(out=betb, in_=bass.AP(
                tensor=bet_d, offset=0, ap=[[0, 128], [0, NT_J], [1, 16]]))

            # ---- PSUM map: 8 banks total ----
            SA = pp.tile([128, 2048], F32)   # banks 0-3
            SB = pp.tile([128, 2048], F32)   # banks 4-7
            PO = SA[:, 1792:2000]            # Out^T accumulator (bank 3 spare)
            rot = [SB[:, 1568:1764], SB[:, 1764:1960]]    # conv scratch (bank 7)
            pco_r = SB[:, 1960:2009]                      # convout scratch
            ptx = [SA[:, 1568:1632].bitcast(BF16),        # transpose scratch
                   SA[:, 1632:1696].bitcast(BF16),
                   SA[:, 1696:1760].bitcast(BF16)]
            ptw = SA[:, 1568:1760].bitcast(BF16)          # all 3, adjacent
            rot_bf = SB[:, 1568:1764].bitcast(BF16)       # alt transpose scratch

            # ---- prep stages (per batch) ----
            kbf_st = {}
            xqc_st = {}
            x1c_st = {}

            def conv1_mm(b, blk, pc=None):
                if pc is None:
                    pc = rot[0]
                for tap in range(9):
                    ky, kx = tap // 3, tap % 3
                    nc.tensor.matmul(out=pc, lhsT=w1s[:, tap, blk, :],
                                     rhs=x0[:, b, blk, ky:ky + 14, kx:kx + 14],
                                     start=(tap == 0), stop=(tap == 8),
                                     skip_group_check=True)
                x1c = sm.tile([128, 196], BF16, tag="x1c")
                nc.vector.tensor_copy(out=x1c, in_=pc)
                x1c_st[(b, blk)] = x1c

            def conv1_tr(b, blk):
                x1c = x1c_st[(b, blk)]
                for hf in range(2):
                    pt = ptx[hf][0:98, :]
                    nc.tensor.transpose(pt, x1c[:, hf * 98:(hf + 1) * 98], ident)
                nc.vector.tensor_copy(
                    out=x1[:, b, :, blk * 128:(blk + 1) * 128],
                    in_=ptw[0:98, 0:256].rearrange("p (h c) -> p h c", h=2))

            def kproj_h(b, m, half, pkf=None):
                if pkf is None:
                    pkf = rot[m % 2]
                for q in range(half * 8, half * 8 + 8):
                    n2 = m * 16 + q
                    po = 64 * (q % 2)
                    psl = pkf[po:po + 64, (q // 2) * 16:(q // 2) * 16 + 16]
                    tp = (0, 64) if (q % 2) else (0, 0)
                    for hf in range(2):
                        nc.tensor.matmul(out=psl, lhsT=eps_s[:, hf, n2, :],
                                         rhs=x1[:, b, hf, n2::32],
                                         start=(hf == 0), stop=(hf == 1),
                                         tile_position=tp, skip_group_check=True)
                if half == 1:
                    nc.vector.tensor_copy(
                        out=k_sb[:, b, m * 8:(m + 1) * 8, :],
                        in_=pkf[:, 0:128].rearrange("p (t e) -> p t e", e=16))

            def kbf_stage(b, h):
                kbf = sm.tile([128, 8, 16], BF16, tag="kbf")
                nc.vector.tensor_copy(
                    out=kbf, in_=k_sb[:, b, h * 8:(h + 1) * 8, :])
                kbf_st[(b, h)] = kbf

            def ktT_run(b, t0, nt):
                # nt <= 3 tiles; one batched copy from the adjacent scratch
                for i, t in enumerate(range(t0, t0 + nt)):
                    h, u = divmod(t, 8)
                    if (b, h) not in kbf_st:
                        kbf_stage(b, h)
                    kbf = kbf_st[(b, h)]
                    pt = ptx[i][0:16, :]
                    nc.tensor.transpose(pt, kbf[:, u, :], ident)
                nc.vector.tensor_copy(
                    out=kt[32 * b:32 * b + 16, t0 * 128:(t0 + nt) * 128],
                    in_=ptw[0:16, 0:nt * 128])

            def convq_conv(b, blk, pcq=None):
                if pcq is None:
                    pcq = rot[blk % 2][:, 0:49]
                for tap in range(9):
                    ky, kx = tap // 3, tap % 3
                    nc.tensor.matmul(out=pcq, lhsT=wqs[:, tap, blk, :],
                                     rhs=x0q[:, b, blk, ky:ky + 7, kx:kx + 7],
                                     start=(tap == 0), stop=(tap == 8),
                                     skip_group_check=True)
                xqc = sm.tile([128, 49], BF16, tag="xqc")
                nc.vector.tensor_copy(out=xqc, in_=pcq)
                nc.sync.dma_start(
                    out=fq[b, ds(blk * 6272, 6272)].rearrange(
                        "(p s) -> p s", p=128),
                    in_=xqc)
                xqc_st[(b, blk)] = xqc

            qall_st = {}

            def qt_load(b):
                q_all = pb.tile([128, NT_J, 16], BF16, tag="qall")
                nc.sync.dma_start(
                    out=q_all,
                    in_=fq[b, :].rearrange("(p t e) -> p t e", p=128, e=16))
                qall_st[b] = q_all

            def qt_tr(b, t0, nt, sc=None):
                # nt <= 3 tiles with equal row counts; one strided copy
                if sc is None:
                    sc = ptw
                q_all = qall_st[b]
                rows = (J - 1 - t0) // NT_J + 1
                for i, t in enumerate(range(t0, t0 + nt)):
                    pt = sc[0:16, i * 128:(i + 1) * 128]
                    nc.tensor.transpose(pt[:, :rows], q_all[:rows, t, :],
                                        ident[:rows, :rows])
                src_v = sc[0:16, 0:3 * 128].rearrange(
                    "p (i c) -> p i c", i=3)[:, 0:nt, 0:rows]
                dst = qt[32 * b:32 * b + 16, t0:]
                dst_v = bass.AP(tensor=dst.tensor, offset=dst.offset,
                                ap=[dst.ap[0], [1, nt], [NT_J, rows]])
                nc.vector.tensor_copy(out=dst_v, in_=src_v)

            # ---- post stages (convout + LN for batch b) ----
            post_state = {}

            def xt_load(b):
                x2p = pb.tile([128, 4, 9, 9], BF16, tag="x2p")
                nc.gpsimd.memset(x2p, 0.0)
                for blk in range(4):
                    nc.sync.dma_start(out=x2p[:, blk, 1:8, 1:8], in_=bass.AP(
                        tensor=fo, offset=b * JPAD + blk * 6272,
                        ap=[[49, 128], [7, 7], [1, 7]]))
                x3c = pb.tile([128, 4, 49], F32, tag="x3c")
                post_state[b] = {"x2p": x2p, "x3c": x3c}

            def convout_blk(b, blk):
                st = post_state[b]
                for tap in range(9):
                    ky, kx = tap // 3, tap % 3
                    nc.tensor.matmul(out=pco_r, lhsT=wqs[:, tap, blk, :],
                                     rhs=st["x2p"][:, blk, ky:ky + 7, kx:kx + 7],
                                     start=(tap == 0), stop=(tap == 8),
                                     skip_group_check=True)
                nc.vector.tensor_copy(out=st["x3c"][:, blk, :], in_=pco_r)

            def f3_roundtrip(b):
                st = post_state[b]
                nc.sync.dma_start(out=bass.AP(
                    tensor=f3, offset=b * JPAD, ap=[[49, 128], [6272, 4], [1, 49]]),
                    in_=st["x3c"])
                y = pb.tile([128, NT_J, 16], F32, tag="y")
                nc.sync.dma_start(out=y, in_=f3[b, :].rearrange(
                    "(p t e) -> p t e", p=128, e=16))
                st["y"] = y

            def ln_a(b):
                st = post_state[b]
                y = st["y"]
                sums = pb.tile([128, NT_J], F32, tag="sums")
                nc.vector.tensor_reduce(out=sums, in_=y,
                                        axis=mybir.AxisListType.X, op=ALU.add)
                sq = pb.tile([128, NT_J, 16], F32, tag="sq")
                nc.gpsimd.tensor_mul(out=sq, in0=y, in1=y)
                sqs = pb.tile([128, NT_J], F32, tag="sqs")
                nc.vector.tensor_reduce(out=sqs, in_=sq,
                                        axis=mybir.AxisListType.X, op=ALU.add)
                st["sums"], st["sqs"] = sums, sqs

            def ln_b(b):
                st = post_state[b]
                mu = pb.tile([128, NT_J], F32, tag="mu")
                nc.vector.tensor_scalar_mul(out=mu, in0=st["sums"],
                                            scalar1=1.0 / 16)
                msq = pb.tile([128, NT_J], F32, tag="msq")
                nc.vector.tensor_mul(out=msq, in0=mu, in1=mu)
                vpe = pb.tile([128, NT_J], F32, tag="vpe")
                nc.vector.scalar_tensor_tensor(out=vpe, in0=st["sqs"],
                                               scalar=1.0 / 16, in1=msq,
                                               op0=ALU.mult, op1=ALU.subtract)
                nc.vector.tensor_scalar_add(out=vpe, in0=vpe, scalar1=EPS)
                rstd = pb.tile([128, NT_J], F32, tag="rstd")
                ri = rstd[:, :].bitcast(mybir.dt.int32)
                nc.vector.tensor_scalar(
                    out=ri, in0=vpe[:, :].bitcast(mybir.dt.int32), scalar1=1,
                    scalar2=None, op0=ALU.logical_shift_right)
                nc.vector.tensor_scalar(
                    out=ri, in0=ri, scalar1=-1, scalar2=0x5F3759DF,
                    op0=ALU.mult, op1=ALU.add)
                tnw = pb.tile([128, NT_J], F32, tag="tnw")
                for _ in range(2):
                    nc.vector.tensor_mul(out=tnw, in0=rstd, in1=rstd)
                    nc.vector.tensor_mul(out=tnw, in0=tnw, in1=vpe)
                    nc.vector.tensor_scalar(
                        out=tnw, in0=tnw, scalar1=-0.5, scalar2=1.5,
                        op0=ALU.mult, op1=ALU.add)
                    nc.vector.tensor_mul(out=rstd, in0=rstd, in1=tnw)
                st["mu"], st["rstd"] = mu, rstd

            def ln_c(b):
                st = post_state[b]
                yn = pb.tile([128, NT_J, 16], F32, tag="yn")
                for jt in range(NT_J):
                    eng = nc.gpsimd
                    eng.tensor_scalar(
                        out=yn[:, jt, :], in0=st["y"][:, jt, :],
                        scalar1=st["mu"][:, jt:jt + 1],
                        scalar2=st["rstd"][:, jt:jt + 1],
                        op0=ALU.subtract, op1=ALU.mult)
                st["yn"] = yn

            def ln_d(b):
                st = post_state[b]
                yn = st["yn"]
                nc.vector.tensor_mul(out=yn, in0=yn, in1=gamb)
                nc.vector.tensor_add(out=yn, in0=yn, in1=betb)
                nc.sync.dma_start(
                    out=outy[b, :].rearrange("(p t e) -> p t e", p=128, e=16),
                    in_=yn)

            def post_slots(b):
                return {0: [lambda: xt_load(b)],
                        2: [lambda: convout_blk(b, 0)],
                        4: [lambda: convout_blk(b, 1)],
                        6: [lambda: convout_blk(b, 2)],
                        8: [lambda: convout_blk(b, 3)],
                        11: [lambda: f3_roundtrip(b)],
                        12: [lambda: ln_a(b)],
                        13: [lambda: ln_b(b)],
                        14: [lambda: ln_c(b)],
                        15: [lambda: ln_d(b)]}

            def prep_slots(b):
                return {0: [lambda: conv1_mm(b, 0)],
                        1: [lambda: conv1_tr(b, 0), lambda: convq_conv(b, 0)],
                        2: [lambda: conv1_mm(b, 1), lambda: convq_conv(b, 1)],
                        3: [lambda: conv1_tr(b, 1), lambda: convq_conv(b, 2)],
                        4: [lambda: conv1_mm(b, 2), lambda: convq_conv(b, 3)],
                        5: [lambda: conv1_tr(b, 2), lambda: qt_load(b)],
                        6: [lambda: conv1_mm(b, 3)],
                        7: [lambda: conv1_tr(b, 3), lambda: qt_tr(b, 0, 3)],
                        8: [lambda: kproj_h(b, 0, 0), lambda: kproj_h(b, 0, 1),
                            lambda: qt_tr(b, 3, 3)],
                        9: [lambda: qt_tr(b, 6, 2), lambda: qt_tr(b, 8, 3)],
                        10: [lambda: kproj_h(b, 1, 0), lambda: kproj_h(b, 1, 1),
                             lambda: qt_tr(b, 11, 2)],
                        11: [lambda: ktT_run(b, 0, 3)],
                        12: [lambda: ktT_run(b, 3, 3)],
                        13: [lambda: ktT_run(b, 6, 3)],
                        14: [lambda: ktT_run(b, 9, 3)],
                        15: [lambda: ktT_run(b, 12, 3), lambda: ktT_run(b, 15, 1)]}

            # ---- attention ----
            def attention(b, inject, carry):
                e_t, kp_t = {}, {}
                oa_box = {}

                def emit_out_block(it0, it1, first, last):
                    # contiguous in the PE stream => the PSUM zero-region
                    # cannot be poisoned mid-accumulation
                    for it in range(it0, it1 + 1):
                        e, rows = e_t[it]
                        kp = kp_t[it]
                        for t in range(NT_J):
                            cols = 128 if t < 12 else 32
                            nc.tensor.matmul(
                                out=PO[0:cols, t * 16:(t + 1) * 16],
                                lhsT=e[:rows, t * 128:t * 128 + cols],
                                rhs=kp[:rows, :],
                                start=(it == it0 and t == 0 and first),
                                stop=(it == it1 and last),
                                skip_group_check=True)

                def emit_s(it):
                    rows = 128 if it < 16 else 49
                    X = SA if (b * NT_I + it) % 2 == 0 else SB
                    for (c0, w) in CHUNKS:
                        nc.tensor.matmul(
                            out=X[:rows, c0:c0 + w],
                            lhsT=kt[32 * b:32 * b + 16, it * 128:it * 128 + rows],
                            rhs=qt[32 * b:32 * b + 16, c0:c0 + w],
                            start=True, stop=True,
                            tile_position=(32 * b, 0), skip_group_check=True)
                    return X, rows

                Xr = {0: emit_s(0)}
                for it in range(NT_I):
                    if it + 1 < NT_I:
                        Xr[it + 1] = emit_s(it + 1)
                    if it == 0 and carry is not None:
                        carry()
                    X, rows = Xr[it]
                    e = eb.tile([128, J], BF16, tag="e")
                    z = sm.tile([128, 1], F32, tag="z")
                    nc.scalar.activation(out=e[:rows, :], in_=X[:rows, 0:1568],
                                         func=AF.Exp, scale=SCALE,
                                         accum_out=z[:rows, :])
                    r = sm.tile([128, 1], F32, tag="r")
                    nc.vector.reciprocal(out=r[:rows, :], in_=z[:rows, :])
                    kp = kpl.tile([128, 16], BF16, tag="kp")
                    nc.vector.tensor_scalar_mul(out=kp[:rows, :],
                                                in0=k_sb[:rows, b, it, :],
                                                scalar1=r[:rows, :])
                    e_t[it] = (e, rows)
                    kp_t[it] = kp
                    if it == 9:
                        emit_out_block(0, 8, True, True)
                        oa = pb.tile([128, 208], F32, tag="oa")
                        nc.vector.tensor_copy(out=oa, in_=PO)
                        oa_box["oa"] = oa
                    if it > 0:
                        for th in inject.get(it - 1, []):
                            th()
                for th in inject.get(NT_I - 1, []):
                    th()

                def _carry():
                    emit_out_block(9, NT_I - 1, True, True)
                    fo_sb = pb.tile([128, NT_J, 16], BF16, tag="fos")
                    nc.vector.tensor_add(
                        out=fo_sb,
                        in0=oa_box["oa"].rearrange("p (t e) -> p t e", e=16),
                        in1=PO.rearrange("p (t e) -> p t e", e=16))
                    nc.sync.dma_start(
                        out=fo[b, :].rearrange("(t p e) -> p t e", p=128, e=16),
                        in_=fo_sb)
                return _carry

            # ---- schedule ----
            c1r0 = [SA[:, 0:196], SA[:, 512:708],
                    SB[:, 0:196], SB[:, 512:708]]
            cqr0 = [SA[:, 1024:1073], SA[:, 1136:1185],
                    SB[:, 1024:1073], SB[:, 1136:1185]]
            for blk in range(4):
                conv1_mm(0, blk, pc=c1r0[blk])
            for blk in range(4):
                convq_conv(0, blk, pcq=cqr0[blk])
                conv1_tr(0, blk)
            qt_load(0)
            kproj_h(0, 0, 0)
            kproj_h(0, 0, 1)
            kproj_h(0, 1, 0)
            kproj_h(0, 1, 1)
            ktT_run(0, 0, 3)
            qt_tr(0, 0, 3)
            qt_tr(0, 3, 3, sc=rot_bf)
            qt_tr(0, 6, 2)
            qt_tr(0, 8, 3, sc=rot_bf)
            qt_tr(0, 11, 2)

            carry = None
            for b in range(NB):
                inject = {}
                if b == 0:
                    inject[0] = [lambda: ktT_run(0, 3, 3)]
                    inject[1] = [lambda: ktT_run(0, 6, 3)]
                    inject[2] = [lambda: ktT_run(0, 9, 3)]
                    inject[3] = [lambda: ktT_run(0, 12, 3)]
                    inject[4] = [lambda: ktT_run(0, 15, 1)]
                if b + 1 < NB:
                    for k, v in prep_slots(b + 1).items():
                        inject.setdefault(k, []).extend(v)
                if b >= 1:
                    for k, v in post_slots(b - 1).items():
                        inject.setdefault(k, []).extend(v)
                carry = attention(b, inject, carry)
            carry()
            xt_load(NB - 1)
            for blk in range(4):
                convout_blk(NB - 1, blk)
            f3_roundtrip(NB - 1)
            ln_a(NB - 1)
            ln_b(NB - 1)
            ln_c(NB - 1)
            ln_d(NB - 1)
            if DBG:
                nc.sync.dma_start(out=dbg_kt[:, :], in_=kt)
                nc.sync.dma_start(out=dbg_qt[:, :], in_=qt)
                nc.sync.dma_start(out=dbg_ksb[:, :, :, :], in_=k_sb)

    nc.compile()
    return nc


def _blockdiag_pm(w):
    # torch OIHW grouped weights -> partition-major block-diag [128, 9, 4, 128]
    out = np.zeros((128, 9, 4, 128), np.float32)
    for blk in range(4):
        for g in range(8):
            grp = blk * 8 + g
            for ky in range(3):
                for kx in range(3):
                    out[g * 16:(g + 1) * 16, ky * 3 + kx, blk,
                        g * 16:(g + 1) * 16] = w[grp * 16:(grp + 1) * 16,
                                                 :, ky, kx].T
    return out


def kernel(current_pose, next_pose, current_w, next_w, E_proj, rel_embedd,
           ln_gamma, ln_beta, num_iter=None):
    global _PROG
    if _PROG is None:
        _PROG = _build()

    bf = ml_dtypes.bfloat16
    cp_raw = np.ascontiguousarray(
        np.asarray(current_pose, np.float32).transpose(0, 1, 4, 2, 3)
    ).reshape(B, C, H, H)
    cp_img = np.zeros((B, C, 16, 16), np.float32)
    cp_img[:, :, 1:15, 1:15] = cp_raw
    x0_h = np.ascontiguousarray(
        cp_img.reshape(B, 4, 128, 16, 16).transpose(2, 0, 1, 3, 4)).astype(bf)
    qp_raw = np.ascontiguousarray(
        np.asarray(next_pose, np.float32).transpose(0, 1, 4, 2, 3)
    ).reshape(B, C, HO, HO)
    qp_img = np.zeros((B, C, 9, 9), np.float32)
    qp_img[:, :, 1:8, 1:8] = qp_raw
    x0q_h = np.ascontiguousarray(
        qp_img.reshape(B, 4, 128, 9, 9).transpose(2, 0, 1, 3, 4)).astype(bf)

    w1_h = _blockdiag_pm(np.asarray(current_w, np.float32)).astype(bf)
    wq_h = _blockdiag_pm(np.asarray(next_w, np.float32)).astype(bf)
    ep_h = np.ascontiguousarray(
        np.asarray(E_proj, np.float32).reshape(IN_N, 2, 98, HID)
        .transpose(2, 1, 0, 3)).astype(bf)
    rel = np.asarray(rel_embedd, np.float32)
    ident = np.eye(128, dtype=np.float32).astype(bf)

    common = {
        "w1d": w1_h, "wqd": wq_h, "epd": ep_h,
        "rel_k": np.ascontiguousarray(rel.T).astype(np.float32),
        "rel_kt": rel.astype(bf),
        "gam": np.asarray(ln_gamma, np.float32),
        "bet": np.asarray(ln_beta, np.float32),
        "ident": ident,
    }
    core_ids = list(range(8))
    in_maps = []
    for c in core_ids:
        sl = slice(c * NB, (c + 1) * NB)
        in_maps.append({**common,
                        "x0d": np.ascontiguousarray(x0_h[:, sl]),
                        "x0qd": np.ascontiguousarray(x0q_h[:, sl])})

    res = run_bass_kernel_spmd(_PROG, in_maps, core_ids)
    out = np.empty((B, J * 16), np.float32)
    for c in core_ids:
        out[c * NB:(c + 1) * NB] = res.results[c]["outy"][:, :J * 16]
    return out.reshape(B, OUT_N, HO, HO, OUT_D)


if __name__ == "__main__":
    import reference as ref
    inputs = ref.setup_inputs()
    expected = np.asarray(ref.reference(**inputs))
    actual = kernel(**{k: np.asarray(v) if not np.isscalar(v) else v
                       for k, v in inputs.items()})
    err = np.abs(actual - expected)
    sc = np.abs(expected).max()
    print("absmax err:", err.max(), "scale:", sc, "rel:", err.max() / sc)


# revision 6
# speedup vs baseline: 1.8380x; 1.0279x over previous
"""Trainium2 Bass kernel for nn_BilinearLinformerCapsuleFC (v2).

Data-parallel over batch (32 -> 4 per core x 8 cores). Single-core program:
grouped convs as block-diag matmuls (9 shifted taps, PSUM-accumulated),
Linformer key projection, column-softmax attention computed in
S^T [keys, queries] layout. The softmax exp runs as ONE whole-row
Activation instruction per i-tile (with accum_out row-sum normalizer),
double-buffered across two 4-bank PSUM halves so the Act engine (the
roofline for this problem) streams back-to-back. Out^T is accumulated
j-major (13 matmuls of 16-wide output each, nearly free on PE) directly
into a spare PSUM region, which makes the output relayout a single copy +
DMA. Prep for batches 0/1 runs up front in the still-free S-buffer banks;
prep for batches 2/3 is spread at half density over two attention windows
each; conv/LayerNorm post-processing trails one batch behind.
"""
import numpy as np
import ml_dtypes

import concourse.bass as bass
import concourse.mybir as mybir
import concourse.tile as tile
from concourse import bacc
from concourse.bass import ds
from concourse.bass_utils import run_bass_kernel_spmd

BF16 = mybir.dt.bfloat16
F32 = mybir.dt.float32
AF = mybir.ActivationFunctionType
ALU = mybir.AluOpType

B, IN_N, IN_D, H, OUT_N, OUT_D, HO, HID = 32, 32, 16, 14, 32, 16, 7, 64
C = IN_N * IN_D            # 512
NB = 4                     # batch items per core
NKEY = IN_N * HID + HO * HO  # 2097
J = OUT_N * HO * HO        # 1568
NT_I = 17                  # i tiles (16x128 + 49)
NT_J = 13                  # j tiles (12x128 + 32)
JPAD = NT_J * 2048         # 26624
EPS = 1e-5
SCALE = IN_D ** -0.5
CHUNKS = [(0, 512), (512, 512), (1024, 512), (1536, 32)]

_PROG = None


def _build():
    nc = bacc.Bacc("TRN2", target_bir_lowering=False, debug=False, num_devices=1)

    x0_d = nc.dram_tensor("x0d", [128, NB, 4, 16, 16], BF16, kind="ExternalInput")
    x0q_d = nc.dram_tensor("x0qd", [128, NB, 4, 9, 9], BF16, kind="ExternalInput")
    w1_d = nc.dram_tensor("w1d", [128, 9, 4, 128], BF16, kind="ExternalInput")
    wq_d = nc.dram_tensor("wqd", [128, 9, 4, 128], BF16, kind="ExternalInput")
    eps_d = nc.dram_tensor("epd", [98, 2, IN_N, HID], BF16, kind="ExternalInput")
    rel_k_d = nc.dram_tensor("rel_k", [49, 16], F32, kind="ExternalInput")
    rel_kt_d = nc.dram_tensor("rel_kt", [16, 49], BF16, kind="ExternalInput")
    gam_d = nc.dram_tensor("gam", [16], F32, kind="ExternalInput")
    bet_d = nc.dram_tensor("bet", [16], F32, kind="ExternalInput")
    ident_d = nc.dram_tensor("ident", [128, 128], BF16, kind="ExternalInput")

    import os
    DBG = os.environ.get("K2_DEBUG", "") == "1"
    okind = {"kind": "ExternalOutput"} if DBG else {}
    fq = nc.dram_tensor("fq", [NB, JPAD], BF16)
    fo = nc.dram_tensor("fo", [NB, JPAD], BF16, **okind)
    f3 = nc.dram_tensor("f3", [NB, JPAD], F32, **okind)
    if DBG:
        dbg_kt = nc.dram_tensor("dbg_kt", [128, 2112], BF16, kind="ExternalOutput")
        dbg_qt = nc.dram_tensor("dbg_qt", [128, J], BF16, kind="ExternalOutput")
        dbg_ksb = nc.dram_tensor("dbg_ksb", [128, NB, NT_I, 16], F32,
                                 kind="ExternalOutput")
    outy = nc.dram_tensor("outy", [NB, JPAD], F32, kind="ExternalOutput")

    with tile.TileContext(nc) as tc:
        from contextlib import ExitStack
        with ExitStack() as ctx:
            consts = ctx.enter_context(tc.tile_pool(name="consts", bufs=1))
            sm = ctx.enter_context(tc.tile_pool(name="sm", bufs=6))
            eb = ctx.enter_context(tc.tile_pool(name="eb", bufs=18))
            kpl = ctx.enter_context(tc.tile_pool(name="kpl", bufs=18))
            pb = ctx.enter_context(tc.tile_pool(name="pb", bufs=3))
            pp = ctx.enter_context(tc.tile_pool(name="pp", bufs=1, space="PSUM"))

            # ---- persistent SBUF ----
            ident = consts.tile([128, 128], BF16)
            w1s = consts.tile([128, 9, 4, 128], BF16)
            wqs = consts.tile([128, 9, 4, 128], BF16)
            eps_s = consts.tile([98, 2, IN_N, HID], BF16)
            gamb = consts.tile([128, NT_J, 16], F32)
            betb = consts.tile([128, NT_J, 16], F32)
            x0 = consts.tile([128, NB, 4, 16, 16], BF16)
            x0q = consts.tile([128, NB, 4, 9, 9], BF16)
            x1 = consts.tile([98, NB, 2, C], BF16)
            kt = consts.tile([128, 2112], BF16)   # b at partition 32b, rows 0:16
            qt = consts.tile([128, J], BF16)      # b at partition 32b
            k_sb = consts.tile([128, NB, NT_I, 16], F32)

            # ---- input DMAs (SP + Act queues; HWDGE is shared anyway) ----
            nc.sync.dma_start(out=x0[:, 0:2, :, :, :], in_=x0_d[:, 0:2, :, :, :])
            nc.scalar.dma_start(out=w1s, in_=w1_d[:, :, :, :])
            nc.sync.dma_start(out=ident, in_=ident_d[:, :])
            nc.sync.dma_start(out=x0q[:, 0:2, :, :, :], in_=x0q_d[:, 0:2, :, :, :])
            nc.sync.dma_start(out=wqs, in_=wq_d[:, :, :, :])
            nc.scalar.dma_start(out=eps_s, in_=eps_d[:, :, :, :])
            nc.sync.dma_start(out=x0[:, 2:, :, :, :], in_=x0_d[:, 2:, :, :, :])
            nc.sync.dma_start(out=x0q[:, 2:, :, :, :], in_=x0q_d[:, 2:, :, :, :])
            nc.sync.dma_start(out=k_sb[0:49, :, 16, :], in_=bass.AP(
                tensor=rel_k_d, offset=0, ap=[[16, 49], [0, NB], [1, 16]]))
            for b in range(NB):
                nc.sync.dma_start(out=kt[32 * b:32 * b + 16, 2048:2097],
                                  in_=rel_kt_d[:, :])
            nc.sync.dma_start(out=gamb, in_=bass.AP(
                tensor=gam_d, offset=0, ap=[[0, 128], [0, NT_J], [1, 16]]))
            nc.sync.dma_start(out=betb, in_=bass.AP(
                tensor=bet_d, offset=0, ap=[[0, 128], [0, NT_J], [1, 16]]))

            # ---- PSUM map: 8 banks total ----
            SA = pp.tile([128, 2048], F32)   # banks 0-3
            SB = pp.tile([128, 2048], F32)   # banks 4-7
            PO = SA[:, 1792:2000]            # Out^T accumulator (bank 3 spare)
            rot = [SB[:, 1568:1764], SB[:, 1764:1960]]    # conv scratch (bank 7)
            pco_r = SB[:, 1960:2009]                      # convout scratch
            ptx = [SA[:, 1568:1632].bitcast(BF16),        # transpose scratch
                   SA[:, 1632:1696].bitcast(BF16),
                   SA[:, 1696:1760].bitcast(BF16)]
            ptw = SA[:, 1568:1760].bitcast(BF16)          # all 3, adjacent
            rot_bf = SB[:, 1568:1764].bitcast(BF16)       # alt transpose scratch
            rot1_bf = SB[:, 1764:1960].bitcast(BF16)      # alt scratch 2 (rot[1])

            # ---- prep stages (per batch) ----
            kbf_st = {}
            xqc_st = {}
            x1c_st = {}

            def conv1_mm(b, blk, pc=None):
                if pc is None:
                    pc = rot[0]
                for tap in range(9):
                    ky, kx = tap // 3, tap % 3
                    nc.tensor.matmul(out=pc, lhsT=w1s[:, tap, blk, :],
                                     rhs=x0[:, b, blk, ky:ky + 14, kx:kx + 14],
                                     start=(tap == 0), stop=(tap == 8),
                                     skip_group_check=True)
                x1c = sm.tile([128, 196], BF16, tag="x1c")
                nc.vector.tensor_copy(out=x1c, in_=pc)
                x1c_st[(b, blk)] = x1c

            def conv1_tr(b, blk):
                x1c = x1c_st[(b, blk)]
                for hf in range(2):
                    pt = ptx[hf][0:98, :]
                    nc.tensor.transpose(pt, x1c[:, hf * 98:(hf + 1) * 98], ident)
                nc.vector.tensor_copy(
                    out=x1[:, b, :, blk * 128:(blk + 1) * 128],
                    in_=ptw[0:98, 0:256].rearrange("p (h c) -> p h c", h=2))

            def kproj_h(b, m, half, pkf=None):
                if pkf is None:
                    pkf = rot[m % 2]
                for q in range(half * 8, half * 8 + 8):
                    n2 = m * 16 + q
                    po = 64 * (q % 2)
                    psl = pkf[po:po + 64, (q // 2) * 16:(q // 2) * 16 + 16]
                    tp = (0, 64) if (q % 2) else (0, 0)
                    for hf in range(2):
                        nc.tensor.matmul(out=psl, lhsT=eps_s[:, hf, n2, :],
                                         rhs=x1[:, b, hf, n2::32],
                                         start=(hf == 0), stop=(hf == 1),
                                         tile_position=tp, skip_group_check=True)
                if half == 1:
                    nc.vector.tensor_copy(
                        out=k_sb[:, b, m * 8:(m + 1) * 8, :],
                        in_=pkf[:, 0:128].rearrange("p (t e) -> p t e", e=16))

            def kbf_stage(b, h):
                kbf = sm.tile([128, 8, 16], BF16, tag="kbf")
                nc.vector.tensor_copy(
                    out=kbf, in_=k_sb[:, b, h * 8:(h + 1) * 8, :])
                kbf_st[(b, h)] = kbf

            def ktT_run(b, t0, nt, sc=None):
                # nt <= 3 tiles; one batched copy from the adjacent scratch
                if sc is None:
                    sc = ptw
                for i, t in enumerate(range(t0, t0 + nt)):
                    h, u = divmod(t, 8)
                    if (b, h) not in kbf_st:
                        kbf_stage(b, h)
                    kbf = kbf_st[(b, h)]
                    pt = sc[0:16, i * 128:(i + 1) * 128]
                    nc.tensor.transpose(pt, kbf[:, u, :], ident)
                nc.vector.tensor_copy(
                    out=kt[32 * b:32 * b + 16, t0 * 128:(t0 + nt) * 128],
                    in_=sc[0:16, 0:nt * 128])

            def convq_conv(b, blk, pcq=None):
                if pcq is None:
                    pcq = rot[blk % 2][:, 0:49]
                for tap in range(9):
                    ky, kx = tap // 3, tap % 3
                    nc.tensor.matmul(out=pcq, lhsT=wqs[:, tap, blk, :],
                                     rhs=x0q[:, b, blk, ky:ky + 7, kx:kx + 7],
                                     start=(tap == 0), stop=(tap == 8),
                                     skip_group_check=True)
                xqc = sm.tile([128, 49], BF16, tag="xqc")
                nc.vector.tensor_copy(out=xqc, in_=pcq)
                nc.sync.dma_start(
                    out=fq[b, ds(blk * 6272, 6272)].rearrange(
                        "(p s) -> p s", p=128),
                    in_=xqc)
                xqc_st[(b, blk)] = xqc

            qall_st = {}

            def qt_load(b):
                q_all = pb.tile([128, NT_J, 16], BF16, tag="qall")
                nc.sync.dma_start(
                    out=q_all,
                    in_=fq[b, :].rearrange("(p t e) -> p t e", p=128, e=16))
                qall_st[b] = q_all

            def qt_tr(b, t0, nt, sc=None):
                # nt <= 3 tiles with equal row counts; one strided copy
                if sc is None:
                    sc = ptw
                q_all = qall_st[b]
                rows = (J - 1 - t0) // NT_J + 1
                for i, t in enumerate(range(t0, t0 + nt)):
                    pt = sc[0:16, i * 128:(i + 1) * 128]
                    nc.tensor.transpose(pt[:, :rows], q_all[:rows, t, :],
                                        ident[:rows, :rows])
                src_v = sc[0:16, 0:3 * 128].rearrange(
                    "p (i c) -> p i c", i=3)[:, 0:nt, 0:rows]
                dst = qt[32 * b:32 * b + 16, t0:]
                dst_v = bass.AP(tensor=dst.tensor, offset=dst.offset,
                                ap=[dst.ap[0], [1, nt], [NT_J, rows]])
                nc.vector.tensor_copy(out=dst_v, in_=src_v)

            # ---- post stages (convout + LN for batch b) ----
            post_state = {}

            def xt_load(b):
                x2p = pb.tile([128, 4, 9, 9], BF16, tag="x2p")
                nc.gpsimd.memset(x2p, 0.0)
                for blk in range(4):
                    nc.sync.dma_start(out=x2p[:, blk, 1:8, 1:8], in_=bass.AP(
                        tensor=fo, offset=b * JPAD + blk * 6272,
                        ap=[[49, 128], [7, 7], [1, 7]]))
                x3c = pb.tile([128, 4, 49], F32, tag="x3c")
                post_state[b] = {"x2p": x2p, "x3c": x3c}

            def convout_blk(b, blk):
                st = post_state[b]
                for tap in range(9):
                    ky, kx = tap // 3, tap % 3
                    nc.tensor.matmul(out=pco_r, lhsT=wqs[:, tap, blk, :],
                                     rhs=st["x2p"][:, blk, ky:ky + 7, kx:kx + 7],
                                     start=(tap == 0), stop=(tap == 8),
                                     skip_group_check=True)
                nc.vector.tensor_copy(out=st["x3c"][:, blk, :], in_=pco_r)

            def f3_roundtrip(b):
                st = post_state[b]
                nc.sync.dma_start(out=bass.AP(
                    tensor=f3, offset=b * JPAD, ap=[[49, 128], [6272, 4], [1, 49]]),
                    in_=st["x3c"])
                y = pb.tile([128, NT_J, 16], F32, tag="y")
                nc.sync.dma_start(out=y, in_=f3[b, :].rearrange(
                    "(p t e) -> p t e", p=128, e=16))
                st["y"] = y

            def ln_a(b):
                st = post_state[b]
                y = st["y"]
                sums = pb.tile([128, NT_J], F32, tag="sums")
                nc.vector.tensor_reduce(out=sums, in_=y,
                                        axis=mybir.AxisListType.X, op=ALU.add)
                sq = pb.tile([128, NT_J, 16], F32, tag="sq")
                nc.gpsimd.tensor_mul(out=sq, in0=y, in1=y)
                sqs = pb.tile([128, NT_J], F32, tag="sqs")
                nc.vector.tensor_reduce(out=sqs, in_=sq,
                                        axis=mybir.AxisListType.X, op=ALU.add)
                st["sums"], st["sqs"] = sums, sqs

            def ln_b(b):
                st = post_state[b]
                mu = pb.tile([128, NT_J], F32, tag="mu")
                nc.vector.tensor_scalar_mul(out=mu, in0=st["sums"],
                                            scalar1=1.0 / 16)
                msq = pb.tile([128, NT_J], F32, tag="msq")
                nc.vector.tensor_mul(out=msq, in0=mu, in1=mu)
                vpe = pb.tile([128, NT_J], F32, tag="vpe")
                nc.vector.scalar_tensor_tensor(out=vpe, in0=st["sqs"],
                                               scalar=1.0 / 16, in1=msq,
                                               op0=ALU.mult, op1=ALU.subtract)
                nc.vector.tensor_scalar_add(out=vpe, in0=vpe, scalar1=EPS)
                rstd = pb.tile([128, NT_J], F32, tag="rstd")
                ri = rstd[:, :].bitcast(mybir.dt.int32)
                nc.vector.tensor_scalar(
                    out=ri, in0=vpe[:, :].bitcast(mybir.dt.int32), scalar1=1,
                    scalar2=None, op0=ALU.logical_shift_right)
                nc.vector.tensor_scalar(
                    out=ri, in0=ri, scalar1=-1, scalar2=0x5F3759DF,
                    op0=ALU.mult, op1=ALU.add)
                tnw = pb.tile([128, NT_J], F32, tag="tnw")
                for _ in range(2):
                    nc.vector.tensor_mul(out=tnw, in0=rstd, in1=rstd)
                    nc.vector.tensor_mul(out=tnw, in0=tnw, in1=vpe)
                    nc.vector.tensor_scalar(
                        out=tnw, in0=tnw, scalar1=-0.5, scalar2=1.5,
                        op0=ALU.mult, op1=ALU.add)
                    nc.vector.tensor_mul(out=rstd, in0=rstd, in1=tnw)
                st["mu"], st["rstd"] = mu, rstd

            def ln_c(b):
                st = post_state[b]
                yn = pb.tile([128, NT_J, 16], F32, tag="yn")
                for jt in range(NT_J):
                    eng = nc.gpsimd
                    eng.tensor_scalar(
                        out=yn[:, jt, :], in0=st["y"][:, jt, :],
                        scalar1=st["mu"][:, jt:jt + 1],
                        scalar2=st["rstd"][:, jt:jt + 1],
                        op0=ALU.subtract, op1=ALU.mult)
                st["yn"] = yn

            def ln_d(b):
                st = post_state[b]
                yn = st["yn"]
                nc.vector.tensor_mul(out=yn, in0=yn, in1=gamb)
                nc.vector.tensor_add(out=yn, in0=yn, in1=betb)
                nc.sync.dma_start(
                    out=outy[b, :].rearrange("(p t e) -> p t e", p=128, e=16),
                    in_=yn)

            def post_slots(b):
                return {0: [lambda: xt_load(b)],
                        2: [lambda: convout_blk(b, 0)],
                        4: [lambda: convout_blk(b, 1)],
                        6: [lambda: convout_blk(b, 2)],
                        8: [lambda: convout_blk(b, 3)],
                        11: [lambda: f3_roundtrip(b)],
                        12: [lambda: ln_a(b)],
                        13: [lambda: ln_b(b)],
                        14: [lambda: ln_c(b)],
                        15: [lambda: ln_d(b)]}

            def prep_slots(b):
                return {0: [lambda: conv1_mm(b, 0)],
                        1: [lambda: conv1_tr(b, 0), lambda: convq_conv(b, 0)],
                        2: [lambda: conv1_mm(b, 1), lambda: convq_conv(b, 1)],
                        3: [lambda: conv1_tr(b, 1), lambda: convq_conv(b, 2)],
                        4: [lambda: conv1_mm(b, 2), lambda: convq_conv(b, 3)],
                        5: [lambda: conv1_tr(b, 2), lambda: qt_load(b)],
                        6: [lambda: conv1_mm(b, 3)],
                        7: [lambda: conv1_tr(b, 3), lambda: qt_tr(b, 0, 3)],
                        8: [lambda: kproj_h(b, 0, 0), lambda: kproj_h(b, 0, 1),
                            lambda: qt_tr(b, 3, 3, sc=rot1_bf)],
                        9: [lambda: qt_tr(b, 6, 2),
                            lambda: qt_tr(b, 8, 3, sc=rot1_bf)],
                        10: [lambda: kproj_h(b, 1, 0), lambda: kproj_h(b, 1, 1),
                             lambda: qt_tr(b, 11, 2)],
                        11: [lambda: ktT_run(b, 0, 3)],
                        12: [lambda: ktT_run(b, 3, 3, sc=rot1_bf)],
                        13: [lambda: ktT_run(b, 6, 3)],
                        14: [lambda: ktT_run(b, 9, 3, sc=rot1_bf)],
                        15: [lambda: ktT_run(b, 12, 3),
                             lambda: ktT_run(b, 15, 1, sc=rot1_bf)]}

            # ---- attention ----
            def attention(b, inject, carry):
                e_t, kp_t = {}, {}
                oa_box = {}

                def emit_out_block(it0, it1, first, last):
                    # contiguous in the PE stream => the PSUM zero-region
                    # cannot be poisoned mid-accumulation
                    for it in range(it0, it1 + 1):
                        e, rows = e_t[it]
                        kp = kp_t[it]
                        for t in range(NT_J):
                            cols = 128 if t < 12 else 32
                            nc.tensor.matmul(
                                out=PO[0:cols, t * 16:(t + 1) * 16],
                                lhsT=e[:rows, t * 128:t * 128 + cols],
                                rhs=kp[:rows, :],
                                start=(it == it0 and t == 0 and first),
                                stop=(it == it1 and last),
                                skip_group_check=True)

                def emit_s(it):
                    rows = 128 if it < 16 else 49
                    X = SA if (b * NT_I + it) % 2 == 0 else SB
                    for (c0, w) in CHUNKS:
                        nc.tensor.matmul(
                            out=X[:rows, c0:c0 + w],
                            lhsT=kt[32 * b:32 * b + 16, it * 128:it * 128 + rows],
                            rhs=qt[32 * b:32 * b + 16, c0:c0 + w],
                            start=True, stop=True,
                            tile_position=(32 * b, 0), skip_group_check=True)
                    return X, rows

                Xr = {0: emit_s(0)}
                for it in range(NT_I):
                    if it + 1 < NT_I:
                        Xr[it + 1] = emit_s(it + 1)
                    if it == 0 and carry is not None:
                        carry()
                    X, rows = Xr[it]
                    e = eb.tile([128, J], BF16, tag="e")
                    z = sm.tile([128, 1], F32, tag="z")
                    nc.scalar.activation(out=e[:rows, :], in_=X[:rows, 0:1568],
                                         func=AF.Exp, scale=SCALE,
                                         accum_out=z[:rows, :])
                    r = sm.tile([128, 1], F32, tag="r")
                    nc.vector.reciprocal(out=r[:rows, :], in_=z[:rows, :])
                    kp = kpl.tile([128, 16], BF16, tag="kp")
                    nc.vector.tensor_scalar_mul(out=kp[:rows, :],
                                                in0=k_sb[:rows, b, it, :],
                                                scalar1=r[:rows, :])
                    e_t[it] = (e, rows)
                    kp_t[it] = kp
                    if it == 9:
                        emit_out_block(0, 8, True, True)
                        oa = pb.tile([128, 208], F32, tag="oa")
                        nc.vector.tensor_copy(out=oa, in_=PO)
                        oa_box["oa"] = oa
                    if it > 0:
                        for th in inject.get(it - 1, []):
                            th()
                for th in inject.get(NT_I - 1, []):
                    th()

                def _carry():
                    emit_out_block(9, NT_I - 1, True, True)
                    fo_sb = pb.tile([128, NT_J, 16], BF16, tag="fos")
                    nc.vector.tensor_add(
                        out=fo_sb,
                        in0=oa_box["oa"].rearrange("p (t e) -> p t e", e=16),
                        in1=PO.rearrange("p (t e) -> p t e", e=16))
                    nc.sync.dma_start(
                        out=fo[b, :].rearrange("(t p e) -> p t e", p=128, e=16),
                        in_=fo_sb)
                return _carry

            # ---- schedule ----
            c1r0 = [SA[:, 0:196], SA[:, 512:708],
                    SB[:, 0:196], SB[:, 512:708]]
            cqr0 = [SA[:, 1024:1073], SA[:, 1136:1185],
                    SB[:, 1024:1073], SB[:, 1136:1185]]
            for blk in range(4):
                conv1_mm(0, blk, pc=c1r0[blk])
            for blk in range(4):
                convq_conv(0, blk, pcq=cqr0[blk])
                conv1_tr(0, blk)
            qt_load(0)
            kproj_h(0, 0, 0)
            kproj_h(0, 0, 1)
            kproj_h(0, 1, 0)
            kproj_h(0, 1, 1)
            ktT_run(0, 0, 3)
            qt_tr(0, 0, 3)
            qt_tr(0, 3, 3, sc=rot_bf)
            qt_tr(0, 6, 2)
            qt_tr(0, 8, 3, sc=rot_bf)
            qt_tr(0, 11, 2)

            carry = None
            for b in range(NB):
                inject = {}
                if b == 0:
                    inject[0] = [lambda: ktT_run(0, 3, 3)]
                    inject[1] = [lambda: ktT_run(0, 6, 3)]
                    inject[2] = [lambda: ktT_run(0, 9, 3)]
                    inject[3] = [lambda: ktT_run(0, 12, 3)]
                    inject[4] = [lambda: ktT_run(0, 15, 1)]
                if b + 1 < NB:
                    for k, v in prep_slots(b + 1).items():
                        inject.setdefault(k, []).extend(v)
                if b >= 1:
                    for k, v in post_slots(b - 1).items():
                        inject.setdefault(k, []).extend(v)
                carry = attention(b, inject, carry)
            carry()
            xt_load(NB - 1)
            for blk in range(4):
                convout_blk(NB - 1, blk)
            f3_roundtrip(NB - 1)
            ln_a(NB - 1)
            ln_b(NB - 1)
            ln_c(NB - 1)
            ln_d(NB - 1)
            if DBG:
                nc.sync.dma_start(out=dbg_kt[:, :], in_=kt)
                nc.sync.dma_start(out=dbg_qt[:, :], in_=qt)
                nc.sync.dma_start(out=dbg_ksb[:, :, :, :], in_=k_sb)

    nc.compile()
    return nc


def _blockdiag_pm(w):
    # torch OIHW grouped weights -> partition-major block-diag [128, 9, 4, 128]
    out = np.zeros((128, 9, 4, 128), np.float32)
    for blk in range(4):
        for g in range(8):
            grp = blk * 8 + g
            for ky in range(3):
                for kx in range(3):
                    out[g * 16:(g + 1) * 16, ky * 3 + kx, blk,
                        g * 16:(g + 1) * 16] = w[grp * 16:(grp + 1) * 16,
                                                 :, ky, kx].T
    return out


def kernel(current_pose, next_pose, current_w, next_w, E_proj, rel_embedd,
           ln_gamma, ln_beta, num_iter=None):
    global _PROG
    if _PROG is None:
        _PROG = _build()

    bf = ml_dtypes.bfloat16
    cp_raw = np.ascontiguousarray(
        np.asarray(current_pose, np.float32).transpose(0, 1, 4, 2, 3)
    ).reshape(B, C, H, H)
    cp_img = np.zeros((B, C, 16, 16), np.float32)
    cp_img[:, :, 1:15, 1:15] = cp_raw
    x0_h = np.ascontiguousarray(
        cp_img.reshape(B, 4, 128, 16, 16).transpose(2, 0, 1, 3, 4)).astype(bf)
    qp_raw = np.ascontiguousarray(
        np.asarray(next_pose, np.float32).transpose(0, 1, 4, 2, 3)
    ).reshape(B, C, HO, HO)
    qp_img = np.zeros((B, C, 9, 9), np.float32)
    qp_img[:, :, 1:8, 1:8] = qp_raw
    x0q_h = np.ascontiguousarray(
        qp_img.reshape(B, 4, 128, 9, 9).transpose(2, 0, 1, 3, 4)).astype(bf)

    w1_h = _blockdiag_pm(np.asarray(current_w, np.float32)).astype(bf)
    wq_h = _blockdiag_pm(np.asarray(next_w, np.float32)).astype(bf)
    ep_h = np.ascontiguousarray(
        np.asarray(E_proj, np.float32).reshape(IN_N, 2, 98, HID)
        .transpose(2, 1, 0, 3)).astype(bf)
    rel = np.asarray(rel_embedd, np.float32)
    ident = np.eye(128, dtype=np.float32).astype(bf)

    common = {
        "w1d": w1_h, "wqd": wq_h, "epd": ep_h,
        "rel_k": np.ascontiguousarray(rel.T).astype(np.float32),
        "rel_kt": rel.astype(bf),
        "gam": np.asarray(ln_gamma, np.float32),
        "bet": np.asarray(ln_beta, np.float32),
        "ident": ident,
    }
    core_ids = list(range(8))
    in_maps = []
    for c in core_ids:
        sl = slice(c * NB, (c + 1) * NB)
        in_maps.append({**common,
                        "x0d": np.ascontiguousarray(x0_h[:, sl]),
                        "x0qd": np.ascontiguousarray(x0q_h[:, sl])})

    res = run_bass_kernel_spmd(_PROG, in_maps, core_ids)
    out = np.empty((B, J * 16), np.float32)
    for c in core_ids:
        out[c * NB:(c + 1) * NB] = res.results[c]["outy"][:, :J * 16]
    return out.reshape(B, OUT_N, HO, HO, OUT_D)


if __name__ == "__main__":
    import reference as ref
    inputs = ref.setup_inputs()
    expected = np.asarray(ref.reference(**inputs))
    actual = kernel(**{k: np.asarray(v) if not np.isscalar(v) else v
                       for k, v in inputs.items()})
    err = np.abs(actual - expected)
    sc = np.abs(expected).max()
    print("absmax err:", err.max(), "scale:", sc, "rel:", err.max() / sc)


# revision 7
# speedup vs baseline: 1.8648x; 1.0146x over previous
"""Trainium2 Bass kernel for nn_BilinearLinformerCapsuleFC (v2).

Data-parallel over batch (32 -> 4 per core x 8 cores). Single-core program:
grouped convs as block-diag matmuls (9 shifted taps, PSUM-accumulated),
Linformer key projection, column-softmax attention computed in
S^T [keys, queries] layout. The softmax exp runs as ONE whole-row
Activation instruction per i-tile (with accum_out row-sum normalizer),
double-buffered across two 4-bank PSUM halves so the Act engine (the
roofline for this problem) streams back-to-back. Out^T is accumulated
j-major (13 matmuls of 16-wide output each, nearly free on PE) directly
into a spare PSUM region, which makes the output relayout a single copy +
DMA. Prep for batches 0/1 runs up front in the still-free S-buffer banks;
prep for batches 2/3 is spread at half density over two attention windows
each; conv/LayerNorm post-processing trails one batch behind.
"""
import numpy as np
import ml_dtypes

import concourse.bass as bass
import concourse.mybir as mybir
import concourse.tile as tile
from concourse import bacc
from concourse.bass import ds
from concourse.bass_utils import run_bass_kernel_spmd

BF16 = mybir.dt.bfloat16
F32 = mybir.dt.float32
AF = mybir.ActivationFunctionType
ALU = mybir.AluOpType

B, IN_N, IN_D, H, OUT_N, OUT_D, HO, HID = 32, 32, 16, 14, 32, 16, 7, 64
C = IN_N * IN_D            # 512
NB = 4                     # batch items per core
NKEY = IN_N * HID + HO * HO  # 2097
J = OUT_N * HO * HO        # 1568
NT_I = 17                  # i tiles (16x128 + 49)
NT_J = 13                  # j tiles (12x128 + 32)
JPAD = NT_J * 2048         # 26624
EPS = 1e-5
SCALE = IN_D ** -0.5
CHUNKS = [(0, 512), (512, 512), (1024, 512), (1536, 32)]

_PROG = None


def _build():
    nc = bacc.Bacc("TRN2", target_bir_lowering=False, debug=False, num_devices=1)

    x0_d = nc.dram_tensor("x0d", [128, NB, 4, 16, 16], BF16, kind="ExternalInput")
    x0q_d = nc.dram_tensor("x0qd", [128, NB, 4, 9, 9], BF16, kind="ExternalInput")
    w1_d = nc.dram_tensor("w1d", [128, 9, 4, 128], BF16, kind="ExternalInput")
    wq_d = nc.dram_tensor("wqd", [128, 9, 4, 128], BF16, kind="ExternalInput")
    eps_d = nc.dram_tensor("epd", [98, 2, IN_N, HID], BF16, kind="ExternalInput")
    rel_k_d = nc.dram_tensor("rel_k", [49, 16], F32, kind="ExternalInput")
    rel_kt_d = nc.dram_tensor("rel_kt", [16, 49], BF16, kind="ExternalInput")
    gam_d = nc.dram_tensor("gam", [16], F32, kind="ExternalInput")
    bet_d = nc.dram_tensor("bet", [16], F32, kind="ExternalInput")
    ident_d = nc.dram_tensor("ident", [128, 128], BF16, kind="ExternalInput")

    import os
    DBG = os.environ.get("K2_DEBUG", "") == "1"
    okind = {"kind": "ExternalOutput"} if DBG else {}
    fq = nc.dram_tensor("fq", [NB, JPAD], BF16)
    fo = nc.dram_tensor("fo", [NB, JPAD], BF16, **okind)
    f3 = nc.dram_tensor("f3", [NB, JPAD], F32, **okind)
    if DBG:
        dbg_kt = nc.dram_tensor("dbg_kt", [128, 2112], BF16, kind="ExternalOutput")
        dbg_qt = nc.dram_tensor("dbg_qt", [128, J], BF16, kind="ExternalOutput")
        dbg_ksb = nc.dram_tensor("dbg_ksb", [128, NB, NT_I, 16], F32,
                                 kind="ExternalOutput")
    outy = nc.dram_tensor("outy", [NB, JPAD], F32, kind="ExternalOutput")

    with tile.TileContext(nc) as tc:
        from contextlib import ExitStack
        with ExitStack() as ctx:
            consts = ctx.enter_context(tc.tile_pool(name="consts", bufs=1))
            sm = ctx.enter_context(tc.tile_pool(name="sm", bufs=6))
            eb = ctx.enter_context(tc.tile_pool(name="eb", bufs=18))
            kpl = ctx.enter_context(tc.tile_pool(name="kpl", bufs=18))
            pb = ctx.enter_context(tc.tile_pool(name="pb", bufs=3))
            pp = ctx.enter_context(tc.tile_pool(name="pp", bufs=1, space="PSUM"))

            # ---- persistent SBUF ----
            ident = consts.tile([128, 128], BF16)
            w1s = consts.tile([128, 9, 4, 128], BF16)
            wqs = consts.tile([128, 9, 4, 128], BF16)
            eps_s = consts.tile([98, 2, IN_N, HID], BF16)
            gamb = consts.tile([128, NT_J, 16], F32)
            betb = consts.tile([128, NT_J, 16], F32)
            x0 = consts.tile([128, NB, 4, 16, 16], BF16)
            x0q = consts.tile([128, NB, 4, 9, 9], BF16)
            x1 = consts.tile([98, NB, 2, C], BF16)
            kt = consts.tile([128, 2112], BF16)   # b at partition 32b, rows 0:16
            qt = consts.tile([128, J], BF16)      # b at partition 32b
            k_sb = consts.tile([128, NB, NT_I, 16], F32)

            # ---- input DMAs (SP + Act queues; HWDGE is shared anyway) ----
            nc.sync.dma_start(out=x0[:, 0:2, :, :, :], in_=x0_d[:, 0:2, :, :, :])
            nc.scalar.dma_start(out=w1s, in_=w1_d[:, :, :, :])
            nc.sync.dma_start(out=ident, in_=ident_d[:, :])
            nc.sync.dma_start(out=x0q[:, 0:2, :, :, :], in_=x0q_d[:, 0:2, :, :, :])
            nc.sync.dma_start(out=wqs, in_=wq_d[:, :, :, :])
            nc.scalar.dma_start(out=eps_s, in_=eps_d[:, :, :, :])
            nc.sync.dma_start(out=x0[:, 2:, :, :, :], in_=x0_d[:, 2:, :, :, :])
            nc.sync.dma_start(out=x0q[:, 2:, :, :, :], in_=x0q_d[:, 2:, :, :, :])
            nc.sync.dma_start(out=k_sb[0:49, :, 16, :], in_=bass.AP(
                tensor=rel_k_d, offset=0, ap=[[16, 49], [0, NB], [1, 16]]))
            for b in range(NB):
                nc.sync.dma_start(out=kt[32 * b:32 * b + 16, 2048:2097],
                                  in_=rel_kt_d[:, :])
            nc.sync.dma_start(out=gamb, in_=bass.AP(
                tensor=gam_d, offset=0, ap=[[0, 128], [0, NT_J], [1, 16]]))
            nc.sync.dma_start(out=betb, in_=bass.AP(
                tensor=bet_d, offset=0, ap=[[0, 128], [0, NT_J], [1, 16]]))

            # ---- PSUM map: 8 banks total ----
            SA = pp.tile([128, 2048], F32)   # banks 0-3
            SB = pp.tile([128, 2048], F32)   # banks 4-7
            PO = SA[:, 1788:1996]            # Out^T accumulator (bank 3 spare)
            cqc_r = SA[:, 1996:2045]         # convq-conv scratch (bank 3 tail)
            rot = [SB[:, 1568:1764], SB[:, 1764:1960]]    # conv scratch (bank 7)
            pco_r = SB[:, 1960:2009]                      # convout scratch
            ptx = [SA[:, 1568:1632].bitcast(BF16),        # transpose scratch
                   SA[:, 1632:1696].bitcast(BF16),
                   SA[:, 1696:1760].bitcast(BF16)]
            ptw = SA[:, 1568:1760].bitcast(BF16)          # all 3, adjacent
            rot_bf = SB[:, 1568:1764].bitcast(BF16)       # alt transpose scratch
            rot1_bf = SB[:, 1764:1960].bitcast(BF16)      # alt scratch 2 (rot[1])

            # ---- prep stages (per batch) ----
            kbf_st = {}
            xqc_st = {}
            x1c_st = {}

            def conv1_mm(b, blk, pc=None):
                if pc is None:
                    pc = rot[blk % 2]
                for tap in range(9):
                    ky, kx = tap // 3, tap % 3
                    nc.tensor.matmul(out=pc, lhsT=w1s[:, tap, blk, :],
                                     rhs=x0[:, b, blk, ky:ky + 14, kx:kx + 14],
                                     start=(tap == 0), stop=(tap == 8),
                                     skip_group_check=True)
                x1c = sm.tile([128, 196], BF16, tag="x1c")
                nc.vector.tensor_copy(out=x1c, in_=pc)
                x1c_st[(b, blk)] = x1c

            def conv1_tr(b, blk):
                x1c = x1c_st[(b, blk)]
                for hf in range(2):
                    pt = ptx[hf][0:98, :]
                    nc.tensor.transpose(pt, x1c[:, hf * 98:(hf + 1) * 98], ident)
                nc.vector.tensor_copy(
                    out=x1[:, b, :, blk * 128:(blk + 1) * 128],
                    in_=ptw[0:98, 0:256].rearrange("p (h c) -> p h c", h=2))

            def kproj_h(b, m, half, pkf=None):
                if pkf is None:
                    pkf = rot[m % 2]
                for q in range(half * 8, half * 8 + 8):
                    n2 = m * 16 + q
                    po = 64 * (q % 2)
                    psl = pkf[po:po + 64, (q // 2) * 16:(q // 2) * 16 + 16]
                    tp = (0, 64) if (q % 2) else (0, 0)
                    for hf in range(2):
                        nc.tensor.matmul(out=psl, lhsT=eps_s[:, hf, n2, :],
                                         rhs=x1[:, b, hf, n2::32],
                                         start=(hf == 0), stop=(hf == 1),
                                         tile_position=tp, skip_group_check=True)
                if half == 1:
                    nc.vector.tensor_copy(
                        out=k_sb[:, b, m * 8:(m + 1) * 8, :],
                        in_=pkf[:, 0:128].rearrange("p (t e) -> p t e", e=16))

            def kbf_stage(b, h):
                kbf = sm.tile([128, 8, 16], BF16, tag="kbf")
                nc.vector.tensor_copy(
                    out=kbf, in_=k_sb[:, b, h * 8:(h + 1) * 8, :])
                kbf_st[(b, h)] = kbf

            def ktT_run(b, t0, nt, sc=None):
                # nt <= 3 tiles; one batched copy from the adjacent scratch
                if sc is None:
                    sc = ptw
                for i, t in enumerate(range(t0, t0 + nt)):
                    h, u = divmod(t, 8)
                    if (b, h) not in kbf_st:
                        kbf_stage(b, h)
                    kbf = kbf_st[(b, h)]
                    pt = sc[0:16, i * 128:(i + 1) * 128]
                    nc.tensor.transpose(pt, kbf[:, u, :], ident)
                nc.vector.tensor_copy(
                    out=kt[32 * b:32 * b + 16, t0 * 128:(t0 + nt) * 128],
                    in_=sc[0:16, 0:nt * 128])

            def convq_conv(b, blk, pcq=None):
                if pcq is None:
                    pcq = cqc_r
                for tap in range(9):
                    ky, kx = tap // 3, tap % 3
                    nc.tensor.matmul(out=pcq, lhsT=wqs[:, tap, blk, :],
                                     rhs=x0q[:, b, blk, ky:ky + 7, kx:kx + 7],
                                     start=(tap == 0), stop=(tap == 8),
                                     skip_group_check=True)
                xqc = sm.tile([128, 49], BF16, tag="xqc")
                nc.vector.tensor_copy(out=xqc, in_=pcq)
                nc.sync.dma_start(
                    out=fq[b, ds(blk * 6272, 6272)].rearrange(
                        "(p s) -> p s", p=128),
                    in_=xqc)
                xqc_st[(b, blk)] = xqc

            qall_st = {}

            def qt_load(b):
                q_all = pb.tile([128, NT_J, 16], BF16, tag="qall")
                nc.sync.dma_start(
                    out=q_all,
                    in_=fq[b, :].rearrange("(p t e) -> p t e", p=128, e=16))
                qall_st[b] = q_all

            def qt_tr(b, t0, nt, sc=None):
                # nt <= 3 tiles with equal row counts; one strided copy
                if sc is None:
                    sc = ptw
                q_all = qall_st[b]
                rows = (J - 1 - t0) // NT_J + 1
                for i, t in enumerate(range(t0, t0 + nt)):
                    pt = sc[0:16, i * 128:(i + 1) * 128]
                    nc.tensor.transpose(pt[:, :rows], q_all[:rows, t, :],
                                        ident[:rows, :rows])
                src_v = sc[0:16, 0:3 * 128].rearrange(
                    "p (i c) -> p i c", i=3)[:, 0:nt, 0:rows]
                dst = qt[32 * b:32 * b + 16, t0:]
                dst_v = bass.AP(tensor=dst.tensor, offset=dst.offset,
                                ap=[dst.ap[0], [1, nt], [NT_J, rows]])
                nc.vector.tensor_copy(out=dst_v, in_=src_v)

            # ---- post stages (convout + LN for batch b) ----
            post_state = {}

            def xt_load(b):
                x2p = pb.tile([128, 4, 9, 9], BF16, tag="x2p")
                nc.gpsimd.memset(x2p, 0.0)
                for blk in range(4):
                    nc.sync.dma_start(out=x2p[:, blk, 1:8, 1:8], in_=bass.AP(
                        tensor=fo, offset=b * JPAD + blk * 6272,
                        ap=[[49, 128], [7, 7], [1, 7]]))
                x3c = pb.tile([128, 4, 49], F32, tag="x3c")
                post_state[b] = {"x2p": x2p, "x3c": x3c}

            def convout_blk(b, blk):
                st = post_state[b]
                for tap in range(9):
                    ky, kx = tap // 3, tap % 3
                    nc.tensor.matmul(out=pco_r, lhsT=wqs[:, tap, blk, :],
                                     rhs=st["x2p"][:, blk, ky:ky + 7, kx:kx + 7],
                                     start=(tap == 0), stop=(tap == 8),
                                     skip_group_check=True)
                nc.vector.tensor_copy(out=st["x3c"][:, blk, :], in_=pco_r)

            def f3_roundtrip(b):
                st = post_state[b]
                nc.sync.dma_start(out=bass.AP(
                    tensor=f3, offset=b * JPAD, ap=[[49, 128], [6272, 4], [1, 49]]),
                    in_=st["x3c"])
                y = pb.tile([128, NT_J, 16], F32, tag="y")
                nc.sync.dma_start(out=y, in_=f3[b, :].rearrange(
                    "(p t e) -> p t e", p=128, e=16))
                st["y"] = y

            def ln_a(b):
                st = post_state[b]
                y = st["y"]
                sums = pb.tile([128, NT_J], F32, tag="sums")
                nc.vector.tensor_reduce(out=sums, in_=y,
                                        axis=mybir.AxisListType.X, op=ALU.add)
                sq = pb.tile([128, NT_J, 16], F32, tag="sq")
                nc.gpsimd.tensor_mul(out=sq, in0=y, in1=y)
                sqs = pb.tile([128, NT_J], F32, tag="sqs")
                nc.vector.tensor_reduce(out=sqs, in_=sq,
                                        axis=mybir.AxisListType.X, op=ALU.add)
                st["sums"], st["sqs"] = sums, sqs

            def ln_b(b):
                st = post_state[b]
                mu = pb.tile([128, NT_J], F32, tag="mu")
                nc.vector.tensor_scalar_mul(out=mu, in0=st["sums"],
                                            scalar1=1.0 / 16)
                msq = pb.tile([128, NT_J], F32, tag="msq")
                nc.vector.tensor_mul(out=msq, in0=mu, in1=mu)
                vpe = pb.tile([128, NT_J], F32, tag="vpe")
                nc.vector.scalar_tensor_tensor(out=vpe, in0=st["sqs"],
                                               scalar=1.0 / 16, in1=msq,
                                               op0=ALU.mult, op1=ALU.subtract)
                nc.vector.tensor_scalar_add(out=vpe, in0=vpe, scalar1=EPS)
                rstd = pb.tile([128, NT_J], F32, tag="rstd")
                ri = rstd[:, :].bitcast(mybir.dt.int32)
                nc.vector.tensor_scalar(
                    out=ri, in0=vpe[:, :].bitcast(mybir.dt.int32), scalar1=1,
                    scalar2=None, op0=ALU.logical_shift_right)
                nc.vector.tensor_scalar(
                    out=ri, in0=ri, scalar1=-1, scalar2=0x5F3759DF,
                    op0=ALU.mult, op1=ALU.add)
                tnw = pb.tile([128, NT_J], F32, tag="tnw")
                for _ in range(2):
                    nc.vector.tensor_mul(out=tnw, in0=rstd, in1=rstd)
                    nc.vector.tensor_mul(out=tnw, in0=tnw, in1=vpe)
                    nc.vector.tensor_scalar(
                        out=tnw, in0=tnw, scalar1=-0.5, scalar2=1.5,
                        op0=ALU.mult, op1=ALU.add)
                    nc.vector.tensor_mul(out=rstd, in0=rstd, in1=tnw)
                st["mu"], st["rstd"] = mu, rstd

            def ln_c(b):
                st = post_state[b]
                yn = pb.tile([128, NT_J, 16], F32, tag="yn")
                for jt in range(NT_J):
                    eng = nc.gpsimd
                    eng.tensor_scalar(
                        out=yn[:, jt, :], in0=st["y"][:, jt, :],
                        scalar1=st["mu"][:, jt:jt + 1],
                        scalar2=st["rstd"][:, jt:jt + 1],
                        op0=ALU.subtract, op1=ALU.mult)
                st["yn"] = yn

            def ln_d(b):
                st = post_state[b]
                yn = st["yn"]
                nc.vector.tensor_mul(out=yn, in0=yn, in1=gamb)
                nc.vector.tensor_add(out=yn, in0=yn, in1=betb)
                nc.sync.dma_start(
                    out=outy[b, :].rearrange("(p t e) -> p t e", p=128, e=16),
                    in_=yn)

            def post_slots(b):
                return {0: [lambda: xt_load(b)],
                        2: [lambda: convout_blk(b, 0)],
                        4: [lambda: convout_blk(b, 1)],
                        6: [lambda: convout_blk(b, 2)],
                        8: [lambda: convout_blk(b, 3)],
                        11: [lambda: f3_roundtrip(b)],
                        12: [lambda: ln_a(b)],
                        13: [lambda: ln_b(b)],
                        14: [lambda: ln_c(b)],
                        15: [lambda: ln_d(b)]}

            def prep_slots(b):
                return {0: [lambda: conv1_mm(b, 0)],
                        1: [lambda: conv1_tr(b, 0), lambda: convq_conv(b, 0)],
                        2: [lambda: conv1_mm(b, 1), lambda: convq_conv(b, 1)],
                        3: [lambda: conv1_tr(b, 1), lambda: convq_conv(b, 2)],
                        4: [lambda: conv1_mm(b, 2), lambda: convq_conv(b, 3)],
                        5: [lambda: conv1_tr(b, 2), lambda: qt_load(b)],
                        6: [lambda: conv1_mm(b, 3)],
                        7: [lambda: conv1_tr(b, 3), lambda: qt_tr(b, 0, 3)],
                        8: [lambda: kproj_h(b, 0, 0), lambda: kproj_h(b, 0, 1),
                            lambda: qt_tr(b, 3, 3, sc=rot1_bf)],
                        9: [lambda: qt_tr(b, 6, 2),
                            lambda: qt_tr(b, 8, 3, sc=rot1_bf)],
                        10: [lambda: kproj_h(b, 1, 0), lambda: kproj_h(b, 1, 1),
                             lambda: qt_tr(b, 11, 2)],
                        11: [lambda: ktT_run(b, 0, 3)],
                        12: [lambda: ktT_run(b, 3, 3, sc=rot1_bf)],
                        13: [lambda: ktT_run(b, 6, 3)],
                        14: [lambda: ktT_run(b, 9, 3, sc=rot1_bf)],
                        15: [lambda: ktT_run(b, 12, 3),
                             lambda: ktT_run(b, 15, 1, sc=rot1_bf)]}

            # ---- attention ----
            def attention(b, inject, carry):
                e_t, kp_t = {}, {}
                oa_box = {}

                def emit_out_block(it0, it1, first, last):
                    # contiguous in the PE stream => the PSUM zero-region
                    # cannot be poisoned mid-accumulation
                    for it in range(it0, it1 + 1):
                        e, rows = e_t[it]
                        kp = kp_t[it]
                        for t in range(NT_J):
                            cols = 128 if t < 12 else 32
                            nc.tensor.matmul(
                                out=PO[0:cols, t * 16:(t + 1) * 16],
                                lhsT=e[:rows, t * 128:t * 128 + cols],
                                rhs=kp[:rows, :],
                                start=(it == it0 and t == 0 and first),
                                stop=(it == it1 and last),
                                skip_group_check=True)

                def emit_s(it):
                    rows = 128 if it < 16 else 49
                    X = SA if (b * NT_I + it) % 2 == 0 else SB
                    for (c0, w) in CHUNKS:
                        nc.tensor.matmul(
                            out=X[:rows, c0:c0 + w],
                            lhsT=kt[32 * b:32 * b + 16, it * 128:it * 128 + rows],
                            rhs=qt[32 * b:32 * b + 16, c0:c0 + w],
                            start=True, stop=True,
                            tile_position=(32 * b, 0), skip_group_check=True)
                    return X, rows

                Xr = {0: emit_s(0)}
                for it in range(NT_I):
                    if it + 1 < NT_I:
                        Xr[it + 1] = emit_s(it + 1)
                    if it == 0 and carry is not None:
                        carry()
                    X, rows = Xr[it]
                    e = eb.tile([128, J], BF16, tag="e")
                    z = sm.tile([128, 1], F32, tag="z")
                    nc.scalar.activation(out=e[:rows, :], in_=X[:rows, 0:1568],
                                         func=AF.Exp, scale=SCALE,
                                         accum_out=z[:rows, :])
                    r = sm.tile([128, 1], F32, tag="r")
                    nc.vector.reciprocal(out=r[:rows, :], in_=z[:rows, :])
                    kp = kpl.tile([128, 16], BF16, tag="kp")
                    nc.vector.tensor_scalar_mul(out=kp[:rows, :],
                                                in0=k_sb[:rows, b, it, :],
                                                scalar1=r[:rows, :])
                    e_t[it] = (e, rows)
                    kp_t[it] = kp
                    if it == 9:
                        emit_out_block(0, 8, True, True)
                        oa = pb.tile([128, 208], F32, tag="oa")
                        nc.vector.tensor_copy(out=oa, in_=PO)
                        oa_box["oa"] = oa
                    if it > 0:
                        for th in inject.get(it - 1, []):
                            th()
                for th in inject.get(NT_I - 1, []):
                    th()

                def _carry():
                    emit_out_block(9, NT_I - 1, True, True)
                    fo_sb = pb.tile([128, NT_J, 16], BF16, tag="fos")
                    nc.vector.tensor_add(
                        out=fo_sb,
                        in0=oa_box["oa"].rearrange("p (t e) -> p t e", e=16),
                        in1=PO.rearrange("p (t e) -> p t e", e=16))
                    nc.sync.dma_start(
                        out=fo[b, :].rearrange("(t p e) -> p t e", p=128, e=16),
                        in_=fo_sb)
                return _carry

            # ---- schedule ----
            c1r0 = [SA[:, 0:196], SA[:, 512:708],
                    SB[:, 0:196], SB[:, 512:708]]
            cqr0 = [SA[:, 1024:1073], SA[:, 1136:1185],
                    SB[:, 1024:1073], SB[:, 1136:1185]]
            for blk in range(4):
                conv1_mm(0, blk, pc=c1r0[blk])
            for blk in range(4):
                convq_conv(0, blk, pcq=cqr0[blk])
                conv1_tr(0, blk)
            qt_load(0)
            kproj_h(0, 0, 0)
            kproj_h(0, 0, 1)
            kproj_h(0, 1, 0)
            kproj_h(0, 1, 1)
            ktT_run(0, 0, 3)
            qt_tr(0, 0, 3)
            qt_tr(0, 3, 3, sc=rot_bf)
            qt_tr(0, 6, 2)
            qt_tr(0, 8, 3, sc=rot_bf)
            qt_tr(0, 11, 2)

            carry = None
            for b in range(NB):
                inject = {}
                if b == 0:
                    inject[0] = [lambda: ktT_run(0, 3, 3)]
                    inject[1] = [lambda: ktT_run(0, 6, 3)]
                    inject[2] = [lambda: ktT_run(0, 9, 3)]
                    inject[3] = [lambda: ktT_run(0, 12, 3)]
                    inject[4] = [lambda: ktT_run(0, 15, 1)]
                if b + 1 < NB:
                    for k, v in prep_slots(b + 1).items():
                        inject.setdefault(k, []).extend(v)
                if b >= 1:
                    for k, v in post_slots(b - 1).items():
                        inject.setdefault(k, []).extend(v)
                carry = attention(b, inject, carry)
            carry()
            xt_load(NB - 1)
            for blk in range(4):
                convout_blk(NB - 1, blk)
            f3_roundtrip(NB - 1)
            ln_a(NB - 1)
            ln_b(NB - 1)
            ln_c(NB - 1)
            ln_d(NB - 1)
            if DBG:
                nc.sync.dma_start(out=dbg_kt[:, :], in_=kt)
                nc.sync.dma_start(out=dbg_qt[:, :], in_=qt)
                nc.sync.dma_start(out=dbg_ksb[:, :, :, :], in_=k_sb)

    nc.compile()
    return nc


def _blockdiag_pm(w):
    # torch OIHW grouped weights -> partition-major block-diag [128, 9, 4, 128]
    out = np.zeros((128, 9, 4, 128), np.float32)
    for blk in range(4):
        for g in range(8):
            grp = blk * 8 + g
            for ky in range(3):
                for kx in range(3):
                    out[g * 16:(g + 1) * 16, ky * 3 + kx, blk,
                        g * 16:(g + 1) * 16] = w[grp * 16:(grp + 1) * 16,
                                                 :, ky, kx].T
    return out


def kernel(current_pose, next_pose, current_w, next_w, E_proj, rel_embedd,
           ln_gamma, ln_beta, num_iter=None):
    global _PROG
    if _PROG is None:
        _PROG = _build()

    bf = ml_dtypes.bfloat16
    cp_raw = np.ascontiguousarray(
        np.asarray(current_pose, np.float32).transpose(0, 1, 4, 2, 3)
    ).reshape(B, C, H, H)
    cp_img = np.zeros((B, C, 16, 16), np.float32)
    cp_img[:, :, 1:15, 1:15] = cp_raw
    x0_h = np.ascontiguousarray(
        cp_img.reshape(B, 4, 128, 16, 16).transpose(2, 0, 1, 3, 4)).astype(bf)
    qp_raw = np.ascontiguousarray(
        np.asarray(next_pose, np.float32).transpose(0, 1, 4, 2, 3)
    ).reshape(B, C, HO, HO)
    qp_img = np.zeros((B, C, 9, 9), np.float32)
    qp_img[:, :, 1:8, 1:8] = qp_raw
    x0q_h = np.ascontiguousarray(
        qp_img.reshape(B, 4, 128, 9, 9).transpose(2, 0, 1, 3, 4)).astype(bf)

    w1_h = _blockdiag_pm(np.asarray(current_w, np.float32)).astype(bf)
    wq_h = _blockdiag_pm(np.asarray(next_w, np.float32)).astype(bf)
    ep_h = np.ascontiguousarray(
        np.asarray(E_proj, np.float32).reshape(IN_N, 2, 98, HID)
        .transpose(2, 1, 0, 3)).astype(bf)
    rel = np.asarray(rel_embedd, np.float32)
    ident = np.eye(128, dtype=np.float32).astype(bf)

    common = {
        "w1d": w1_h, "wqd": wq_h, "epd": ep_h,
        "rel_k": np.ascontiguousarray(rel.T).astype(np.float32),
        "rel_kt": rel.astype(bf),
        "gam": np.asarray(ln_gamma, np.float32),
        "bet": np.asarray(ln_beta, np.float32),
        "ident": ident,
    }
    core_ids = list(range(8))
    in_maps = []
    for c in core_ids:
        sl = slice(c * NB, (c + 1) * NB)
        in_maps.append({**common,
                        "x0d": np.ascontiguousarray(x0_h[:, sl]),
                        "x0qd": np.ascontiguousarray(x0q_h[:, sl])})

    res = run_bass_kernel_spmd(_PROG, in_maps, core_ids)
    out = np.empty((B, J * 16), np.float32)
    for c in core_ids:
        out[c * NB:(c + 1) * NB] = res.results[c]["outy"][:, :J * 16]
    return out.reshape(B, OUT_N, HO, HO, OUT_D)


if __name__ == "__main__":
    import reference as ref
    inputs = ref.setup_inputs()
    expected = np.asarray(ref.reference(**inputs))
    actual = kernel(**{k: np.asarray(v) if not np.isscalar(v) else v
                       for k, v in inputs.items()})
    err = np.abs(actual - expected)
    sc = np.abs(expected).max()
    print("absmax err:", err.max(), "scale:", sc, "rel:", err.max() / sc)


# revision 8
# speedup vs baseline: 1.8765x; 1.0063x over previous
"""Trainium2 Bass kernel for nn_BilinearLinformerCapsuleFC (v2).

Data-parallel over batch (32 -> 4 per core x 8 cores). Single-core program:
grouped convs as block-diag matmuls (9 shifted taps, PSUM-accumulated),
Linformer key projection, column-softmax attention computed in
S^T [keys, queries] layout. The softmax exp runs as ONE whole-row
Activation instruction per i-tile (with accum_out row-sum normalizer),
double-buffered across two 4-bank PSUM halves so the Act engine (the
roofline for this problem) streams back-to-back. Out^T is accumulated
j-major (13 matmuls of 16-wide output each, nearly free on PE) directly
into a spare PSUM region, which makes the output relayout a single copy +
DMA. Prep for batches 0/1 runs up front in the still-free S-buffer banks;
prep for batches 2/3 is spread at half density over two attention windows
each; conv/LayerNorm post-processing trails one batch behind.
"""
import numpy as np
import ml_dtypes

import concourse.bass as bass
import concourse.mybir as mybir
import concourse.tile as tile
from concourse import bacc
from concourse.bass import ds
from concourse.bass_utils import run_bass_kernel_spmd

BF16 = mybir.dt.bfloat16
F32 = mybir.dt.float32
AF = mybir.ActivationFunctionType
ALU = mybir.AluOpType

B, IN_N, IN_D, H, OUT_N, OUT_D, HO, HID = 32, 32, 16, 14, 32, 16, 7, 64
C = IN_N * IN_D            # 512
NB = 4                     # batch items per core
NKEY = IN_N * HID + HO * HO  # 2097
J = OUT_N * HO * HO        # 1568
NT_I = 17                  # i tiles (16x128 + 49)
NT_J = 13                  # j tiles (12x128 + 32)
JPAD = NT_J * 2048         # 26624
EPS = 1e-5
SCALE = IN_D ** -0.5
CHUNKS = [(0, 512), (512, 512), (1024, 512), (1536, 32)]

_PROG = None


def _build():
    nc = bacc.Bacc("TRN2", target_bir_lowering=False, debug=False, num_devices=1)

    x0_d = nc.dram_tensor("x0d", [128, NB, 4, 16, 16], BF16, kind="ExternalInput")
    x0q_d = nc.dram_tensor("x0qd", [128, NB, 4, 9, 9], BF16, kind="ExternalInput")
    w1_d = nc.dram_tensor("w1d", [128, 9, 4, 128], BF16, kind="ExternalInput")
    wq_d = nc.dram_tensor("wqd", [128, 9, 4, 128], BF16, kind="ExternalInput")
    eps_d = nc.dram_tensor("epd", [98, 2, IN_N, HID], BF16, kind="ExternalInput")
    rel_k_d = nc.dram_tensor("rel_k", [49, 16], F32, kind="ExternalInput")
    rel_kt_d = nc.dram_tensor("rel_kt", [16, 49], BF16, kind="ExternalInput")
    gam_d = nc.dram_tensor("gam", [16], F32, kind="ExternalInput")
    bet_d = nc.dram_tensor("bet", [16], F32, kind="ExternalInput")
    ident_d = nc.dram_tensor("ident", [128, 128], BF16, kind="ExternalInput")

    import os
    DBG = os.environ.get("K2_DEBUG", "") == "1"
    okind = {"kind": "ExternalOutput"} if DBG else {}
    fq = nc.dram_tensor("fq", [NB, JPAD], BF16)
    fo = nc.dram_tensor("fo", [NB, JPAD], BF16, **okind)
    f3 = nc.dram_tensor("f3", [NB, JPAD], F32, **okind)
    if DBG:
        dbg_kt = nc.dram_tensor("dbg_kt", [128, 2112], BF16, kind="ExternalOutput")
        dbg_qt = nc.dram_tensor("dbg_qt", [128, J], BF16, kind="ExternalOutput")
        dbg_ksb = nc.dram_tensor("dbg_ksb", [128, NB, NT_I, 16], F32,
                                 kind="ExternalOutput")
    outy = nc.dram_tensor("outy", [NB, JPAD], F32, kind="ExternalOutput")

    with tile.TileContext(nc) as tc:
        from contextlib import ExitStack
        with ExitStack() as ctx:
            consts = ctx.enter_context(tc.tile_pool(name="consts", bufs=1))
            sm = ctx.enter_context(tc.tile_pool(name="sm", bufs=8))
            eb = ctx.enter_context(tc.tile_pool(name="eb", bufs=18))
            kpl = ctx.enter_context(tc.tile_pool(name="kpl", bufs=18))
            pb = ctx.enter_context(tc.tile_pool(name="pb", bufs=4))
            pp = ctx.enter_context(tc.tile_pool(name="pp", bufs=1, space="PSUM"))

            # ---- persistent SBUF ----
            ident = consts.tile([128, 128], BF16)
            w1s = consts.tile([128, 9, 4, 128], BF16)
            wqs = consts.tile([128, 9, 4, 128], BF16)
            eps_s = consts.tile([98, 2, IN_N, HID], BF16)
            gamb = consts.tile([128, NT_J, 16], F32)
            betb = consts.tile([128, NT_J, 16], F32)
            x0 = consts.tile([128, NB, 4, 16, 16], BF16)
            x0q = consts.tile([128, NB, 4, 9, 9], BF16)
            x1 = consts.tile([98, NB, 2, C], BF16)
            kt = consts.tile([128, 2112], BF16)   # b at partition 32b, rows 0:16
            qt = consts.tile([128, J], BF16)      # b at partition 32b
            k_sb = consts.tile([128, NB, NT_I, 16], F32)

            # ---- input DMAs (SP + Act queues; HWDGE is shared anyway) ----
            nc.sync.dma_start(out=x0[:, 0:2, :, :, :], in_=x0_d[:, 0:2, :, :, :])
            nc.scalar.dma_start(out=w1s, in_=w1_d[:, :, :, :])
            nc.sync.dma_start(out=ident, in_=ident_d[:, :])
            nc.sync.dma_start(out=x0q[:, 0:2, :, :, :], in_=x0q_d[:, 0:2, :, :, :])
            nc.sync.dma_start(out=wqs, in_=wq_d[:, :, :, :])
            nc.scalar.dma_start(out=eps_s, in_=eps_d[:, :, :, :])
            nc.sync.dma_start(out=x0[:, 2:, :, :, :], in_=x0_d[:, 2:, :, :, :])
            nc.sync.dma_start(out=x0q[:, 2:, :, :, :], in_=x0q_d[:, 2:, :, :, :])
            nc.sync.dma_start(out=k_sb[0:49, :, 16, :], in_=bass.AP(
                tensor=rel_k_d, offset=0, ap=[[16, 49], [0, NB], [1, 16]]))
            for b in range(NB):
                nc.sync.dma_start(out=kt[32 * b:32 * b + 16, 2048:2097],
                                  in_=rel_kt_d[:, :])
            nc.sync.dma_start(out=gamb, in_=bass.AP(
                tensor=gam_d, offset=0, ap=[[0, 128], [0, NT_J], [1, 16]]))
            nc.sync.dma_start(out=betb, in_=bass.AP(
                tensor=bet_d, offset=0, ap=[[0, 128], [0, NT_J], [1, 16]]))

            # ---- PSUM map: 8 banks total ----
            SA = pp.tile([128, 2048], F32)   # banks 0-3
            SB = pp.tile([128, 2048], F32)   # banks 4-7
            PO = SA[:, 1788:1996]            # Out^T accumulator (bank 3 spare)
            cqc_r = SA[:, 1996:2045]         # convq-conv scratch (bank 3 tail)
            rot = [SB[:, 1568:1764], SB[:, 1764:1960]]    # conv scratch (bank 7)
            pco_r = SB[:, 1960:2009]                      # convout scratch
            ptx = [SA[:, 1568:1632].bitcast(BF16),        # transpose scratch
                   SA[:, 1632:1696].bitcast(BF16),
                   SA[:, 1696:1760].bitcast(BF16)]
            ptw = SA[:, 1568:1760].bitcast(BF16)          # all 3, adjacent
            rot_bf = SB[:, 1568:1764].bitcast(BF16)       # alt transpose scratch
            rot1_bf = SB[:, 1764:1960].bitcast(BF16)      # alt scratch 2 (rot[1])

            # ---- prep stages (per batch) ----
            kbf_st = {}
            xqc_st = {}
            x1c_st = {}

            def conv1_mm(b, blk, pc=None):
                if pc is None:
                    pc = rot[blk % 2]
                for tap in range(9):
                    ky, kx = tap // 3, tap % 3
                    nc.tensor.matmul(out=pc, lhsT=w1s[:, tap, blk, :],
                                     rhs=x0[:, b, blk, ky:ky + 14, kx:kx + 14],
                                     start=(tap == 0), stop=(tap == 8),
                                     skip_group_check=True)
                x1c = sm.tile([128, 196], BF16, tag="x1c")
                nc.vector.tensor_copy(out=x1c, in_=pc)
                x1c_st[(b, blk)] = x1c

            def conv1_tr(b, blk):
                x1c = x1c_st[(b, blk)]
                for hf in range(2):
                    pt = ptx[hf][0:98, :]
                    nc.tensor.transpose(pt, x1c[:, hf * 98:(hf + 1) * 98], ident)
                nc.vector.tensor_copy(
                    out=x1[:, b, :, blk * 128:(blk + 1) * 128],
                    in_=ptw[0:98, 0:256].rearrange("p (h c) -> p h c", h=2))

            def kproj_h(b, m, half, pkf=None):
                if pkf is None:
                    pkf = rot[m % 2]
                for q in range(half * 8, half * 8 + 8):
                    n2 = m * 16 + q
                    po = 64 * (q % 2)
                    psl = pkf[po:po + 64, (q // 2) * 16:(q // 2) * 16 + 16]
                    tp = (0, 64) if (q % 2) else (0, 0)
                    for hf in range(2):
                        nc.tensor.matmul(out=psl, lhsT=eps_s[:, hf, n2, :],
                                         rhs=x1[:, b, hf, n2::32],
                                         start=(hf == 0), stop=(hf == 1),
                                         tile_position=tp, skip_group_check=True)
                if half == 1:
                    nc.vector.tensor_copy(
                        out=k_sb[:, b, m * 8:(m + 1) * 8, :],
                        in_=pkf[:, 0:128].rearrange("p (t e) -> p t e", e=16))

            def kbf_stage(b, h):
                kbf = sm.tile([128, 8, 16], BF16, tag="kbf")
                nc.vector.tensor_copy(
                    out=kbf, in_=k_sb[:, b, h * 8:(h + 1) * 8, :])
                kbf_st[(b, h)] = kbf

            def ktT_run(b, t0, nt, sc=None):
                # nt <= 3 tiles; one batched copy from the adjacent scratch
                if sc is None:
                    sc = ptw
                for i, t in enumerate(range(t0, t0 + nt)):
                    h, u = divmod(t, 8)
                    if (b, h) not in kbf_st:
                        kbf_stage(b, h)
                    kbf = kbf_st[(b, h)]
                    pt = sc[0:16, i * 128:(i + 1) * 128]
                    nc.tensor.transpose(pt, kbf[:, u, :], ident)
                nc.vector.tensor_copy(
                    out=kt[32 * b:32 * b + 16, t0 * 128:(t0 + nt) * 128],
                    in_=sc[0:16, 0:nt * 128])

            def convq_conv(b, blk, pcq=None):
                if pcq is None:
                    pcq = cqc_r
                for tap in range(9):
                    ky, kx = tap // 3, tap % 3
                    nc.tensor.matmul(out=pcq, lhsT=wqs[:, tap, blk, :],
                                     rhs=x0q[:, b, blk, ky:ky + 7, kx:kx + 7],
                                     start=(tap == 0), stop=(tap == 8),
                                     skip_group_check=True)
                xqc = sm.tile([128, 49], BF16, tag="xqc")
                nc.vector.tensor_copy(out=xqc, in_=pcq)
                nc.sync.dma_start(
                    out=fq[b, ds(blk * 6272, 6272)].rearrange(
                        "(p s) -> p s", p=128),
                    in_=xqc)
                xqc_st[(b, blk)] = xqc

            qall_st = {}

            def qt_load(b):
                q_all = pb.tile([128, NT_J, 16], BF16, tag="qall")
                nc.sync.dma_start(
                    out=q_all,
                    in_=fq[b, :].rearrange("(p t e) -> p t e", p=128, e=16))
                qall_st[b] = q_all

            def qt_tr(b, t0, nt, sc=None):
                # nt <= 3 tiles with equal row counts; one strided copy
                if sc is None:
                    sc = ptw
                q_all = qall_st[b]
                rows = (J - 1 - t0) // NT_J + 1
                for i, t in enumerate(range(t0, t0 + nt)):
                    pt = sc[0:16, i * 128:(i + 1) * 128]
                    nc.tensor.transpose(pt[:, :rows], q_all[:rows, t, :],
                                        ident[:rows, :rows])
                src_v = sc[0:16, 0:3 * 128].rearrange(
                    "p (i c) -> p i c", i=3)[:, 0:nt, 0:rows]
                dst = qt[32 * b:32 * b + 16, t0:]
                dst_v = bass.AP(tensor=dst.tensor, offset=dst.offset,
                                ap=[dst.ap[0], [1, nt], [NT_J, rows]])
                nc.vector.tensor_copy(out=dst_v, in_=src_v)

            # ---- post stages (convout + LN for batch b) ----
            post_state = {}

            def xt_load(b):
                x2p = pb.tile([128, 4, 9, 9], BF16, tag="x2p")
                nc.gpsimd.memset(x2p, 0.0)
                for blk in range(4):
                    nc.sync.dma_start(out=x2p[:, blk, 1:8, 1:8], in_=bass.AP(
                        tensor=fo, offset=b * JPAD + blk * 6272,
                        ap=[[49, 128], [7, 7], [1, 7]]))
                x3c = pb.tile([128, 4, 49], F32, tag="x3c")
                post_state[b] = {"x2p": x2p, "x3c": x3c}

            def convout_blk(b, blk, pco=None):
                st = post_state[b]
                if pco is None:
                    pco = pco_r
                for tap in range(9):
                    ky, kx = tap // 3, tap % 3
                    nc.tensor.matmul(out=pco, lhsT=wqs[:, tap, blk, :],
                                     rhs=st["x2p"][:, blk, ky:ky + 7, kx:kx + 7],
                                     start=(tap == 0), stop=(tap == 8),
                                     skip_group_check=True)
                nc.vector.tensor_copy(out=st["x3c"][:, blk, :], in_=pco)

            def f3_store_half(b, h):
                st = post_state[b]
                nc.sync.dma_start(out=bass.AP(
                    tensor=f3, offset=b * JPAD + h * 2 * 6272,
                    ap=[[49, 128], [6272, 2], [1, 49]]),
                    in_=st["x3c"][:, h * 2:h * 2 + 2, :])

            def y_load(b):
                st = post_state[b]
                y = pb.tile([128, NT_J, 16], F32, tag="y")
                nc.sync.dma_start(out=y, in_=f3[b, :].rearrange(
                    "(p t e) -> p t e", p=128, e=16))
                st["y"] = y

            def f3_roundtrip(b):
                st = post_state[b]
                nc.sync.dma_start(out=bass.AP(
                    tensor=f3, offset=b * JPAD, ap=[[49, 128], [6272, 4], [1, 49]]),
                    in_=st["x3c"])
                y = pb.tile([128, NT_J, 16], F32, tag="y")
                nc.sync.dma_start(out=y, in_=f3[b, :].rearrange(
                    "(p t e) -> p t e", p=128, e=16))
                st["y"] = y

            def ln_a(b):
                st = post_state[b]
                y = st["y"]
                sums = pb.tile([128, NT_J], F32, tag="sums")
                nc.vector.tensor_reduce(out=sums, in_=y,
                                        axis=mybir.AxisListType.X, op=ALU.add)
                sq = pb.tile([128, NT_J, 16], F32, tag="sq")
                nc.gpsimd.tensor_mul(out=sq, in0=y, in1=y)
                sqs = pb.tile([128, NT_J], F32, tag="sqs")
                nc.vector.tensor_reduce(out=sqs, in_=sq,
                                        axis=mybir.AxisListType.X, op=ALU.add)
                st["sums"], st["sqs"] = sums, sqs

            def ln_b(b):
                st = post_state[b]
                mu = pb.tile([128, NT_J], F32, tag="mu")
                nc.vector.tensor_scalar_mul(out=mu, in0=st["sums"],
                                            scalar1=1.0 / 16)
                msq = pb.tile([128, NT_J], F32, tag="msq")
                nc.vector.tensor_mul(out=msq, in0=mu, in1=mu)
                vpe = pb.tile([128, NT_J], F32, tag="vpe")
                nc.vector.scalar_tensor_tensor(out=vpe, in0=st["sqs"],
                                               scalar=1.0 / 16, in1=msq,
                                               op0=ALU.mult, op1=ALU.subtract)
                nc.vector.tensor_scalar_add(out=vpe, in0=vpe, scalar1=EPS)
                rstd = pb.tile([128, NT_J], F32, tag="rstd")
                ri = rstd[:, :].bitcast(mybir.dt.int32)
                nc.vector.tensor_scalar(
                    out=ri, in0=vpe[:, :].bitcast(mybir.dt.int32), scalar1=1,
                    scalar2=None, op0=ALU.logical_shift_right)
                nc.vector.tensor_scalar(
                    out=ri, in0=ri, scalar1=-1, scalar2=0x5F3759DF,
                    op0=ALU.mult, op1=ALU.add)
                tnw = pb.tile([128, NT_J], F32, tag="tnw")
                for _ in range(2):
                    nc.vector.tensor_mul(out=tnw, in0=rstd, in1=rstd)
                    nc.vector.tensor_mul(out=tnw, in0=tnw, in1=vpe)
                    nc.vector.tensor_scalar(
                        out=tnw, in0=tnw, scalar1=-0.5, scalar2=1.5,
                        op0=ALU.mult, op1=ALU.add)
                    nc.vector.tensor_mul(out=rstd, in0=rstd, in1=tnw)
                st["mu"], st["rstd"] = mu, rstd

            def ln_c(b, dve=False):
                st = post_state[b]
                yn = pb.tile([128, NT_J, 16], F32, tag="yn")
                for jt in range(NT_J):
                    eng = nc.vector if (dve and jt % 2 == 0) else nc.gpsimd
                    eng.tensor_scalar(
                        out=yn[:, jt, :], in0=st["y"][:, jt, :],
                        scalar1=st["mu"][:, jt:jt + 1],
                        scalar2=st["rstd"][:, jt:jt + 1],
                        op0=ALU.subtract, op1=ALU.mult)
                st["yn"] = yn

            def ln_d(b):
                st = post_state[b]
                yn = st["yn"]
                nc.vector.tensor_mul(out=yn, in0=yn, in1=gamb)
                nc.vector.tensor_add(out=yn, in0=yn, in1=betb)
                nc.sync.dma_start(
                    out=outy[b, :].rearrange("(p t e) -> p t e", p=128, e=16),
                    in_=yn)

            def post_slots(b):
                return {0: [lambda: xt_load(b)],
                        2: [lambda: convout_blk(b, 0)],
                        4: [lambda: convout_blk(b, 1)],
                        6: [lambda: convout_blk(b, 2)],
                        8: [lambda: convout_blk(b, 3)],
                        11: [lambda: f3_roundtrip(b)],
                        12: [lambda: ln_a(b)],
                        13: [lambda: ln_b(b)],
                        14: [lambda: ln_c(b)],
                        15: [lambda: ln_d(b)]}

            def prep_slots(b):
                return {0: [lambda: conv1_mm(b, 0)],
                        1: [lambda: conv1_tr(b, 0), lambda: convq_conv(b, 0)],
                        2: [lambda: conv1_mm(b, 1), lambda: convq_conv(b, 1)],
                        3: [lambda: conv1_tr(b, 1), lambda: convq_conv(b, 2)],
                        4: [lambda: conv1_mm(b, 2), lambda: convq_conv(b, 3)],
                        5: [lambda: conv1_tr(b, 2), lambda: qt_load(b)],
                        6: [lambda: conv1_mm(b, 3)],
                        7: [lambda: conv1_tr(b, 3), lambda: qt_tr(b, 0, 3)],
                        8: [lambda: kproj_h(b, 0, 0), lambda: kproj_h(b, 0, 1),
                            lambda: qt_tr(b, 3, 3, sc=rot1_bf)],
                        9: [lambda: qt_tr(b, 6, 2),
                            lambda: qt_tr(b, 8, 3, sc=rot1_bf)],
                        10: [lambda: kproj_h(b, 1, 0), lambda: kproj_h(b, 1, 1),
                             lambda: qt_tr(b, 11, 2)],
                        11: [lambda: ktT_run(b, 0, 3)],
                        12: [lambda: ktT_run(b, 3, 3, sc=rot1_bf)],
                        13: [lambda: ktT_run(b, 6, 3)],
                        14: [lambda: ktT_run(b, 9, 3, sc=rot1_bf)],
                        15: [lambda: ktT_run(b, 12, 3),
                             lambda: ktT_run(b, 15, 1, sc=rot1_bf)]}

            # ---- attention ----
            def attention(b, inject, carry):
                e_t, kp_t = {}, {}
                oa_box = {}

                def emit_out_block(it0, it1, first, last):
                    # contiguous in the PE stream => the PSUM zero-region
                    # cannot be poisoned mid-accumulation
                    for it in range(it0, it1 + 1):
                        e, rows = e_t[it]
                        kp = kp_t[it]
                        for t in range(NT_J):
                            cols = 128 if t < 12 else 32
                            nc.tensor.matmul(
                                out=PO[0:cols, t * 16:(t + 1) * 16],
                                lhsT=e[:rows, t * 128:t * 128 + cols],
                                rhs=kp[:rows, :],
                                start=(it == it0 and t == 0 and first),
                                stop=(it == it1 and last),
                                skip_group_check=True)

                def emit_s(it):
                    rows = 128 if it < 16 else 49
                    X = SA if (b * NT_I + it) % 2 == 0 else SB
                    for (c0, w) in CHUNKS:
                        nc.tensor.matmul(
                            out=X[:rows, c0:c0 + w],
                            lhsT=kt[32 * b:32 * b + 16, it * 128:it * 128 + rows],
                            rhs=qt[32 * b:32 * b + 16, c0:c0 + w],
                            start=True, stop=True,
                            tile_position=(32 * b, 0), skip_group_check=True)
                    return X, rows

                Xr = {0: emit_s(0)}
                for it in range(NT_I):
                    if it + 1 < NT_I:
                        Xr[it + 1] = emit_s(it + 1)
                    if it == 0 and carry is not None:
                        carry()
                    X, rows = Xr[it]
                    e = eb.tile([128, J], BF16, tag="e")
                    z = sm.tile([128, 1], F32, tag="z")
                    nc.scalar.activation(out=e[:rows, :], in_=X[:rows, 0:1568],
                                         func=AF.Exp, scale=SCALE,
                                         accum_out=z[:rows, :])
                    r = sm.tile([128, 1], F32, tag="r")
                    nc.vector.reciprocal(out=r[:rows, :], in_=z[:rows, :])
                    kp = kpl.tile([128, 16], BF16, tag="kp")
                    nc.vector.tensor_scalar_mul(out=kp[:rows, :],
                                                in0=k_sb[:rows, b, it, :],
                                                scalar1=r[:rows, :])
                    e_t[it] = (e, rows)
                    kp_t[it] = kp
                    if it == 9:
                        emit_out_block(0, 8, True, True)
                        oa = pb.tile([128, 208], F32, tag="oa")
                        nc.vector.tensor_copy(out=oa, in_=PO)
                        oa_box["oa"] = oa
                    if it > 0:
                        for th in inject.get(it - 1, []):
                            th()
                for th in inject.get(NT_I - 1, []):
                    th()

                def _carry():
                    emit_out_block(9, NT_I - 1, True, True)
                    fo_sb = pb.tile([128, NT_J, 16], BF16, tag="fos")
                    nc.vector.tensor_add(
                        out=fo_sb,
                        in0=oa_box["oa"].rearrange("p (t e) -> p t e", e=16),
                        in1=PO.rearrange("p (t e) -> p t e", e=16))
                    nc.sync.dma_start(
                        out=fo[b, :].rearrange("(t p e) -> p t e", p=128, e=16),
                        in_=fo_sb)
                return _carry

            # ---- schedule ----
            c1r0 = [SA[:, 0:196], SA[:, 512:708],
                    SB[:, 0:196], SB[:, 512:708]]
            cqr0 = [SA[:, 1024:1073], SA[:, 1136:1185],
                    SB[:, 1024:1073], SB[:, 1136:1185]]
            for blk in range(4):
                conv1_mm(0, blk, pc=c1r0[blk])
            for blk in range(4):
                convq_conv(0, blk, pcq=cqr0[blk])
                conv1_tr(0, blk)
            qt_load(0)
            kproj_h(0, 0, 0)
            kproj_h(0, 0, 1)
            kproj_h(0, 1, 0)
            kproj_h(0, 1, 1)
            ktT_run(0, 0, 3)
            qt_tr(0, 0, 3)
            qt_tr(0, 3, 3, sc=rot_bf)
            qt_tr(0, 6, 2)
            qt_tr(0, 8, 3, sc=rot_bf)
            qt_tr(0, 11, 2)

            carry = None
            for b in range(NB):
                inject = {}
                if b == 0:
                    inject[0] = [lambda: ktT_run(0, 3, 3)]
                    inject[1] = [lambda: ktT_run(0, 6, 3)]
                    inject[2] = [lambda: ktT_run(0, 9, 3)]
                    inject[3] = [lambda: ktT_run(0, 12, 3)]
                    inject[4] = [lambda: ktT_run(0, 15, 1)]
                if b + 1 < NB:
                    for k, v in prep_slots(b + 1).items():
                        inject.setdefault(k, []).extend(v)
                if b >= 1:
                    for k, v in post_slots(b - 1).items():
                        inject.setdefault(k, []).extend(v)
                carry = attention(b, inject, carry)
            carry()
            xt_load(NB - 1)
            convout_blk(NB - 1, 0)
            convout_blk(NB - 1, 1, pco=rot[0][:, 0:49])
            f3_store_half(NB - 1, 0)
            convout_blk(NB - 1, 2)
            convout_blk(NB - 1, 3, pco=rot[0][:, 0:49])
            f3_store_half(NB - 1, 1)
            y_load(NB - 1)
            ln_a(NB - 1)
            ln_b(NB - 1)
            ln_c(NB - 1, dve=True)
            ln_d(NB - 1)
            if DBG:
                nc.sync.dma_start(out=dbg_kt[:, :], in_=kt)
                nc.sync.dma_start(out=dbg_qt[:, :], in_=qt)
                nc.sync.dma_start(out=dbg_ksb[:, :, :, :], in_=k_sb)

    nc.compile()
    return nc


def _blockdiag_pm(w):
    # torch OIHW grouped weights -> partition-major block-diag [128, 9, 4, 128]
    out = np.zeros((128, 9, 4, 128), np.float32)
    for blk in range(4):
        for g in range(8):
            grp = blk * 8 + g
            for ky in range(3):
                for kx in range(3):
                    out[g * 16:(g + 1) * 16, ky * 3 + kx, blk,
                        g * 16:(g + 1) * 16] = w[grp * 16:(grp + 1) * 16,
                                                 :, ky, kx].T
    return out


def kernel(current_pose, next_pose, current_w, next_w, E_proj, rel_embedd,
           ln_gamma, ln_beta, num_iter=None):
    global _PROG
    if _PROG is None:
        _PROG = _build()

    bf = ml_dtypes.bfloat16
    cp_raw = np.ascontiguousarray(
        np.asarray(current_pose, np.float32).transpose(0, 1, 4, 2, 3)
    ).reshape(B, C, H, H)
    cp_img = np.zeros((B, C, 16, 16), np.float32)
    cp_img[:, :, 1:15, 1:15] = cp_raw
    x0_h = np.ascontiguousarray(
        cp_img.reshape(B, 4, 128, 16, 16).transpose(2, 0, 1, 3, 4)).astype(bf)
    qp_raw = np.ascontiguousarray(
        np.asarray(next_pose, np.float32).transpose(0, 1, 4, 2, 3)
    ).reshape(B, C, HO, HO)
    qp_img = np.zeros((B, C, 9, 9), np.float32)
    qp_img[:, :, 1:8, 1:8] = qp_raw
    x0q_h = np.ascontiguousarray(
        qp_img.reshape(B, 4, 128, 9, 9).transpose(2, 0, 1, 3, 4)).astype(bf)

    w1_h = _blockdiag_pm(np.asarray(current_w, np.float32)).astype(bf)
    wq_h = _blockdiag_pm(np.asarray(next_w, np.float32)).astype(bf)
    ep_h = np.ascontiguousarray(
        np.asarray(E_proj, np.float32).reshape(IN_N, 2, 98, HID)
        .transpose(2, 1, 0, 3)).astype(bf)
    rel = np.asarray(rel_embedd, np.float32)
    ident = np.eye(128, dtype=np.float32).astype(bf)

    common = {
        "w1d": w1_h, "wqd": wq_h, "epd": ep_h,
        "rel_k": np.ascontiguousarray(rel.T).astype(np.float32),
        "rel_kt": rel.astype(bf),
        "gam": np.asarray(ln_gamma, np.float32),
        "bet": np.asarray(ln_beta, np.float32),
        "ident": ident,
    }
    core_ids = list(range(8))
    in_maps = []
    for c in core_ids:
        sl = slice(c * NB, (c + 1) * NB)
        in_maps.append({**common,
                        "x0d": np.ascontiguousarray(x0_h[:, sl]),
                        "x0qd": np.ascontiguousarray(x0q_h[:, sl])})

    res = run_bass_kernel_spmd(_PROG, in_maps, core_ids)
    out = np.empty((B, J * 16), np.float32)
    for c in core_ids:
        out[c * NB:(c + 1) * NB] = res.results[c]["outy"][:, :J * 16]
    return out.reshape(B, OUT_N, HO, HO, OUT_D)


if __name__ == "__main__":
    import reference as ref
    inputs = ref.setup_inputs()
    expected = np.asarray(ref.reference(**inputs))
    actual = kernel(**{k: np.asarray(v) if not np.isscalar(v) else v
                       for k, v in inputs.items()})
    err = np.abs(actual - expected)
    sc = np.abs(expected).max()
    print("absmax err:", err.max(), "scale:", sc, "rel:", err.max() / sc)


# revision 9
# speedup vs baseline: 1.8876x; 1.0059x over previous
"""Trainium2 Bass kernel for nn_BilinearLinformerCapsuleFC (v2).

Data-parallel over batch (32 -> 4 per core x 8 cores). Single-core program:
grouped convs as block-diag matmuls (9 shifted taps, PSUM-accumulated),
Linformer key projection, column-softmax attention computed in
S^T [keys, queries] layout. The softmax exp runs as ONE whole-row
Activation instruction per i-tile (with accum_out row-sum normalizer),
double-buffered across two 4-bank PSUM halves so the Act engine (the
roofline for this problem) streams back-to-back. Out^T is accumulated
j-major (13 matmuls of 16-wide output each, nearly free on PE) directly
into a spare PSUM region, which makes the output relayout a single copy +
DMA. Prep for batches 0/1 runs up front in the still-free S-buffer banks;
prep for batches 2/3 is spread at half density over two attention windows
each; conv/LayerNorm post-processing trails one batch behind.
"""
import numpy as np
import ml_dtypes

import concourse.bass as bass
import concourse.mybir as mybir
import concourse.tile as tile
from concourse import bacc
from concourse.bass import ds
from concourse.bass_utils import run_bass_kernel_spmd

BF16 = mybir.dt.bfloat16
F32 = mybir.dt.float32
AF = mybir.ActivationFunctionType
ALU = mybir.AluOpType

B, IN_N, IN_D, H, OUT_N, OUT_D, HO, HID = 32, 32, 16, 14, 32, 16, 7, 64
C = IN_N * IN_D            # 512
NB = 4                     # batch items per core
NKEY = IN_N * HID + HO * HO  # 2097
J = OUT_N * HO * HO        # 1568
NT_I = 17                  # i tiles (16x128 + 49)
NT_J = 13                  # j tiles (12x128 + 32)
JPAD = NT_J * 2048         # 26624
EPS = 1e-5
SCALE = IN_D ** -0.5
CHUNKS = [(0, 512), (512, 512), (1024, 512), (1536, 32)]

_PROG = None


def _build():
    nc = bacc.Bacc("TRN2", target_bir_lowering=False, debug=False, num_devices=1)

    x0_d = nc.dram_tensor("x0d", [128, NB, 4, 16, 16], BF16, kind="ExternalInput")
    x0q_d = nc.dram_tensor("x0qd", [128, NB, 4, 9, 9], BF16, kind="ExternalInput")
    w1_d = nc.dram_tensor("w1d", [128, 9, 4, 128], BF16, kind="ExternalInput")
    wq_d = nc.dram_tensor("wqd", [128, 9, 4, 128], BF16, kind="ExternalInput")
    eps_d = nc.dram_tensor("epd", [98, 2, IN_N, HID], BF16, kind="ExternalInput")
    rel_k_d = nc.dram_tensor("rel_k", [49, 16], F32, kind="ExternalInput")
    rel_kt_d = nc.dram_tensor("rel_kt", [16, 49], BF16, kind="ExternalInput")
    gam_d = nc.dram_tensor("gam", [16], F32, kind="ExternalInput")
    bet_d = nc.dram_tensor("bet", [16], F32, kind="ExternalInput")
    ident_d = nc.dram_tensor("ident", [128, 128], BF16, kind="ExternalInput")

    import os
    DBG = os.environ.get("K2_DEBUG", "") == "1"
    okind = {"kind": "ExternalOutput"} if DBG else {}
    fq = nc.dram_tensor("fq", [NB, JPAD], BF16)
    fo = nc.dram_tensor("fo", [NB, JPAD], BF16, **okind)
    f3 = nc.dram_tensor("f3", [NB, JPAD], F32, **okind)
    if DBG:
        dbg_kt = nc.dram_tensor("dbg_kt", [128, 2112], BF16, kind="ExternalOutput")
        dbg_qt = nc.dram_tensor("dbg_qt", [128, J], BF16, kind="ExternalOutput")
        dbg_ksb = nc.dram_tensor("dbg_ksb", [128, NB, NT_I, 16], F32,
                                 kind="ExternalOutput")
    outy = nc.dram_tensor("outy", [NB, JPAD], F32, kind="ExternalOutput")

    with tile.TileContext(nc) as tc:
        from contextlib import ExitStack
        with ExitStack() as ctx:
            consts = ctx.enter_context(tc.tile_pool(name="consts", bufs=1))
            sm = ctx.enter_context(tc.tile_pool(name="sm", bufs=8))
            eb = ctx.enter_context(tc.tile_pool(name="eb", bufs=18))
            kpl = ctx.enter_context(tc.tile_pool(name="kpl", bufs=18))
            pb = ctx.enter_context(tc.tile_pool(name="pb", bufs=4))
            pp = ctx.enter_context(tc.tile_pool(name="pp", bufs=1, space="PSUM"))

            # ---- persistent SBUF ----
            ident = consts.tile([128, 128], BF16)
            w1s = consts.tile([128, 9, 4, 128], BF16)
            wqs = consts.tile([128, 9, 4, 128], BF16)
            eps_s = consts.tile([98, 2, IN_N, HID], BF16)
            gamb = consts.tile([128, NT_J, 16], F32)
            betb = consts.tile([128, NT_J, 16], F32)
            x0 = consts.tile([128, NB, 4, 16, 16], BF16)
            x0q = consts.tile([128, NB, 4, 9, 9], BF16)
            x1 = consts.tile([98, NB, 2, C], BF16)
            kt = consts.tile([128, 2112], BF16)   # b at partition 32b, rows 0:16
            qt = consts.tile([128, J], BF16)      # b at partition 32b
            k_sb = consts.tile([128, NB, NT_I, 16], F32)

            # ---- input DMAs (SP + Act queues; HWDGE is shared anyway) ----
            nc.sync.dma_start(out=x0[:, 0:2, :, :, :], in_=x0_d[:, 0:2, :, :, :])
            nc.scalar.dma_start(out=w1s, in_=w1_d[:, :, :, :])
            nc.sync.dma_start(out=ident, in_=ident_d[:, :])
            nc.sync.dma_start(out=x0q[:, 0:2, :, :, :], in_=x0q_d[:, 0:2, :, :, :])
            nc.sync.dma_start(out=wqs, in_=wq_d[:, :, :, :])
            nc.scalar.dma_start(out=eps_s, in_=eps_d[:, :, :, :])
            nc.sync.dma_start(out=x0[:, 2:, :, :, :], in_=x0_d[:, 2:, :, :, :])
            nc.sync.dma_start(out=x0q[:, 2:, :, :, :], in_=x0q_d[:, 2:, :, :, :])
            nc.sync.dma_start(out=k_sb[0:49, :, 16, :], in_=bass.AP(
                tensor=rel_k_d, offset=0, ap=[[16, 49], [0, NB], [1, 16]]))
            for b in range(NB):
                nc.sync.dma_start(out=kt[32 * b:32 * b + 16, 2048:2097],
                                  in_=rel_kt_d[:, :])
            nc.sync.dma_start(out=gamb, in_=bass.AP(
                tensor=gam_d, offset=0, ap=[[0, 128], [0, NT_J], [1, 16]]))
            nc.sync.dma_start(out=betb, in_=bass.AP(
                tensor=bet_d, offset=0, ap=[[0, 128], [0, NT_J], [1, 16]]))

            # ---- PSUM map: 8 banks total ----
            SA = pp.tile([128, 2048], F32)   # banks 0-3
            SB = pp.tile([128, 2048], F32)   # banks 4-7
            PO = SA[:, 1788:1996]            # Out^T accumulator (bank 3 spare)
            cqc_r = SA[:, 1996:2045]         # convq-conv scratch (bank 3 tail)
            rot = [SB[:, 1568:1764], SB[:, 1764:1960]]    # conv scratch (bank 7)
            pco_r = SB[:, 1960:2009]                      # convout scratch
            ptx = [SA[:, 1568:1632].bitcast(BF16),        # transpose scratch
                   SA[:, 1632:1696].bitcast(BF16),
                   SA[:, 1696:1760].bitcast(BF16)]
            ptw = SA[:, 1568:1760].bitcast(BF16)          # all 3, adjacent
            rot_bf = SB[:, 1568:1764].bitcast(BF16)       # alt transpose scratch
            rot1_bf = SB[:, 1764:1960].bitcast(BF16)      # alt scratch 2 (rot[1])

            # ---- prep stages (per batch) ----
            kbf_st = {}
            xqc_st = {}
            x1c_st = {}

            def conv1_mm(b, blk, pc=None):
                if pc is None:
                    pc = rot[blk % 2]
                for tap in range(9):
                    ky, kx = tap // 3, tap % 3
                    nc.tensor.matmul(out=pc, lhsT=w1s[:, tap, blk, :],
                                     rhs=x0[:, b, blk, ky:ky + 14, kx:kx + 14],
                                     start=(tap == 0), stop=(tap == 8),
                                     skip_group_check=True)
                x1c = sm.tile([128, 196], BF16, tag="x1c")
                nc.vector.tensor_copy(out=x1c, in_=pc)
                x1c_st[(b, blk)] = x1c

            def conv1_tr(b, blk):
                x1c = x1c_st[(b, blk)]
                for hf in range(2):
                    pt = ptx[hf][0:98, :]
                    nc.tensor.transpose(pt, x1c[:, hf * 98:(hf + 1) * 98], ident)
                nc.vector.tensor_copy(
                    out=x1[:, b, :, blk * 128:(blk + 1) * 128],
                    in_=ptw[0:98, 0:256].rearrange("p (h c) -> p h c", h=2))

            def kproj_h(b, m, half, pkf=None):
                if pkf is None:
                    pkf = rot[m % 2]
                for q in range(half * 8, half * 8 + 8):
                    n2 = m * 16 + q
                    po = 64 * (q % 2)
                    psl = pkf[po:po + 64, (q // 2) * 16:(q // 2) * 16 + 16]
                    tp = (0, 64) if (q % 2) else (0, 0)
                    for hf in range(2):
                        nc.tensor.matmul(out=psl, lhsT=eps_s[:, hf, n2, :],
                                         rhs=x1[:, b, hf, n2::32],
                                         start=(hf == 0), stop=(hf == 1),
                                         tile_position=tp, skip_group_check=True)
                if half == 1:
                    nc.vector.tensor_copy(
                        out=k_sb[:, b, m * 8:(m + 1) * 8, :],
                        in_=pkf[:, 0:128].rearrange("p (t e) -> p t e", e=16))

            def kbf_stage(b, h):
                kbf = sm.tile([128, 8, 16], BF16, tag="kbf")
                nc.vector.tensor_copy(
                    out=kbf, in_=k_sb[:, b, h * 8:(h + 1) * 8, :])
                kbf_st[(b, h)] = kbf

            def ktT_run(b, t0, nt, sc=None):
                # nt <= 3 tiles; one batched copy from the adjacent scratch
                if sc is None:
                    sc = ptw
                for i, t in enumerate(range(t0, t0 + nt)):
                    h, u = divmod(t, 8)
                    if (b, h) not in kbf_st:
                        kbf_stage(b, h)
                    kbf = kbf_st[(b, h)]
                    pt = sc[0:16, i * 128:(i + 1) * 128]
                    nc.tensor.transpose(pt, kbf[:, u, :], ident)
                nc.vector.tensor_copy(
                    out=kt[32 * b:32 * b + 16, t0 * 128:(t0 + nt) * 128],
                    in_=sc[0:16, 0:nt * 128])

            def convq_conv(b, blk, pcq=None):
                if pcq is None:
                    pcq = cqc_r
                for tap in range(9):
                    ky, kx = tap // 3, tap % 3
                    nc.tensor.matmul(out=pcq, lhsT=wqs[:, tap, blk, :],
                                     rhs=x0q[:, b, blk, ky:ky + 7, kx:kx + 7],
                                     start=(tap == 0), stop=(tap == 8),
                                     skip_group_check=True)
                if blk == 0:
                    xqc4 = sm.tile([128, 4, 49], BF16, tag="xqc4")
                    xqc_st[b] = xqc4
                xqc4 = xqc_st[b]
                nc.vector.tensor_copy(out=xqc4[:, blk, :], in_=pcq)
                if blk == 3:
                    nc.sync.dma_start(
                        out=bass.AP(tensor=fq, offset=b * JPAD,
                                    ap=[[49, 128], [6272, 4], [1, 49]]),
                        in_=xqc_st[b])

            qall_st = {}

            def qt_load(b):
                q_all = pb.tile([128, NT_J, 16], BF16, tag="qall")
                nc.sync.dma_start(
                    out=q_all,
                    in_=fq[b, :].rearrange("(p t e) -> p t e", p=128, e=16))
                qall_st[b] = q_all

            def qt_tr(b, t0, nt, sc=None):
                # nt <= 3 tiles with equal row counts; one strided copy
                if sc is None:
                    sc = ptw
                q_all = qall_st[b]
                rows = (J - 1 - t0) // NT_J + 1
                for i, t in enumerate(range(t0, t0 + nt)):
                    pt = sc[0:16, i * 128:(i + 1) * 128]
                    nc.tensor.transpose(pt[:, :rows], q_all[:rows, t, :],
                                        ident[:rows, :rows])
                src_v = sc[0:16, 0:3 * 128].rearrange(
                    "p (i c) -> p i c", i=3)[:, 0:nt, 0:rows]
                dst = qt[32 * b:32 * b + 16, t0:]
                dst_v = bass.AP(tensor=dst.tensor, offset=dst.offset,
                                ap=[dst.ap[0], [1, nt], [NT_J, rows]])
                nc.vector.tensor_copy(out=dst_v, in_=src_v)

            # ---- post stages (convout + LN for batch b) ----
            post_state = {}

            def xt_load(b):
                x2p = pb.tile([128, 4, 9, 9], BF16, tag="x2p")
                nc.gpsimd.memset(x2p, 0.0)
                for blk in range(4):
                    nc.sync.dma_start(out=x2p[:, blk, 1:8, 1:8], in_=bass.AP(
                        tensor=fo, offset=b * JPAD + blk * 6272,
                        ap=[[49, 128], [7, 7], [1, 7]]))
                x3c = pb.tile([128, 4, 49], F32, tag="x3c")
                post_state[b] = {"x2p": x2p, "x3c": x3c}

            def convout_blk(b, blk, pco=None):
                st = post_state[b]
                if pco is None:
                    pco = pco_r
                for tap in range(9):
                    ky, kx = tap // 3, tap % 3
                    nc.tensor.matmul(out=pco, lhsT=wqs[:, tap, blk, :],
                                     rhs=st["x2p"][:, blk, ky:ky + 7, kx:kx + 7],
                                     start=(tap == 0), stop=(tap == 8),
                                     skip_group_check=True)
                nc.vector.tensor_copy(out=st["x3c"][:, blk, :], in_=pco)

            def f3_store_half(b, h):
                st = post_state[b]
                nc.sync.dma_start(out=bass.AP(
                    tensor=f3, offset=b * JPAD + h * 2 * 6272,
                    ap=[[49, 128], [6272, 2], [1, 49]]),
                    in_=st["x3c"][:, h * 2:h * 2 + 2, :])

            def y_load(b):
                st = post_state[b]
                y = pb.tile([128, NT_J, 16], F32, tag="y")
                nc.sync.dma_start(out=y, in_=f3[b, :].rearrange(
                    "(p t e) -> p t e", p=128, e=16))
                st["y"] = y

            def f3_roundtrip(b):
                st = post_state[b]
                nc.sync.dma_start(out=bass.AP(
                    tensor=f3, offset=b * JPAD, ap=[[49, 128], [6272, 4], [1, 49]]),
                    in_=st["x3c"])
                y = pb.tile([128, NT_J, 16], F32, tag="y")
                nc.sync.dma_start(out=y, in_=f3[b, :].rearrange(
                    "(p t e) -> p t e", p=128, e=16))
                st["y"] = y

            def ln_a(b):
                st = post_state[b]
                y = st["y"]
                sums = pb.tile([128, NT_J], F32, tag="sums")
                nc.vector.tensor_reduce(out=sums, in_=y,
                                        axis=mybir.AxisListType.X, op=ALU.add)
                sq = pb.tile([128, NT_J, 16], F32, tag="sq")
                nc.gpsimd.tensor_mul(out=sq, in0=y, in1=y)
                sqs = pb.tile([128, NT_J], F32, tag="sqs")
                nc.vector.tensor_reduce(out=sqs, in_=sq,
                                        axis=mybir.AxisListType.X, op=ALU.add)
                st["sums"], st["sqs"] = sums, sqs

            def ln_b(b):
                st = post_state[b]
                mu = pb.tile([128, NT_J], F32, tag="mu")
                nc.vector.tensor_scalar_mul(out=mu, in0=st["sums"],
                                            scalar1=1.0 / 16)
                msq = pb.tile([128, NT_J], F32, tag="msq")
                nc.vector.tensor_mul(out=msq, in0=mu, in1=mu)
                vpe = pb.tile([128, NT_J], F32, tag="vpe")
                nc.vector.scalar_tensor_tensor(out=vpe, in0=st["sqs"],
                                               scalar=1.0 / 16, in1=msq,
                                               op0=ALU.mult, op1=ALU.subtract)
                nc.vector.tensor_scalar_add(out=vpe, in0=vpe, scalar1=EPS)
                rstd = pb.tile([128, NT_J], F32, tag="rstd")
                ri = rstd[:, :].bitcast(mybir.dt.int32)
                nc.vector.tensor_scalar(
                    out=ri, in0=vpe[:, :].bitcast(mybir.dt.int32), scalar1=1,
                    scalar2=None, op0=ALU.logical_shift_right)
                nc.vector.tensor_scalar(
                    out=ri, in0=ri, scalar1=-1, scalar2=0x5F3759DF,
                    op0=ALU.mult, op1=ALU.add)
                tnw = pb.tile([128, NT_J], F32, tag="tnw")
                for _ in range(2):
                    nc.vector.tensor_mul(out=tnw, in0=rstd, in1=rstd)
                    nc.vector.tensor_mul(out=tnw, in0=tnw, in1=vpe)
                    nc.vector.tensor_scalar(
                        out=tnw, in0=tnw, scalar1=-0.5, scalar2=1.5,
                        op0=ALU.mult, op1=ALU.add)
                    nc.vector.tensor_mul(out=rstd, in0=rstd, in1=tnw)
                st["mu"], st["rstd"] = mu, rstd

            def ln_c(b, dve=False):
                st = post_state[b]
                yn = pb.tile([128, NT_J, 16], F32, tag="yn")
                for jt in range(NT_J):
                    eng = nc.vector if (dve and jt % 2 == 0) else nc.gpsimd
                    eng.tensor_scalar(
                        out=yn[:, jt, :], in0=st["y"][:, jt, :],
                        scalar1=st["mu"][:, jt:jt + 1],
                        scalar2=st["rstd"][:, jt:jt + 1],
                        op0=ALU.subtract, op1=ALU.mult)
                st["yn"] = yn

            def ln_d(b):
                st = post_state[b]
                yn = st["yn"]
                nc.vector.tensor_mul(out=yn, in0=yn, in1=gamb)
                nc.vector.tensor_add(out=yn, in0=yn, in1=betb)
                nc.sync.dma_start(
                    out=outy[b, :].rearrange("(p t e) -> p t e", p=128, e=16),
                    in_=yn)

            def post_slots(b):
                return {0: [lambda: xt_load(b)],
                        2: [lambda: convout_blk(b, 0)],
                        4: [lambda: convout_blk(b, 1)],
                        6: [lambda: convout_blk(b, 2)],
                        8: [lambda: convout_blk(b, 3)],
                        11: [lambda: f3_roundtrip(b)],
                        12: [lambda: ln_a(b)],
                        13: [lambda: ln_b(b)],
                        14: [lambda: ln_c(b)],
                        15: [lambda: ln_d(b)]}

            def prep_slots(b):
                return {0: [lambda: conv1_mm(b, 0)],
                        1: [lambda: conv1_tr(b, 0), lambda: convq_conv(b, 0)],
                        2: [lambda: conv1_mm(b, 1), lambda: convq_conv(b, 1)],
                        3: [lambda: conv1_tr(b, 1), lambda: convq_conv(b, 2)],
                        4: [lambda: conv1_mm(b, 2), lambda: convq_conv(b, 3)],
                        5: [lambda: conv1_tr(b, 2), lambda: qt_load(b)],
                        6: [lambda: conv1_mm(b, 3)],
                        7: [lambda: conv1_tr(b, 3), lambda: qt_tr(b, 0, 3)],
                        8: [lambda: kproj_h(b, 0, 0), lambda: kproj_h(b, 0, 1),
                            lambda: qt_tr(b, 3, 3, sc=rot1_bf)],
                        9: [lambda: qt_tr(b, 6, 2),
                            lambda: qt_tr(b, 8, 3, sc=rot1_bf)],
                        10: [lambda: kproj_h(b, 1, 0), lambda: kproj_h(b, 1, 1),
                             lambda: qt_tr(b, 11, 2)],
                        11: [lambda: ktT_run(b, 0, 3)],
                        12: [lambda: ktT_run(b, 3, 3, sc=rot1_bf)],
                        13: [lambda: ktT_run(b, 6, 3)],
                        14: [lambda: ktT_run(b, 9, 3, sc=rot1_bf)],
                        15: [lambda: ktT_run(b, 12, 3),
                             lambda: ktT_run(b, 15, 1, sc=rot1_bf)]}

            # ---- attention ----
            def attention(b, inject, carry):
                e_t, kp_t = {}, {}
                oa_box = {}

                def emit_out_block(it0, it1, first, last):
                    # contiguous in the PE stream => the PSUM zero-region
                    # cannot be poisoned mid-accumulation
                    for it in range(it0, it1 + 1):
                        e, rows = e_t[it]
                        kp = kp_t[it]
                        for t in range(NT_J):
                            cols = 128 if t < 12 else 32
                            nc.tensor.matmul(
                                out=PO[0:cols, t * 16:(t + 1) * 16],
                                lhsT=e[:rows, t * 128:t * 128 + cols],
                                rhs=kp[:rows, :],
                                start=(it == it0 and t == 0 and first),
                                stop=(it == it1 and last),
                                skip_group_check=True)

                def emit_s(it):
                    rows = 128 if it < 16 else 49
                    X = SA if (b * NT_I + it) % 2 == 0 else SB
                    for (c0, w) in CHUNKS:
                        nc.tensor.matmul(
                            out=X[:rows, c0:c0 + w],
                            lhsT=kt[32 * b:32 * b + 16, it * 128:it * 128 + rows],
                            rhs=qt[32 * b:32 * b + 16, c0:c0 + w],
                            start=True, stop=True,
                            tile_position=(32 * b, 0), skip_group_check=True)
                    return X, rows

                Xr = {0: emit_s(0)}
                for it in range(NT_I):
                    if it + 1 < NT_I:
                        Xr[it + 1] = emit_s(it + 1)
                    if it == 0 and carry is not None:
                        carry()
                    X, rows = Xr[it]
                    e = eb.tile([128, J], BF16, tag="e")
                    z = sm.tile([128, 1], F32, tag="z")
                    nc.scalar.activation(out=e[:rows, :], in_=X[:rows, 0:1568],
                                         func=AF.Exp, scale=SCALE,
                                         accum_out=z[:rows, :])
                    r = sm.tile([128, 1], F32, tag="r")
                    nc.vector.reciprocal(out=r[:rows, :], in_=z[:rows, :])
                    kp = kpl.tile([128, 16], BF16, tag="kp")
                    nc.vector.tensor_scalar_mul(out=kp[:rows, :],
                                                in0=k_sb[:rows, b, it, :],
                                                scalar1=r[:rows, :])
                    e_t[it] = (e, rows)
                    kp_t[it] = kp
                    if it == 9:
                        emit_out_block(0, 8, True, True)
                        oa = pb.tile([128, 208], F32, tag="oa")
                        nc.vector.tensor_copy(out=oa, in_=PO)
                        oa_box["oa"] = oa
                    if it > 0:
                        for th in inject.get(it - 1, []):
                            th()
                for th in inject.get(NT_I - 1, []):
                    th()

                def _carry():
                    emit_out_block(9, NT_I - 1, True, True)
                    fo_sb = pb.tile([128, NT_J, 16], BF16, tag="fos")
                    nc.vector.tensor_add(
                        out=fo_sb,
                        in0=oa_box["oa"].rearrange("p (t e) -> p t e", e=16),
                        in1=PO.rearrange("p (t e) -> p t e", e=16))
                    nc.sync.dma_start(
                        out=fo[b, :].rearrange("(t p e) -> p t e", p=128, e=16),
                        in_=fo_sb)
                return _carry

            # ---- schedule ----
            c1r0 = [SA[:, 0:196], SA[:, 512:708],
                    SB[:, 0:196], SB[:, 512:708]]
            cqr0 = [SA[:, 1024:1073], SA[:, 1136:1185],
                    SB[:, 1024:1073], SB[:, 1136:1185]]
            for blk in range(4):
                conv1_mm(0, blk, pc=c1r0[blk])
            for blk in range(4):
                convq_conv(0, blk, pcq=cqr0[blk])
                conv1_tr(0, blk)
            qt_load(0)
            kproj_h(0, 0, 0)
            kproj_h(0, 0, 1)
            kproj_h(0, 1, 0)
            kproj_h(0, 1, 1)
            ktT_run(0, 0, 3)
            qt_tr(0, 0, 3)
            qt_tr(0, 3, 3, sc=rot_bf)
            qt_tr(0, 6, 2)
            qt_tr(0, 8, 3, sc=rot_bf)
            qt_tr(0, 11, 2)

            carry = None
            for b in range(NB):
                inject = {}
                if b == 0:
                    inject[0] = [lambda: ktT_run(0, 3, 3)]
                    inject[1] = [lambda: ktT_run(0, 6, 3)]
                    inject[2] = [lambda: ktT_run(0, 9, 3)]
                    inject[3] = [lambda: ktT_run(0, 12, 3)]
                    inject[4] = [lambda: ktT_run(0, 15, 1)]
                if b + 1 < NB:
                    for k, v in prep_slots(b + 1).items():
                        inject.setdefault(k, []).extend(v)
                if b >= 1:
                    for k, v in post_slots(b - 1).items():
                        inject.setdefault(k, []).extend(v)
                carry = attention(b, inject, carry)
            carry()
            xt_load(NB - 1)
            convout_blk(NB - 1, 0)
            convout_blk(NB - 1, 1, pco=rot[0][:, 0:49])
            f3_store_half(NB - 1, 0)
            convout_blk(NB - 1, 2)
            convout_blk(NB - 1, 3, pco=rot[0][:, 0:49])
            f3_store_half(NB - 1, 1)
            y_load(NB - 1)
            ln_a(NB - 1)
            ln_b(NB - 1)
            ln_c(NB - 1, dve=True)
            ln_d(NB - 1)
            if DBG:
                nc.sync.dma_start(out=dbg_kt[:, :], in_=kt)
                nc.sync.dma_start(out=dbg_qt[:, :], in_=qt)
                nc.sync.dma_start(out=dbg_ksb[:, :, :, :], in_=k_sb)

    nc.compile()
    return nc


def _blockdiag_pm(w):
    # torch OIHW grouped weights -> partition-major block-diag [128, 9, 4, 128]
    out = np.zeros((128, 9, 4, 128), np.float32)
    for blk in range(4):
        for g in range(8):
            grp = blk * 8 + g
            for ky in range(3):
                for kx in range(3):
                    out[g * 16:(g + 1) * 16, ky * 3 + kx, blk,
                        g * 16:(g + 1) * 16] = w[grp * 16:(grp + 1) * 16,
                                                 :, ky, kx].T
    return out


def kernel(current_pose, next_pose, current_w, next_w, E_proj, rel_embedd,
           ln_gamma, ln_beta, num_iter=None):
    global _PROG
    if _PROG is None:
        _PROG = _build()

    bf = ml_dtypes.bfloat16
    cp_raw = np.ascontiguousarray(
        np.asarray(current_pose, np.float32).transpose(0, 1, 4, 2, 3)
    ).reshape(B, C, H, H)
    cp_img = np.zeros((B, C, 16, 16), np.float32)
    cp_img[:, :, 1:15, 1:15] = cp_raw
    x0_h = np.ascontiguousarray(
        cp_img.reshape(B, 4, 128, 16, 16).transpose(2, 0, 1, 3, 4)).astype(bf)
    qp_raw = np.ascontiguousarray(
        np.asarray(next_pose, np.float32).transpose(0, 1, 4, 2, 3)
    ).reshape(B, C, HO, HO)
    qp_img = np.zeros((B, C, 9, 9), np.float32)
    qp_img[:, :, 1:8, 1:8] = qp_raw
    x0q_h = np.ascontiguousarray(
        qp_img.reshape(B, 4, 128, 9, 9).transpose(2, 0, 1, 3, 4)).astype(bf)

    w1_h = _blockdiag_pm(np.asarray(current_w, np.float32)).astype(bf)
    wq_h = _blockdiag_pm(np.asarray(next_w, np.float32)).astype(bf)
    ep_h = np.ascontiguousarray(
        np.asarray(E_proj, np.float32).reshape(IN_N, 2, 98, HID)
        .transpose(2, 1, 0, 3)).astype(bf)
    rel = np.asarray(rel_embedd, np.float32)
    ident = np.eye(128, dtype=np.float32).astype(bf)

    common = {
        "w1d": w1_h, "wqd": wq_h, "epd": ep_h,
        "rel_k": np.ascontiguousarray(rel.T).astype(np.float32),
        "rel_kt": rel.astype(bf),
        "gam": np.asarray(ln_gamma, np.float32),
        "bet": np.asarray(ln_beta, np.float32),
        "ident": ident,
    }
    core_ids = list(range(8))
    in_maps = []
    for c in core_ids:
        sl = slice(c * NB, (c + 1) * NB)
        in_maps.append({**common,
                        "x0d": np.ascontiguousarray(x0_h[:, sl]),
                        "x0qd": np.ascontiguousarray(x0q_h[:, sl])})

    res = run_bass_kernel_spmd(_PROG, in_maps, core_ids)
    out = np.empty((B, J * 16), np.float32)
    for c in core_ids:
        out[c * NB:(c + 1) * NB] = res.results[c]["outy"][:, :J * 16]
    return out.reshape(B, OUT_N, HO, HO, OUT_D)


if __name__ == "__main__":
    import reference as ref
    inputs = ref.setup_inputs()
    expected = np.asarray(ref.reference(**inputs))
    actual = kernel(**{k: np.asarray(v) if not np.isscalar(v) else v
                       for k, v in inputs.items()})
    err = np.abs(actual - expected)
    sc = np.abs(expected).max()
    print("absmax err:", err.max(), "scale:", sc, "rel:", err.max() / sc)
